# revision 2
# baseline (speedup 1.0000x reference)
"""CFD-GCN Trainium2 kernel: 6-layer GCN on a batched random mesh graph +
KNN interpolation, distributed over 8 NeuronCores (one sample per core pair).

Self-contained: hardcodes all shapes. kernel(**inputs) -> np.ndarray [80000, 3].
"""
import sys

sys.path.insert(0, "/opt/trn_rl_repo")

import numpy as np
import ml_dtypes

from concourse import bass, bacc
from concourse.bass_utils import run_bass_kernel_spmd
import concourse.mybir as mybir
from contextlib import ExitStack

f32, bf16 = mybir.dt.float32, mybir.dt.bfloat16
i16, u16 = mybir.dt.int16, mybir.dt.uint16
ALU = mybir.AluOpType
ACTF = mybir.ActivationFunctionType
bfnp = ml_dtypes.bfloat16

# ---------------- problem constants ----------------
B, NF, NC, H, D_IN, OUT = 4, 20000, 2000, 512, 5, 3
E_PER = 6 * NF
NT = 158                      # node tiles per sample
NPAD = NT * 128               # 20224
NCPAD = 2048                  # padded coarse count
SLICES = 8                    # 128-slot edge slices per dest tile (uniform)
TSLOTS = SLICES * 128         # 1024 slots per tile
ECAP = NT * TSLOTS            # 161792 edge slots per sample
RANGE_T = 16                  # node tiles per hT transpose-load range
ICH = 8                       # interp gather chunk (tiles)
N_CORES = 8
PHASE = 99                    # debug: truncate program after checkpoint N

LAYERS = [
    dict(kc6=True, fy=False, relu=True, e2=False),   # pre0
    dict(kc6=False, fy=False, relu=True, e2=False),  # pre1
    dict(kc6=False, fy=False, relu=True, e2=False),  # pre2
    dict(kc6=False, fy=True, relu=True, e2=False),   # end0
    dict(kc6=False, fy=False, relu=True, e2=False),  # end1
    dict(kc6=False, fy=False, relu=False, e2=True),  # end2
]


def _ranges():
    r, t0 = [], 0
    while t0 < NT:
        r.append((t0, min(RANGE_T, NT - t0)))
        t0 += RANGE_T
    return r


def build_program():
    nc = bacc.Bacc()

    Din = {}
    def din(name, shape, dt):
        Din[name] = nc.declare_dram_parameter(name, list(shape), dt, isOutput=False)
    def dout(name, shape, dt):
        Din[name] = nc.declare_dram_parameter(name, list(shape), dt, isOutput=True)

    din("xT3", (3, NPAD), f32)
    din("cxT3", (3, NCPAD), f32)
    din("negf2", (128, NT), f32)
    din("h0T", (6, NPAD), bf16)
    din("W0", (6, H), bf16)
    din("W1", (128, 4, H), bf16)      # p-major k-chunked
    din("W2", (128, 4, H), bf16)
    din("W3a", (128, 4, H), bf16)
    din("W3b", (3, H), bf16)
    din("W4", (128, 4, H), bf16)
    din("W5", (128, 4, 128), bf16)
    din("brows", (1, 6, H), bf16)
    din("ones1", (1, 128), bf16)
    din("identb", (128, 128), bf16)
    din("sTw", (128, ECAP // 128, 128), bf16)
    din("groww", (128, ECAP // 16), i16)
    din("ctab", (NCPAD, 128), bf16)

    g_d = nc.dram_tensor("g_d", [NPAD, H], bf16)
    fy_d = nc.dram_tensor("fy_d", [3, NPAD], bf16)
    g2_d = nc.dram_tensor("g2_d", [NPAD, 128], bf16)
    h_d = nc.dram_tensor("h_d", [NPAD, H], bf16)
    dout("out_nm", (NPAD, 128), f32)

    es = ExitStack()
    def sb(name, shape, dt):
        return es.enter_context(nc.sbuf_tensor(name, list(shape), dt))
    def psum(name, shape, dt):
        return es.enter_context(nc.psum_tensor(name, list(shape), dt))

    xt_s = [sb(f"xt_s{i}", (3, 128), f32) for i in range(2)]
    cxT3_s = sb("cxT3_s", (3, NCPAD), f32)
    negf2_s = sb("negf2_s", (128, NT), f32)
    h0_s = [sb(f"h0_s{i}", (6, 128), bf16) for i in range(2)]
    W0_s = sb("W0_s", (6, H), bf16)
    W1_s = sb("W1_s", (128, 4, H), bf16)
    W2_s = sb("W2_s", (128, 4, H), bf16)
    W3a_s = sb("W3a_s", (128, 4, H), bf16)
    W3b_s = sb("W3b_s", (3, H), bf16)
    W4_s = sb("W4_s", (128, 4, H), bf16)
    W5_s = sb("W5_s", (128, 4, 128), bf16)
    brows_s = sb("brows_s", (1, 6, H), bf16)
    ones1_s = sb("ones1_s", (1, 128), bf16)
    identb_s = sb("identb_s", (128, 128), bf16)
    gro_s = [sb(f"gro_s{i}", (128, 128), i16) for i in range(2)]

    hT_s = [sb(f"hT_s{i}", (128, 4, RANGE_T * 128), bf16) for i in range(2)]
    gsb_s = [sb(f"gsb_s{i}", (128, H), bf16) for i in range(4)]
    hsb_s = [sb(f"hsb_s{i}", (128, H), bf16) for i in range(4)]
    osb_s = [sb(f"osb_s{i}", (128, 128), f32) for i in range(2)]
    gath_s = [sb(f"gath_s{i}", (128, 16, H), bf16) for i in range(2)]
    gath2_s = [sb(f"gath2_s{i}", (128, 16, 128), bf16) for i in range(2)]
    sT_s = [sb(f"sT_s{i}", (128, 16, 128), bf16) for i in range(2)]

    nd2_s = [sb(f"nd2_s{i}", (128, NCPAD), f32) for i in range(2)]
    bm_s = sb("bm_s", (128, 8, NT), f32)
    bi_s = sb("bi_s", (128, 8, NT), u16)
    d2c_s = sb("d2c_s", (128, 3, NT), f32)
    w_s = sb("w_s", (128, 3, NT), f32)
    wsum_s = sb("wsum_s", (128, NT), f32)
    rs_s = sb("rs_s", (128, NT), f32)
    wnb_s = sb("wnb_s", (128, 3, NT), f32)
    wrap_s = sb("wrap_s", (128, 3, NT, 8), u16)
    gk_s = [[sb(f"gk_s{k}_{i}", (128, ICH, 128), bf16) for i in range(2)]
            for k in range(3)]
    diag_s = [sb(f"diag_s{i}", (128, 128), bf16) for i in range(6)]
    fyw_s = [sb(f"fyw_s{i}", (3, 128), bf16) for i in range(2)]
    fyr_s = [sb(f"fyr_s{i}", (3, 128), bf16) for i in range(2)]

    pz = [psum(f"pz{i}", (128, H), f32) for i in range(2)]
    pa = [psum(f"pa{i}", (128, H), f32) for i in range(2)]

    class Sem:
        def __init__(self, name):
            self.h = es.enter_context(nc.semaphore(name))
            self.n = 0
        def inc(self, k):
            self.n += k
            return (self.h, self.n)
        def now(self):
            return (self.h, self.n)

    class Ring:
        def __init__(self, name, n):
            self.sems = [Sem(f"{name}{i}") for i in range(n)]
            self.nslots = n
        def write(self, slot, k=16):
            s = self.sems[slot % self.nslots]
            return s.inc(k)
        def last(self, slot):
            s = self.sems[slot % self.nslots]
            return (s.h, s.n)
        def all(self):
            return [(s.h, s.n) for s in self.sems]

    def wait_all(engine, ring):
        for sv in ring.all():
            wait(engine, sv)

    s_in = Sem("s_in")
    s_kpe = Sem("s_kpe"); s_kact = Sem("s_kact"); s_kmax = Sem("s_kmax")
    s_wn = Sem("s_wn"); s_wrap = Sem("s_wrap")
    s_dg = Sem("s_dg"); s_ipe = Sem("s_ipe"); s_fy = Sem("s_fy")
    s_dpe = Sem("s_dpe"); s_zpe = Sem("s_zpe")
    s_zact = Sem("s_zact"); s_ape = Sem("s_ape"); s_aact = Sem("s_aact")
    r_gk = None  # created below


    Q = {e: [] for e in ("sync", "tensor", "vector", "scalar", "gpsimd")}
    checkpoints = []
    def checkpoint():
        checkpoints.append({e: len(Q[e]) for e in Q})
    def emit(engine, fn):
        Q[engine].append(fn)
    def wait(engine, semv):
        s, v = semv
        if v > 0:
            emit(engine, lambda e, s=s, v=v: e.wait_ge(s, v))

    r_gk = Ring("r_gk", 2)     # interp table gathers (per gk buf)
    r_xt = Ring("r_xt", 2)     # xT3 tile loads
    r_h0 = Ring("r_h0", 2)     # h0T tile loads
    r_gro = Ring("r_gro", 2)   # gather idx chunk loads
    r_fyw = Ring("r_fyw", 2)   # finey dram writes
    r_fyr = Ring("r_fyr", 2)   # finey tile loads
    r_hT = Ring("r_hT", 2)     # transpose loads (per hT buf)
    r_g = Ring("r_g", 2)       # agg gathers (per gath buf)
    r_s = Ring("r_s", 2)       # S loads (per sT buf)
    r_gw = Ring("r_gw", 4)     # g dram writes (per gsb buf)
    r_hw = Ring("r_hw", 4)     # h dram writes (per hsb buf)
    r_ow = Ring("r_ow", 2)     # out writes (per osb buf)

    # ============ input loads ============
    loads = [
        (cxT3_s[:], "cxT3"), (negf2_s[:], "negf2"),
        (W0_s[:], "W0"), (W1_s[:], "W1"), (W2_s[:], "W2"),
        (W3a_s[:], "W3a"), (W3b_s[:], "W3b"), (W4_s[:], "W4"), (W5_s[:], "W5"),
        (brows_s[:], "brows"), (ones1_s[:], "ones1"), (identb_s[:], "identb"),
    ]
    for dst, srcn in loads:
        sm = s_in.inc(16)
        emit("sync", lambda e, d=dst, s=srcn, sm=sm: e.dma_start(
            out=d, in_=Din[s][:]).then_inc(sm[0], 16))
    IN_ALL = s_in.now()
    checkpoint()   # 0: loads

    # ============ KNN selection ============
    wait("tensor", IN_ALL)
    wait("scalar", IN_ALL)
    wait("vector", IN_ALL)
    NQ = NCPAD // 512
    for t in range(NT):
        if t >= 2:
            wait("sync", (s_kpe.h, NQ * (t - 1)))
        sm = r_xt.write(t)
        emit("sync", lambda e, t=t, sm=sm: e.dma_start(
            out=xt_s[t % 2][:], in_=Din["xT3"][:, t * 128:(t + 1) * 128]
        ).then_inc(sm[0], 16))
        wait("tensor", r_xt.last(t))
        for q in range(NQ):
            gq = NQ * t + q
            if gq >= 2:
                wait("tensor", (s_kact.h, gq - 1))
            sm = s_kpe.inc(1)
            emit("tensor", lambda e, t=t, q=q, gq=gq, sm=sm: e.matmul(
                pz[gq % 2][:, 0:512], xt_s[t % 2][:],
                cxT3_s[:, q * 512:(q + 1) * 512],
                start=True, stop=True).then_inc(sm[0], 1))
        for q in range(NQ):
            gq = NQ * t + q
            wait("scalar", (s_kpe.h, gq + 1))
            if t >= 2 and q == 0:
                wait("scalar", (s_kmax.h, t - 1))
            sm = s_kact.inc(1)
            emit("scalar", lambda e, t=t, q=q, gq=gq, sm=sm: e.activation(
                nd2_s[t % 2][:, q * 512:(q + 1) * 512], pz[gq % 2][:, 0:512],
                ACTF.Identity, bias=negf2_s[:, t:t + 1], scale=1.0
            ).then_inc(sm[0], 1))
        wait("vector", (s_kact.h, NQ * (t + 1)))
        emit("vector", lambda e, t=t: e.max(bm_s[:, :, t], nd2_s[t % 2][:]))
        emit("vector", lambda e: e.drain())
        emit("vector", lambda e, t=t: e.max_index(
            bi_s[:, :, t], bm_s[:, :, t], nd2_s[t % 2][:]))
        sm = s_kmax.inc(1)
        emit("vector", lambda e, sm=sm: e.drain().then_inc(sm[0], 1))

    checkpoint()   # 1: knn select
    # weights on DVE
    emit("vector", lambda e: e.tensor_scalar(
        out=d2c_s[:], in0=bm_s[:, 0:3, :], scalar1=-1.0, scalar2=1e-16,
        op0=ALU.mult, op1=ALU.max))
    emit("vector", lambda e: e.drain())
    emit("vector", lambda e: e.reciprocal(w_s[:], d2c_s[:]))
    emit("vector", lambda e: e.drain())
    emit("vector", lambda e: e.tensor_reduce(
        out=wsum_s[:], in_=bass.AP(w_s, 0, [[3 * NT, 128], [1, NT], [NT, 3]]),
        axis=mybir.AxisListType.X, op=ALU.add))
    emit("vector", lambda e: e.drain())
    emit("vector", lambda e: e.reciprocal(rs_s[:], wsum_s[:]))
    emit("vector", lambda e: e.drain())
    emit("vector", lambda e: e.tensor_tensor(
        out=wnb_s[:], in0=w_s[:],
        in1=bass.AP(rs_s, 0, [[NT, 128], [0, 3], [1, NT]]),
        op=ALU.mult))
    sm = s_wn.inc(1)
    emit("vector", lambda e, sm=sm: e.drain().then_inc(sm[0], 1))

    # wrapped idx build (gpsimd)
    wait("gpsimd", (s_kmax.h, NT))
    for k in range(3):
        for g in range(8):
            sm = s_wrap.inc(16)
            emit("gpsimd", lambda e, k=k, g=g, sm=sm: e.dma_start(
                out=wrap_s[0:16, k, :, g],
                in_=bi_s[16 * g:16 * (g + 1), k, :],
            ).then_inc(sm[0], 16))
    wait("gpsimd", s_wrap.now())
    for rep in range(1, 8):
        sm = s_wrap.inc(16)
        emit("gpsimd", lambda e, rep=rep, sm=sm: e.dma_start(
            out=wrap_s[16 * rep:16 * (rep + 1)],
            in_=wrap_s[0:16],
        ).then_inc(sm[0], 16))
    WRAP_ALL = s_wrap.now()

    # interp
    wait("gpsimd", WRAP_ALL)
    wait("vector", s_wn.now())
    n_ich = (NT + ICH - 1) // ICH
    for c in range(n_ich):
        t0 = c * ICH
        ntile = min(ICH, NT - t0)
        if c >= 2:
            wait("gpsimd", (s_ipe.h, (c - 1) * ICH))
        for k in range(3):
            sm = r_gk.write(c)
            emit("gpsimd", lambda e, k=k, c=c, t0=t0, nt=ntile, sm=sm:
                 e.dma_gather(
                     out_ap=gk_s[k][c % 2][:, 0:nt, :],
                     in_ap=Din["ctab"][:],
                     idxs_ap=wrap_s[:, k, t0:t0 + nt, :].bitcast(i16),
                     num_idxs=nt * 128, num_idxs_reg=nt * 128,
                     elem_size=128,
                 ).then_inc(sm[0], 16))
        GK_NOW = r_gk.last(c)
        for tt in range(ntile):
            t = t0 + tt
            if t >= 2:
                wait("vector", (s_ipe.h, t - 1))
            for k in range(3):
                emit("vector", lambda e, t=t, k=k: e.tensor_scalar(
                    out=diag_s[(3 * t + k) % 6][:], in0=identb_s[:],
                    scalar1=wnb_s[:, k, t:t + 1], scalar2=None, op0=ALU.mult))
            sm = s_dg.inc(3)
            emit("vector", lambda e, sm=sm: e.drain().then_inc(sm[0], 3))
            wait("tensor", GK_NOW)
            wait("tensor", (s_dg.h, s_dg.n))
            wait("tensor", (s_fy.h, t))      # psum WAR (skipped when 0)
            for k in range(3):
                sm = s_ipe.inc(1) if k == 2 else None
                def mk_interp(t=t, tt=tt, k=k, c=c, sm=sm):
                    def f(e):
                        ins = e.matmul(
                            pa[0][:, 0:128], gk_s[k][c % 2][:, tt, :],
                            diag_s[(3 * t + k) % 6][:],
                            start=(k == 0), stop=(k == 2))
                        if sm:
                            ins.then_inc(sm[0], 1)
                    return f
                emit("tensor", mk_interp())
            wait("scalar", (s_ipe.h, s_ipe.n))
            wait("scalar", r_fyw.last(t))
            sm = s_fy.inc(1)
            emit("scalar", lambda e, t=t, sm=sm: e.activation(
                fyw_s[t % 2][:], pa[0][0:3, 0:128],
                ACTF.Copy, bias=0.0, scale=1.0).then_inc(sm[0], 1))
            wait("sync", (s_fy.h, s_fy.n))
            sm = r_fyw.write(t)
            emit("sync", lambda e, t=t, sm=sm: e.dma_start(
                out=fy_d[:, t * 128:(t + 1) * 128],
                in_=fyw_s[t % 2][:]).then_inc(sm[0], 16))
    FY_ALL = s_fy.now()
    KACT_ALL = s_kact.now()
    checkpoint()   # 2: interp

    # ============ GCN layers ============
    WCH = {1: W1_s, 2: W2_s, 3: W3a_s, 4: W4_s, 5: W5_s}

    for li, L in enumerate(LAYERS):
        width = 128 if L["e2"] else H
        gdst = g2_d if L["e2"] else g_d

        # ---------- dense ----------
        zpe_base = s_zpe.n
        zact_base = s_zact.n

        def dense_epilogue(t):
            wait("scalar", (s_zpe.h, zpe_base + t + 1))
            wait("scalar", r_gw.last(t))
            sm = s_zact.inc(1)
            emit("scalar", lambda e, t=t, w=width, sm=sm: e.activation(
                gsb_s[t % 4][:, 0:w], pz[t % 2][:, 0:w], ACTF.Copy,
                bias=0.0, scale=1.0).then_inc(sm[0], 1))
            wait("sync", (s_zact.h, s_zact.n))
            sm = r_gw.write(t)
            emit("sync", lambda e, t=t, gd=gdst, w=width, sm=sm: e.dma_start(
                out=gd[t * 128:(t + 1) * 128, :],
                in_=gsb_s[t % 4][:, 0:w]).then_inc(sm[0], 16))

        if li == 0:
            wait("tensor", KACT_ALL)      # pz WAR vs KNN ACT
            for t in range(NT):
                if t >= 2:
                    wait("sync", (s_zpe.h, zpe_base + t - 1))
                sm = r_h0.write(t)
                emit("sync", lambda e, t=t, sm=sm: e.dma_start(
                    out=h0_s[t % 2][:], in_=Din["h0T"][:, t * 128:(t + 1) * 128]
                ).then_inc(sm[0], 16))
                wait("tensor", r_h0.last(t))
                wait("tensor", (s_zact.h, zact_base if t < 2 else zact_base + t - 1))
                sm = s_zpe.inc(1)
                emit("tensor", lambda e, t=t, sm=sm: e.matmul(
                    pz[t % 2][:, 0:H], h0_s[t % 2][:],
                    W0_s[:], start=True, stop=True).then_inc(sm[0], 1))
                dense_epilogue(t)
        else:
            Wl = WCH[li]
            range_zpe = []
            for ri, (rt0, rnt) in enumerate(_ranges()):
                wait("sync", (s_zpe.h,
                              zpe_base if ri < 2 else range_zpe[ri - 2]))
                for cch in range(4):
                    sm = r_hT.write(ri)
                    emit("sync", lambda e, ri=ri, rt0=rt0, rnt=rnt, c=cch, sm=sm:
                         e.dma_start_transpose(
                             hT_s[ri % 2][:, c, 0:rnt * 128],
                             h_d[rt0 * 128:(rt0 + rnt) * 128,
                                 c * 128:(c + 1) * 128],
                         ).then_inc(sm[0], 16))
                wait("tensor", r_hT.last(ri))
                if li == 3 and ri == 0:
                    wait_all("sync", r_fyw)
                for tt in range(rnt):
                    t = rt0 + tt
                    range_last = (tt == rnt - 1)
                    if L["fy"]:
                        if t >= 2:
                            wait("sync", (s_zpe.h, zpe_base + t - 1))
                        sm = r_fyr.write(t)
                        emit("sync", lambda e, t=t, sm=sm: e.dma_start(
                            out=fyr_s[t % 2][:],
                            in_=fy_d[:, t * 128:(t + 1) * 128]
                        ).then_inc(sm[0], 16))
                    wait("tensor", (s_zact.h,
                                    zact_base if t < 2 else zact_base + t - 1))
                    for cch in range(4):
                        last = (cch == 3) and not L["fy"]
                        sm = s_zpe.inc(1) if last else None
                        def mk_dense(t=t, tt=tt, ri=ri, cch=cch, Wl=Wl,
                                     w=width, last=last, sm=sm):
                            def f(e):
                                ins = e.matmul(
                                    pz[t % 2][:, 0:w],
                                    hT_s[ri % 2][:, cch,
                                                 tt * 128:(tt + 1) * 128],
                                    Wl[:, cch, 0:w],
                                    start=(cch == 0), stop=last)
                                if sm:
                                    ins.then_inc(sm[0], 1)
                            return f
                        emit("tensor", mk_dense())
                    if L["fy"]:
                        wait("tensor", r_fyr.last(t))
                        sm = s_zpe.inc(1)
                        emit("tensor", lambda e, t=t, sm=sm: e.matmul(
                            pz[t % 2][:, 0:H],
                            fyr_s[t % 2][:],
                            W3b_s[:], start=False, stop=True).then_inc(sm[0], 1))
                    if range_last:
                        range_zpe.append(s_zpe.n)
                    dense_epilogue(t)
        checkpoint()   # dense of this layer done
        # ---------- agg ----------
        gbufs = gath2_s if L["e2"] else gath_s
        ape_base = s_ape.n
        aact_base = s_aact.n
        wait_all("gpsimd", r_gw)
        if li == 0:
            wait("tensor", (s_fy.h, NT))   # pa WAR vs interp
        for c in range(NT // 2):
            wait("gpsimd", (s_ape.h,
                            ape_base if c < 2 else ape_base + 2 * (c - 1)))
            wait("sync", r_g.last(c))
            sm = r_gro.write(c)
            emit("sync", lambda e, c=c, sm=sm: e.dma_start(
                out=gro_s[c % 2][:],
                in_=Din["groww"][:, c * 128:(c + 1) * 128]).then_inc(sm[0], 16))
            wait("gpsimd", r_gro.last(c))
            sm = r_g.write(c)
            emit("gpsimd", lambda e, c=c, gd=gdst, gb=gbufs, w=width, sm=sm:
                 e.dma_gather(
                     out_ap=gb[c % 2][:, :, 0:w],
                     in_ap=gd[:],
                     idxs_ap=gro_s[c % 2][:],
                     num_idxs=2048, num_idxs_reg=2048, elem_size=w,
                     single_packet=False,
                 ).then_inc(sm[0], 16))
            wait("sync", (s_ape.h,
                          ape_base if c < 2 else ape_base + 2 * (c - 1)))
            sm = r_s.write(c)
            emit("sync", lambda e, c=c, sm=sm: e.dma_start(
                out=sT_s[c % 2][:],
                in_=Din["sTw"][:, c * 16:(c + 1) * 16, :]).then_inc(sm[0], 16))
            wait("tensor", r_g.last(c))
            wait("tensor", r_s.last(c))
            for tt in range(2):
                t = 2 * c + tt
                wait("tensor", (s_aact.h,
                                aact_base if t < 2 else aact_base + t - 1))
                for sl in range(SLICES):
                    emit("tensor", lambda e, c=c, tt=tt, t=t, sl=sl, gb=gbufs,
                         w=width: e.matmul(
                        pa[t % 2][:, 0:w],
                        sT_s[c % 2][:, tt * 8 + sl, :],
                        gb[c % 2][:, tt * 8 + sl, 0:w],
                        start=(sl == 0), stop=False))
                sm = s_ape.inc(1)
                emit("tensor", lambda e, t=t, li=li, w=width, sm=sm: e.matmul(
                    pa[t % 2][:, 0:w], ones1_s[:],
                    brows_s[:, li, 0:w], start=False, stop=True
                ).then_inc(sm[0], 1))
                wait("scalar", (s_ape.h, s_ape.n))
                if L["e2"]:
                    wait("scalar", r_ow.last(t))
                else:
                    wait("scalar", r_hw.last(t))
                sm = s_aact.inc(1)
                if L["e2"]:
                    emit("scalar", lambda e, t=t, sm=sm: e.activation(
                        osb_s[t % 2][:], pa[t % 2][:, 0:128], ACTF.Copy,
                        bias=0.0, scale=1.0).then_inc(sm[0], 1))
                else:
                    emit("scalar", lambda e, t=t, sm=sm: e.activation(
                        hsb_s[t % 4][:], pa[t % 2][:, 0:H], ACTF.Relu,
                        bias=0.0, scale=1.0).then_inc(sm[0], 1))
                wait("sync", (s_aact.h, s_aact.n))
                if L["e2"]:
                    sm = r_ow.write(t)
                    emit("sync", lambda e, t=t, sm=sm: e.dma_start(
                        out=Din["out_nm"][t * 128:(t + 1) * 128, :],
                        in_=osb_s[t % 2][:]).then_inc(sm[0], 16))
                else:
                    sm = r_hw.write(t)
                    emit("sync", lambda e, t=t, sm=sm: e.dma_start(
                        out=h_d[t * 128:(t + 1) * 128, :],
                        in_=hsb_s[t % 4][:]).then_inc(sm[0], 16))
        if not L["e2"]:
            wait_all("sync", r_hw)   # barrier before next layer's hT loads
        checkpoint()   # 3+li

    wait_all("sync", r_ow)
    wait_all("sync", r_hw)
    checkpoint()
    if PHASE < len(checkpoints):
        cut = checkpoints[PHASE]
        for e in Q:
            Q[e] = Q[e][:cut[e]]

    with nc.allow_non_contiguous_dma(reason="wrapped idx build"), \
            nc.Block() as block:
        @block.sync
        def _(e):
            for fn in Q["sync"]:
                fn(e)

        @block.tensor
        def _(e):
            for fn in Q["tensor"]:
                fn(e)

        @block.vector
        def _(e):
            for fn in Q["vector"]:
                fn(e)

        @block.scalar
        def _(e):
            for fn in Q["scalar"]:
                fn(e)

        @block.gpsimd
        def _(e):
            for fn in Q["gpsimd"]:
                fn(e)

    nc.finalize()
    return nc


# ================= host side =================

def host_prep(inputs):
    x = np.asarray(inputs["x"], np.float32)
    sdf = np.asarray(inputs["sdf"], np.float32)
    edge_index = np.asarray(inputs["edge_index"], np.int64)
    coarse_x = np.asarray(inputs["coarse_x"], np.float32)
    coarse_y = np.asarray(inputs["coarse_y"], np.float32)
    Ws = {k: np.asarray(inputs[k], np.float32) for k in (
        "pre_W0", "pre_W1", "pre_W2", "end_W0", "end_W1", "end_W2")}
    bs = {k: np.asarray(inputs[k], np.float32) for k in (
        "pre_b0", "pre_b1", "pre_b2", "end_b0", "end_b1", "end_b2")}

    cxT3 = np.zeros((3, NCPAD), np.float32)
    cxT3[0, :NC] = 2 * coarse_x[:, 0]
    cxT3[1, :NC] = 2 * coarse_x[:, 1]
    cxT3[2, :NC] = -(coarse_x[:, 0] ** 2 + coarse_x[:, 1] ** 2)
    cxT3[0, NC:] = 2e4; cxT3[1, NC:] = 2e4; cxT3[2, NC:] = -2e8

    brows = np.zeros((6, H), np.float32)
    for i, k in enumerate(("pre_b0", "pre_b1", "pre_b2", "end_b0", "end_b1")):
        brows[i] = bs[k]
    brows[5, :OUT] = bs["end_b2"]

    W5 = np.zeros((H, 128), np.float32)
    W5[:, :OUT] = Ws["end_W2"]

    def pmaj(w):   # [512, X] -> [128, 4, X]
        return np.ascontiguousarray(
            w.reshape(4, 128, w.shape[1]).transpose(1, 0, 2))

    common = dict(
        cxT3=cxT3,
        W0=Ws["pre_W0"].astype(bfnp),
        W1=pmaj(Ws["pre_W1"]).astype(bfnp),
        W2=pmaj(Ws["pre_W2"]).astype(bfnp),
        W3a=pmaj(Ws["end_W0"][OUT:]).astype(bfnp),
        W3b=Ws["end_W0"][:OUT].astype(bfnp),
        W4=pmaj(Ws["end_W1"]).astype(bfnp),
        W5=pmaj(W5).astype(bfnp),
        brows=brows.astype(bfnp)[None],
        ones1=np.ones((1, 128), bfnp),
        identb=np.eye(128, dtype=np.float32).astype(bfnp),
    )

    in_maps, metas = [], []
    for s in range(B):
        xs = x[s * NF:(s + 1) * NF]
        e = edge_index[:, s * E_PER:(s + 1) * E_PER] - s * NF
        cy = coarse_y[s * NC:(s + 1) * NC]

        deg = np.bincount(e[1], minlength=NF).astype(np.float32) + 1.0
        dinv = (1.0 / np.sqrt(deg)).astype(np.float32)

        # balanced tile assignment (snake over degree-sorted nodes)
        order = np.argsort(-deg, kind="stable")
        tile_seq = np.arange(NT)
        snake = np.concatenate([tile_seq, tile_seq[::-1]])
        bins = np.resize(snake, NF)
        nid = np.empty(NF, np.int64)
        for t in range(NT):
            sel = np.where(bins == t)[0]
            nid[order[sel]] = t * 128 + np.arange(len(sel))

        dinv_new = np.ones(NPAD, np.float32)
        dinv_new[nid] = dinv

        allrow = np.concatenate([nid[e[0]], np.arange(NPAD)])
        allcol = np.concatenate([nid[e[1]], np.arange(NPAD)])
        wts = dinv_new[allrow] * dinv_new[allcol]

        o = np.argsort(allcol, kind="stable")
        allrow, allcol, wts = allrow[o], allcol[o], wts[o]
        tile_of = allcol // 128
        tstart = np.searchsorted(tile_of, np.arange(NT))
        cnts = np.searchsorted(tile_of, np.arange(NT), side="right") - tstart
        assert cnts.max() <= TSLOTS, f"tile overflow {cnts.max()}"

        rank = np.arange(len(allcol)) - np.repeat(tstart, cnts)
        srow = np.zeros((NT, TSLOTS), np.int16)
        srow[tile_of, rank] = allrow.astype(np.int16)
        sT = np.zeros((NT, TSLOTS, 128), np.float32)
        sT[tile_of, rank, allcol % 128] = wts
        sT = sT.reshape(ECAP, 128)
        sTw = np.ascontiguousarray(
            sT.reshape(ECAP // 128, 128, 128).transpose(1, 0, 2)).astype(bfnp)

        grow = srow.reshape(ECAP)
        tmp = np.ascontiguousarray(grow.reshape(ECAP // 16, 16).T)
        groww = np.ascontiguousarray(np.tile(tmp, (8, 1)).astype(np.int16))

        f01 = np.full((NPAD, 2), 1e3, np.float32)
        f01[nid] = xs[:, 0:2]
        xT3 = np.ones((3, NPAD), np.float32)
        xT3[0] = f01[:, 0]; xT3[1] = f01[:, 1]
        negf2 = np.ascontiguousarray(
            (-(f01[:, 0] ** 2 + f01[:, 1] ** 2)).reshape(NT, 128).T)

        h0 = np.zeros((NPAD, 6), np.float32)
        h0[nid, 0:D_IN] = xs
        h0[nid, D_IN] = sdf[:, 0]
        h0T = np.ascontiguousarray(h0.T).astype(bfnp)

        ctab = np.zeros((NCPAD, 128), np.float32)
        ctab[:NC, 0:OUT] = cy
        ctab = ctab.astype(bfnp)

        m = dict(common)
        m.update(xT3=xT3, negf2=negf2, h0T=h0T, sTw=sTw, groww=groww, ctab=ctab)
        in_maps.append(m)
        metas.append(nid)

    full_maps = [in_maps[c // 2] for c in range(N_CORES)]
    return full_maps, metas


_prog_cache = {}


def kernel(**inputs):
    if "nc" not in _prog_cache:
        _prog_cache["nc"] = build_program()
    nc = _prog_cache["nc"]

    in_maps, metas = host_prep(inputs)
    res = run_bass_kernel_spmd(nc, in_maps, list(range(N_CORES)))
    global _last_exec_ns, _last_trace
    _last_exec_ns = res.exec_time_ns
    _last_trace = res.instructions_and_trace

    out = np.empty((B * NF, OUT), np.float32)
    for s in range(B):
        o = np.asarray(res.results[2 * s]["out_nm"])
        out[s * NF:(s + 1) * NF] = o[metas[s], 0:OUT]
    return out



# revision 6
# speedup vs baseline: 1.1380x; 1.1380x over previous
"""CFD-GCN Trainium2 kernel: 6-layer GCN on a batched random mesh graph +
KNN interpolation, distributed over 8 NeuronCores (one sample per core pair).

Self-contained: hardcodes all shapes. kernel(**inputs) -> np.ndarray [80000, 3].
"""
import sys

sys.path.insert(0, "/opt/trn_rl_repo")

import numpy as np
import ml_dtypes

from concourse import bass, bacc
from concourse.bass_utils import run_bass_kernel_spmd
import concourse.mybir as mybir
from contextlib import ExitStack

f32, bf16 = mybir.dt.float32, mybir.dt.bfloat16
i16, u16 = mybir.dt.int16, mybir.dt.uint16
ALU = mybir.AluOpType
ACTF = mybir.ActivationFunctionType
bfnp = ml_dtypes.bfloat16

# ---------------- problem constants ----------------
B, NF, NC, H, D_IN, OUT = 4, 20000, 2000, 512, 5, 3
E_PER = 6 * NF
NT = 158                      # node tiles per sample
NPAD = NT * 128               # 20224
NCPAD = 2048                  # padded coarse count
SLICES = 8                    # 128-slot edge slices per dest tile (uniform)
TSLOTS = SLICES * 128         # 1024 slots per tile
ECAP = NT * TSLOTS            # 161792 edge slots per sample
RANGE_T = 16                  # node tiles per hT transpose-load range
ICH = 8                       # interp gather chunk (tiles)
N_CORES = 8
PHASE = 99                    # debug: truncate program after checkpoint N

LAYERS = [
    dict(kc6=True, fy=False, relu=True, e2=False),   # pre0
    dict(kc6=False, fy=False, relu=True, e2=False),  # pre1
    dict(kc6=False, fy=False, relu=True, e2=False),  # pre2
    dict(kc6=False, fy=True, relu=True, e2=False),   # end0
    dict(kc6=False, fy=False, relu=True, e2=False),  # end1
    dict(kc6=False, fy=False, relu=False, e2=True),  # end2
]


def _ranges():
    r, t0 = [], 0
    while t0 < NT:
        r.append((t0, min(RANGE_T, NT - t0)))
        t0 += RANGE_T
    return r


def build_program():
    nc = bacc.Bacc(num_swdge_queues=2)

    Din = {}
    def din(name, shape, dt):
        Din[name] = nc.declare_dram_parameter(name, list(shape), dt, isOutput=False)
    def dout(name, shape, dt):
        Din[name] = nc.declare_dram_parameter(name, list(shape), dt, isOutput=True)

    din("xT3", (3, NPAD), f32)
    din("cxT3", (3, NCPAD), f32)
    din("negf2", (128, NT), f32)
    din("h0T", (6, NPAD), bf16)
    din("W0", (6, H), bf16)
    din("W1", (128, 4, H), bf16)      # p-major k-chunked
    din("W2", (128, 4, H), bf16)
    din("W3a", (128, 4, H), bf16)
    din("W3b", (3, H), bf16)
    din("W4", (128, 4, H), bf16)
    din("W5", (128, 4, 128), bf16)
    din("brows", (1, 6, H), bf16)
    din("ones1", (1, 128), bf16)
    din("identb", (128, 128), bf16)
    din("sTw", (128, ECAP // 128, 128), bf16)
    din("groww", (128, ECAP // 16), i16)
    din("ctab", (NCPAD, 128), bf16)

    g_d = nc.dram_tensor("g_d", [NPAD, H], bf16)
    fy_d = nc.dram_tensor("fy_d", [3, NPAD], bf16)
    g2_d = nc.dram_tensor("g2_d", [NPAD, 128], bf16)
    h_d = nc.dram_tensor("h_d", [NPAD, H], bf16)
    dout("out_nm", (NPAD, 128), f32)

    es = ExitStack()
    def sb(name, shape, dt):
        return es.enter_context(nc.sbuf_tensor(name, list(shape), dt))
    def psum(name, shape, dt):
        return es.enter_context(nc.psum_tensor(name, list(shape), dt))

    xt_s = [sb(f"xt_s{i}", (3, 128), f32) for i in range(2)]
    cxT3_s = sb("cxT3_s", (3, NCPAD), f32)
    negf2_s = sb("negf2_s", (128, NT), f32)
    h0_s = [sb(f"h0_s{i}", (6, 128), bf16) for i in range(2)]
    W0_s = sb("W0_s", (6, H), bf16)
    W1_s = sb("W1_s", (128, 4, H), bf16)
    W2_s = sb("W2_s", (128, 4, H), bf16)
    W3a_s = sb("W3a_s", (128, 4, H), bf16)
    W3b_s = sb("W3b_s", (3, H), bf16)
    W4_s = sb("W4_s", (128, 4, H), bf16)
    W5_s = sb("W5_s", (128, 4, 128), bf16)
    brows_s = sb("brows_s", (1, 6, H), bf16)
    ones1_s = sb("ones1_s", (1, 128), bf16)
    identb_s = sb("identb_s", (128, 128), bf16)
    gro_s = [sb(f"gro_s{i}", (128, 128), i16) for i in range(2)]

    hT_s = [sb(f"hT_s{i}", (128, 4, RANGE_T * 128), bf16) for i in range(2)]
    gsb_s = [sb(f"gsb_s{i}", (128, H), bf16) for i in range(4)]
    hsb_s = [sb(f"hsb_s{i}", (128, H), bf16) for i in range(4)]
    osb_s = [sb(f"osb_s{i}", (128, 128), f32) for i in range(2)]
    gath_s = [sb(f"gath_s{i}", (128, 16, H), bf16) for i in range(2)]
    gath2_s = [sb(f"gath2_s{i}", (128, 16, 128), bf16) for i in range(2)]
    sT_s = [sb(f"sT_s{i}", (128, 16, 128), bf16) for i in range(2)]

    nd2_s = [sb(f"nd2_s{i}", (128, NCPAD), f32) for i in range(2)]
    bm_s = sb("bm_s", (128, 8, NT), f32)
    bi_s = sb("bi_s", (128, 8, NT), u16)
    d2c_s = sb("d2c_s", (128, 3, NT), f32)
    w_s = sb("w_s", (128, 3, NT), f32)
    wsum_s = sb("wsum_s", (128, NT), f32)
    rs_s = sb("rs_s", (128, NT), f32)
    wnb_s = sb("wnb_s", (128, 3, NT), f32)
    wrap_s = sb("wrap_s", (128, 3, NT, 8), u16)
    gk_s = [[sb(f"gk_s{k}_{i}", (128, ICH, 128), bf16) for i in range(2)]
            for k in range(3)]
    diag_s = [sb(f"diag_s{i}", (128, 128), bf16) for i in range(6)]
    fyw_s = [sb(f"fyw_s{i}", (3, 128), bf16) for i in range(2)]
    fyr_s = [sb(f"fyr_s{i}", (3, 128), bf16) for i in range(2)]

    pz = [psum(f"pz{i}", (128, H), f32) for i in range(2)]
    pa = [psum(f"pa{i}", (128, H), f32) for i in range(2)]

    class Sem:
        def __init__(self, name):
            self.h = es.enter_context(nc.semaphore(name))
            self.n = 0
        def inc(self, k):
            self.n += k
            return (self.h, self.n)
        def now(self):
            return (self.h, self.n)

    class Ring:
        def __init__(self, name, n):
            self.sems = [Sem(f"{name}{i}") for i in range(n)]
            self.nslots = n
        def write(self, slot, k=16):
            s = self.sems[slot % self.nslots]
            return s.inc(k)
        def last(self, slot):
            s = self.sems[slot % self.nslots]
            return (s.h, s.n)
        def all(self):
            return [(s.h, s.n) for s in self.sems]

    def wait_all(engine, ring):
        for sv in ring.all():
            wait(engine, sv)

    s_in = Sem("s_in")
    s_gprep = Sem("s_gprep")
    s_iprep = Sem("s_iprep")
    s_kpe = Sem("s_kpe"); s_kact = Sem("s_kact"); s_kmax = Sem("s_kmax")
    s_wn = Sem("s_wn"); s_wrap = Sem("s_wrap")
    s_dg = Sem("s_dg"); s_ipe = Sem("s_ipe"); s_fy = Sem("s_fy")
    s_dpe = Sem("s_dpe"); s_zpe = Sem("s_zpe")
    s_zact = Sem("s_zact"); s_ape = Sem("s_ape"); s_aact = Sem("s_aact")
    r_gk = None  # created below


    Q = {e: [] for e in ("sync", "tensor", "vector", "scalar", "gpsimd")}
    checkpoints = []
    def checkpoint():
        checkpoints.append({e: len(Q[e]) for e in Q})
    def emit(engine, fn):
        Q[engine].append(fn)
    def wait(engine, semv):
        s, v = semv
        if v > 0:
            emit(engine, lambda e, s=s, v=v: e.wait_ge(s, v))

    r_gk = Ring("r_gk", 2)     # interp table gathers (per gk buf)
    r_xt = Ring("r_xt", 2)     # xT3 tile loads
    r_h0 = Ring("r_h0", 2)     # h0T tile loads
    r_gro = Ring("r_gro", 2)   # gather idx chunk loads
    r_fyw = Ring("r_fyw", 2)   # finey dram writes
    r_fyr = Ring("r_fyr", 2)   # finey tile loads
    r_hT = Ring("r_hT", 2)     # transpose loads (per hT buf)
    r_g = Ring("r_g", 2)       # agg gathers (per gath buf)
    r_s = Ring("r_s", 2)       # S loads (per sT buf)
    r_gw = Ring("r_gw", 4)     # g dram writes (per gsb buf)
    r_hw = Ring("r_hw", 4)     # h dram writes (per hsb buf)
    r_ow = Ring("r_ow", 2)     # out writes (per osb buf)

    # ============ input loads ============
    loads = [
        (cxT3_s[:], "cxT3"), (negf2_s[:], "negf2"),
        (W0_s[:], "W0"), (W1_s[:], "W1"), (W2_s[:], "W2"),
        (W3a_s[:], "W3a"), (W3b_s[:], "W3b"), (W4_s[:], "W4"), (W5_s[:], "W5"),
        (brows_s[:], "brows"), (ones1_s[:], "ones1"), (identb_s[:], "identb"),
    ]
    for dst, srcn in loads:
        sm = s_in.inc(16)
        emit("sync", lambda e, d=dst, s=srcn, sm=sm: e.dma_start(
            out=d, in_=Din[s][:]).then_inc(sm[0], 16))
    IN_ALL = s_in.now()
    checkpoint()   # 0: loads

    # ============ KNN selection ============
    wait("tensor", IN_ALL)
    wait("scalar", IN_ALL)
    wait("vector", IN_ALL)
    NQ = NCPAD // 512
    for t in range(NT):
        if t >= 2:
            wait("sync", (s_kpe.h, NQ * (t - 1)))
        sm = r_xt.write(t)
        emit("sync", lambda e, t=t, sm=sm: e.dma_start(
            out=xt_s[t % 2][:], in_=Din["xT3"][:, t * 128:(t + 1) * 128]
        ).then_inc(sm[0], 16))
        wait("tensor", r_xt.last(t))
        for q in range(NQ):
            gq = NQ * t + q
            if gq >= 2:
                wait("tensor", (s_kact.h, gq - 1))
            sm = s_kpe.inc(1)
            emit("tensor", lambda e, t=t, q=q, gq=gq, sm=sm: e.matmul(
                pz[gq % 2][:, 0:512], xt_s[t % 2][:],
                cxT3_s[:, q * 512:(q + 1) * 512],
                start=True, stop=True).then_inc(sm[0], 1))
        for q in range(NQ):
            gq = NQ * t + q
            wait("scalar", (s_kpe.h, gq + 1))
            if t >= 2 and q == 0:
                wait("scalar", (s_kmax.h, t - 1))
            sm = s_kact.inc(1)
            emit("scalar", lambda e, t=t, q=q, gq=gq, sm=sm: e.activation(
                nd2_s[t % 2][:, q * 512:(q + 1) * 512], pz[gq % 2][:, 0:512],
                ACTF.Identity, bias=negf2_s[:, t:t + 1], scale=1.0
            ).then_inc(sm[0], 1))
        wait("vector", (s_kact.h, NQ * (t + 1)))
        emit("vector", lambda e, t=t: e.max(bm_s[:, :, t], nd2_s[t % 2][:]))
        emit("vector", lambda e: e.drain())
        emit("vector", lambda e, t=t: e.max_index(
            bi_s[:, :, t], bm_s[:, :, t], nd2_s[t % 2][:]))
        sm = s_kmax.inc(1)
        emit("vector", lambda e, sm=sm: e.drain().then_inc(sm[0], 1))

    checkpoint()   # 1: knn select
    # weights on DVE
    emit("vector", lambda e: e.tensor_scalar(
        out=d2c_s[:], in0=bm_s[:, 0:3, :], scalar1=-1.0, scalar2=1e-16,
        op0=ALU.mult, op1=ALU.max))
    emit("vector", lambda e: e.drain())
    emit("vector", lambda e: e.reciprocal(w_s[:], d2c_s[:]))
    emit("vector", lambda e: e.drain())
    emit("vector", lambda e: e.tensor_reduce(
        out=wsum_s[:], in_=bass.AP(w_s, 0, [[3 * NT, 128], [1, NT], [NT, 3]]),
        axis=mybir.AxisListType.X, op=ALU.add))
    emit("vector", lambda e: e.drain())
    emit("vector", lambda e: e.reciprocal(rs_s[:], wsum_s[:]))
    emit("vector", lambda e: e.drain())
    emit("vector", lambda e: e.tensor_tensor(
        out=wnb_s[:], in0=w_s[:],
        in1=bass.AP(rs_s, 0, [[NT, 128], [0, 3], [1, NT]]),
        op=ALU.mult))
    sm = s_wn.inc(1)
    emit("vector", lambda e, sm=sm: e.drain().then_inc(sm[0], 1))

    # wrapped idx build (gpsimd)
    wait("gpsimd", (s_kmax.h, NT))
    for k in range(3):
        for g in range(8):
            sm = s_wrap.inc(16)
            emit("gpsimd", lambda e, k=k, g=g, sm=sm: e.dma_start(
                out=wrap_s[0:16, k, :, g],
                in_=bi_s[16 * g:16 * (g + 1), k, :],
            ).then_inc(sm[0], 16))
    wait("gpsimd", s_wrap.now())
    for rep in range(1, 8):
        sm = s_wrap.inc(16)
        emit("gpsimd", lambda e, rep=rep, sm=sm: e.dma_start(
            out=wrap_s[16 * rep:16 * (rep + 1)],
            in_=wrap_s[0:16],
        ).then_inc(sm[0], 16))
    WRAP_ALL = s_wrap.now()

    # interp
    wait("gpsimd", WRAP_ALL)
    wait("vector", s_wn.now())
    n_ich = (NT + ICH - 1) // ICH
    for c in range(n_ich):
        t0 = c * ICH
        ntile = min(ICH, NT - t0)
        for k in range(3):
            sm = r_gk.write(c)
            smp = s_iprep.inc(1)
            emit("gpsimd", lambda e, k=k, c=c, t0=t0, nt=ntile, sm=sm, smp=smp:
                 e.dma_gather(
                     out_ap=gk_s[k][c % 2][:, 0:nt, :],
                     in_ap=Din["ctab"][:],
                     idxs_ap=wrap_s[:, k, t0:t0 + nt, :].bitcast(i16),
                     num_idxs=nt * 128, num_idxs_reg=nt * 128,
                     elem_size=128,
                     prepare_only=True, sem=r_gk.sems[c % 2].h,
                     queue_num=c % 2,
                 ).then_inc(smp[0], 1))
        wait("gpsimd", s_iprep.now())
        if c >= 2:
            wait("gpsimd", (s_ipe.h, (c - 1) * ICH))
        emit("gpsimd", lambda e, c=c: e.trigger_dma(
            count=3, queue_num=c % 2))
        GK_NOW = r_gk.last(c)
        for tt in range(ntile):
            t = t0 + tt
            if t >= 2:
                wait("vector", (s_ipe.h, t - 1))
            for k in range(3):
                emit("vector", lambda e, t=t, k=k: e.tensor_scalar(
                    out=diag_s[(3 * t + k) % 6][:], in0=identb_s[:],
                    scalar1=wnb_s[:, k, t:t + 1], scalar2=None, op0=ALU.mult))
            sm = s_dg.inc(3)
            emit("vector", lambda e, sm=sm: e.drain().then_inc(sm[0], 3))
            wait("tensor", GK_NOW)
            wait("tensor", (s_dg.h, s_dg.n))
            wait("tensor", (s_fy.h, t))      # psum WAR (skipped when 0)
            for k in range(3):
                sm = s_ipe.inc(1) if k == 2 else None
                def mk_interp(t=t, tt=tt, k=k, c=c, sm=sm):
                    def f(e):
                        ins = e.matmul(
                            pa[0][:, 0:128], gk_s[k][c % 2][:, tt, :],
                            diag_s[(3 * t + k) % 6][:],
                            start=(k == 0), stop=(k == 2))
                        if sm:
                            ins.then_inc(sm[0], 1)
                    return f
                emit("tensor", mk_interp())
            wait("scalar", (s_ipe.h, s_ipe.n))
            wait("scalar", r_fyw.last(t))
            sm = s_fy.inc(1)
            emit("scalar", lambda e, t=t, sm=sm: e.activation(
                fyw_s[t % 2][:], pa[0][0:3, 0:128],
                ACTF.Copy, bias=0.0, scale=1.0).then_inc(sm[0], 1))
            wait("sync", (s_fy.h, s_fy.n))
            sm = r_fyw.write(t)
            emit("sync", lambda e, t=t, sm=sm: e.dma_start(
                out=fy_d[:, t * 128:(t + 1) * 128],
                in_=fyw_s[t % 2][:]).then_inc(sm[0], 16))
    FY_ALL = s_fy.now()
    KACT_ALL = s_kact.now()
    checkpoint()   # 2: interp

    # ============ GCN layers ============
    WCH = {1: W1_s, 2: W2_s, 3: W3a_s, 4: W4_s, 5: W5_s}

    for li, L in enumerate(LAYERS):
        width = 128 if L["e2"] else H
        gdst = g2_d if L["e2"] else g_d

        # ---------- dense ----------
        zpe_base = s_zpe.n
        zact_base = s_zact.n

        def dense_epilogue(t):
            wait("scalar", (s_zpe.h, zpe_base + t + 1))
            wait("scalar", r_gw.last(t))
            sm = s_zact.inc(1)
            emit("scalar", lambda e, t=t, w=width, sm=sm: e.activation(
                gsb_s[t % 4][:, 0:w], pz[t % 2][:, 0:w], ACTF.Copy,
                bias=0.0, scale=1.0).then_inc(sm[0], 1))
            wait("sync", (s_zact.h, s_zact.n))
            sm = r_gw.write(t)
            emit("sync", lambda e, t=t, gd=gdst, w=width, sm=sm: e.dma_start(
                out=gd[t * 128:(t + 1) * 128, :],
                in_=gsb_s[t % 4][:, 0:w]).then_inc(sm[0], 16))

        if li == 0:
            wait("tensor", KACT_ALL)      # pz WAR vs KNN ACT
            for t in range(NT):
                if t >= 2:
                    wait("sync", (s_zpe.h, zpe_base + t - 1))
                sm = r_h0.write(t)
                emit("sync", lambda e, t=t, sm=sm: e.dma_start(
                    out=h0_s[t % 2][:], in_=Din["h0T"][:, t * 128:(t + 1) * 128]
                ).then_inc(sm[0], 16))
                wait("tensor", r_h0.last(t))
                wait("tensor", (s_zact.h, zact_base if t < 2 else zact_base + t - 1))
                sm = s_zpe.inc(1)
                emit("tensor", lambda e, t=t, sm=sm: e.matmul(
                    pz[t % 2][:, 0:H], h0_s[t % 2][:],
                    W0_s[:], start=True, stop=True).then_inc(sm[0], 1))
                dense_epilogue(t)
        else:
            Wl = WCH[li]
            range_zpe = []
            for ri, (rt0, rnt) in enumerate(_ranges()):
                wait("sync", (s_zpe.h,
                              zpe_base if ri < 2 else range_zpe[ri - 2]))
                for cch in range(4):
                    sm = r_hT.write(ri)
                    emit("sync", lambda e, ri=ri, rt0=rt0, rnt=rnt, c=cch, sm=sm:
                         e.dma_start_transpose(
                             hT_s[ri % 2][:, c, 0:rnt * 128],
                             h_d[rt0 * 128:(rt0 + rnt) * 128,
                                 c * 128:(c + 1) * 128],
                         ).then_inc(sm[0], 16))
                wait("tensor", r_hT.last(ri))
                if li == 3 and ri == 0:
                    wait_all("sync", r_fyw)
                for tt in range(rnt):
                    t = rt0 + tt
                    range_last = (tt == rnt - 1)
                    if L["fy"]:
                        if t >= 2:
                            wait("sync", (s_zpe.h, zpe_base + t - 1))
                        sm = r_fyr.write(t)
                        emit("sync", lambda e, t=t, sm=sm: e.dma_start(
                            out=fyr_s[t % 2][:],
                            in_=fy_d[:, t * 128:(t + 1) * 128]
                        ).then_inc(sm[0], 16))
                    wait("tensor", (s_zact.h,
                                    zact_base if t < 2 else zact_base + t - 1))
                    for cch in range(4):
                        last = (cch == 3) and not L["fy"]
                        sm = s_zpe.inc(1) if last else None
                        def mk_dense(t=t, tt=tt, ri=ri, cch=cch, Wl=Wl,
                                     w=width, last=last, sm=sm):
                            def f(e):
                                ins = e.matmul(
                                    pz[t % 2][:, 0:w],
                                    hT_s[ri % 2][:, cch,
                                                 tt * 128:(tt + 1) * 128],
                                    Wl[:, cch, 0:w],
                                    start=(cch == 0), stop=last)
                                if sm:
                                    ins.then_inc(sm[0], 1)
                            return f
                        emit("tensor", mk_dense())
                    if L["fy"]:
                        wait("tensor", r_fyr.last(t))
                        sm = s_zpe.inc(1)
                        emit("tensor", lambda e, t=t, sm=sm: e.matmul(
                            pz[t % 2][:, 0:H],
                            fyr_s[t % 2][:],
                            W3b_s[:], start=False, stop=True).then_inc(sm[0], 1))
                    if range_last:
                        range_zpe.append(s_zpe.n)
                    dense_epilogue(t)
        checkpoint()   # dense of this layer done
        # ---------- agg ----------
        gbufs = gath2_s if L["e2"] else gath_s
        ape_base = s_ape.n
        aact_base = s_aact.n
        wait_all("gpsimd", r_gw)
        if li == 0:
            wait("tensor", (s_fy.h, NT))   # pa WAR vs interp
        for c in range(NT // 2):
            wait("sync", r_g.last(c))
            sm = r_gro.write(c)
            emit("sync", lambda e, c=c, sm=sm: e.dma_start(
                out=gro_s[c % 2][:],
                in_=Din["groww"][:, c * 128:(c + 1) * 128]).then_inc(sm[0], 16))
            wait("gpsimd", r_gro.last(c))
            sm = r_g.write(c)
            smp = s_gprep.inc(1)
            emit("gpsimd", lambda e, c=c, gd=gdst, gb=gbufs, w=width, sm=sm,
                 smp=smp: e.dma_gather(
                     out_ap=gb[c % 2][:, :, 0:w],
                     in_ap=gd[:],
                     idxs_ap=gro_s[c % 2][:],
                     num_idxs=2048, num_idxs_reg=2048, elem_size=w,
                     single_packet=False,
                     prepare_only=True, sem=r_g.sems[c % 2].h,
                     queue_num=c % 2,
                 ).then_inc(smp[0], 1))
            wait("gpsimd", s_gprep.now())
            wait("gpsimd", (s_ape.h,
                            ape_base if c < 2 else ape_base + 2 * (c - 1)))
            emit("gpsimd", lambda e, c=c: e.trigger_dma(
                count=1, queue_num=c % 2))
            wait("sync", (s_ape.h,
                          ape_base if c < 2 else ape_base + 2 * (c - 1)))
            sm = r_s.write(c)
            emit("sync", lambda e, c=c, sm=sm: e.dma_start(
                out=sT_s[c % 2][:],
                in_=Din["sTw"][:, c * 16:(c + 1) * 16, :]).then_inc(sm[0], 16))
            wait("tensor", r_g.last(c))
            wait("tensor", r_s.last(c))
            for tt in range(2):
                t = 2 * c + tt
                wait("tensor", (s_aact.h,
                                aact_base if t < 2 else aact_base + t - 1))
                for sl in range(SLICES):
                    emit("tensor", lambda e, c=c, tt=tt, t=t, sl=sl, gb=gbufs,
                         w=width: e.matmul(
                        pa[t % 2][:, 0:w],
                        sT_s[c % 2][:, tt * 8 + sl, :],
                        gb[c % 2][:, tt * 8 + sl, 0:w],
                        start=(sl == 0), stop=False))
                sm = s_ape.inc(1)
                emit("tensor", lambda e, t=t, li=li, w=width, sm=sm: e.matmul(
                    pa[t % 2][:, 0:w], ones1_s[:],
                    brows_s[:, li, 0:w], start=False, stop=True
                ).then_inc(sm[0], 1))
                wait("scalar", (s_ape.h, s_ape.n))
                if L["e2"]:
                    wait("scalar", r_ow.last(t))
                else:
                    wait("scalar", r_hw.last(t))
                sm = s_aact.inc(1)
                if L["e2"]:
                    emit("scalar", lambda e, t=t, sm=sm: e.activation(
                        osb_s[t % 2][:], pa[t % 2][:, 0:128], ACTF.Copy,
                        bias=0.0, scale=1.0).then_inc(sm[0], 1))
                else:
                    emit("scalar", lambda e, t=t, sm=sm: e.activation(
                        hsb_s[t % 4][:], pa[t % 2][:, 0:H], ACTF.Relu,
                        bias=0.0, scale=1.0).then_inc(sm[0], 1))
                wait("sync", (s_aact.h, s_aact.n))
                if L["e2"]:
                    sm = r_ow.write(t)
                    emit("sync", lambda e, t=t, sm=sm: e.dma_start(
                        out=Din["out_nm"][t * 128:(t + 1) * 128, :],
                        in_=osb_s[t % 2][:]).then_inc(sm[0], 16))
                else:
                    sm = r_hw.write(t)
                    emit("sync", lambda e, t=t, sm=sm: e.dma_start(
                        out=h_d[t * 128:(t + 1) * 128, :],
                        in_=hsb_s[t % 4][:]).then_inc(sm[0], 16))
        if not L["e2"]:
            wait_all("sync", r_hw)   # barrier before next layer's hT loads
        checkpoint()   # 3+li

    wait_all("sync", r_ow)
    wait_all("sync", r_hw)
    checkpoint()
    if PHASE < len(checkpoints):
        cut = checkpoints[PHASE]
        for e in Q:
            Q[e] = Q[e][:cut[e]]

    with nc.allow_non_contiguous_dma(reason="wrapped idx build"), \
            nc.Block() as block:
        @block.sync
        def _(e):
            for fn in Q["sync"]:
                fn(e)

        @block.tensor
        def _(e):
            for fn in Q["tensor"]:
                fn(e)

        @block.vector
        def _(e):
            for fn in Q["vector"]:
                fn(e)

        @block.scalar
        def _(e):
            for fn in Q["scalar"]:
                fn(e)

        @block.gpsimd
        def _(e):
            for fn in Q["gpsimd"]:
                fn(e)

    nc.finalize()
    return nc


# ================= host side =================

def host_prep(inputs):
    x = np.asarray(inputs["x"], np.float32)
    sdf = np.asarray(inputs["sdf"], np.float32)
    edge_index = np.asarray(inputs["edge_index"], np.int64)
    coarse_x = np.asarray(inputs["coarse_x"], np.float32)
    coarse_y = np.asarray(inputs["coarse_y"], np.float32)
    Ws = {k: np.asarray(inputs[k], np.float32) for k in (
        "pre_W0", "pre_W1", "pre_W2", "end_W0", "end_W1", "end_W2")}
    bs = {k: np.asarray(inputs[k], np.float32) for k in (
        "pre_b0", "pre_b1", "pre_b2", "end_b0", "end_b1", "end_b2")}

    cxT3 = np.zeros((3, NCPAD), np.float32)
    cxT3[0, :NC] = 2 * coarse_x[:, 0]
    cxT3[1, :NC] = 2 * coarse_x[:, 1]
    cxT3[2, :NC] = -(coarse_x[:, 0] ** 2 + coarse_x[:, 1] ** 2)
    cxT3[0, NC:] = 2e4; cxT3[1, NC:] = 2e4; cxT3[2, NC:] = -2e8

    brows = np.zeros((6, H), np.float32)
    for i, k in enumerate(("pre_b0", "pre_b1", "pre_b2", "end_b0", "end_b1")):
        brows[i] = bs[k]
    brows[5, :OUT] = bs["end_b2"]

    W5 = np.zeros((H, 128), np.float32)
    W5[:, :OUT] = Ws["end_W2"]

    def pmaj(w):   # [512, X] -> [128, 4, X]
        return np.ascontiguousarray(
            w.reshape(4, 128, w.shape[1]).transpose(1, 0, 2))

    common = dict(
        cxT3=cxT3,
        W0=Ws["pre_W0"].astype(bfnp),
        W1=pmaj(Ws["pre_W1"]).astype(bfnp),
        W2=pmaj(Ws["pre_W2"]).astype(bfnp),
        W3a=pmaj(Ws["end_W0"][OUT:]).astype(bfnp),
        W3b=Ws["end_W0"][:OUT].astype(bfnp),
        W4=pmaj(Ws["end_W1"]).astype(bfnp),
        W5=pmaj(W5).astype(bfnp),
        brows=brows.astype(bfnp)[None],
        ones1=np.ones((1, 128), bfnp),
        identb=np.eye(128, dtype=np.float32).astype(bfnp),
    )

    in_maps, metas = [], []
    for s in range(B):
        xs = x[s * NF:(s + 1) * NF]
        e = edge_index[:, s * E_PER:(s + 1) * E_PER] - s * NF
        cy = coarse_y[s * NC:(s + 1) * NC]

        deg = np.bincount(e[1], minlength=NF).astype(np.float32) + 1.0
        dinv = (1.0 / np.sqrt(deg)).astype(np.float32)

        # balanced tile assignment (snake over degree-sorted nodes)
        order = np.argsort(-deg, kind="stable")
        tile_seq = np.arange(NT)
        snake = np.concatenate([tile_seq, tile_seq[::-1]])
        bins = np.resize(snake, NF)
        nid = np.empty(NF, np.int64)
        for t in range(NT):
            sel = np.where(bins == t)[0]
            nid[order[sel]] = t * 128 + np.arange(len(sel))

        dinv_new = np.ones(NPAD, np.float32)
        dinv_new[nid] = dinv

        allrow = np.concatenate([nid[e[0]], np.arange(NPAD)])
        allcol = np.concatenate([nid[e[1]], np.arange(NPAD)])
        wts = dinv_new[allrow] * dinv_new[allcol]

        o = np.argsort(allcol, kind="stable")
        allrow, allcol, wts = allrow[o], allcol[o], wts[o]
        tile_of = allcol // 128
        tstart = np.searchsorted(tile_of, np.arange(NT))
        cnts = np.searchsorted(tile_of, np.arange(NT), side="right") - tstart
        assert cnts.max() <= TSLOTS, f"tile overflow {cnts.max()}"

        rank = np.arange(len(allcol)) - np.repeat(tstart, cnts)
        srow = np.zeros((NT, TSLOTS), np.int16)
        srow[tile_of, rank] = allrow.astype(np.int16)
        sT = np.zeros((NT, TSLOTS, 128), np.float32)
        sT[tile_of, rank, allcol % 128] = wts
        sT = sT.reshape(ECAP, 128)
        sTw = np.ascontiguousarray(
            sT.reshape(ECAP // 128, 128, 128).transpose(1, 0, 2)).astype(bfnp)

        grow = srow.reshape(ECAP)
        tmp = np.ascontiguousarray(grow.reshape(ECAP // 16, 16).T)
        groww = np.ascontiguousarray(np.tile(tmp, (8, 1)).astype(np.int16))

        f01 = np.full((NPAD, 2), 1e3, np.float32)
        f01[nid] = xs[:, 0:2]
        xT3 = np.ones((3, NPAD), np.float32)
        xT3[0] = f01[:, 0]; xT3[1] = f01[:, 1]
        negf2 = np.ascontiguousarray(
            (-(f01[:, 0] ** 2 + f01[:, 1] ** 2)).reshape(NT, 128).T)

        h0 = np.zeros((NPAD, 6), np.float32)
        h0[nid, 0:D_IN] = xs
        h0[nid, D_IN] = sdf[:, 0]
        h0T = np.ascontiguousarray(h0.T).astype(bfnp)

        ctab = np.zeros((NCPAD, 128), np.float32)
        ctab[:NC, 0:OUT] = cy
        ctab = ctab.astype(bfnp)

        m = dict(common)
        m.update(xT3=xT3, negf2=negf2, h0T=h0T, sTw=sTw, groww=groww, ctab=ctab)
        in_maps.append(m)
        metas.append(nid)

    full_maps = [in_maps[c // 2] for c in range(N_CORES)]
    return full_maps, metas


_prog_cache = {}


def kernel(**inputs):
    if "nc" not in _prog_cache:
        _prog_cache["nc"] = build_program()
    nc = _prog_cache["nc"]

    in_maps, metas = host_prep(inputs)
    res = run_bass_kernel_spmd(nc, in_maps, list(range(N_CORES)))
    global _last_exec_ns, _last_trace
    _last_exec_ns = res.exec_time_ns
    _last_trace = res.instructions_and_trace

    out = np.empty((B * NF, OUT), np.float32)
    for s in range(B):
        o = np.asarray(res.results[2 * s]["out_nm"])
        out[s * NF:(s + 1) * NF] = o[metas[s], 0:OUT]
    return out



# revision 13
# speedup vs baseline: 2.0516x; 1.8028x over previous
"""CFD-GCN Trainium2 kernel: 6-layer GCN on a batched random mesh graph +
KNN interpolation, distributed over 8 NeuronCores.

Each sample (4 total) is split across a PAIR of cores: core 2s owns node
tiles 0..78, core 2s+1 owns 79..157 (79 tiles of 128 nodes each). Dense
(h@W), KNN selection, interpolation and aggregation all run on the owned
half; a pair AllGather shares the dense output g each layer so gathers can
read any source node. Self-loops are folded into a per-tile diagonal
matmul instead of gather slots. Edge-gather descriptors are generated with
prepare_only + trigger_dma on 2 SWDGE queues.

Self-contained: hardcodes all shapes; the slice profile (slots per dest
tile) is derived from the inputs on first call and baked into the program.
kernel(**inputs) -> np.ndarray [80000, 3].
"""
import sys

sys.path.insert(0, "/opt/trn_rl_repo")

import numpy as np
import ml_dtypes

from concourse import bass, bacc
from concourse.bass_utils import run_bass_kernel_spmd
import concourse.mybir as mybir
from contextlib import ExitStack

f32, bf16 = mybir.dt.float32, mybir.dt.bfloat16
i16, u16 = mybir.dt.int16, mybir.dt.uint16
ALU = mybir.AluOpType
ACTF = mybir.ActivationFunctionType
bfnp = ml_dtypes.bfloat16

# ---------------- problem constants ----------------
B, NF, NC, H, D_IN, OUT = 4, 20000, 2000, 512, 5, 3
E_PER = 6 * NF
NT = 158                      # global node tiles per sample
NTH = 79                      # node tiles per core (half sample)
NPAD = NT * 128               # 20224
NPADH = NTH * 128             # 10112
NCPAD = 2048                  # padded coarse count
MAXSL = 8                     # max 128-slot edge slices per dest tile
RANGE_T = 16                  # node tiles per hT transpose-load range
ICH = 8                       # interp gather chunk (tiles)
N_CORES = 8
GB = 3                        # agg ring depth (gather bufs / psum banks)
PHASE = 99                    # debug: truncate program after checkpoint N

LAYERS = [
    dict(kc6=True, fy=False, relu=True, e2=False),   # pre0
    dict(kc6=False, fy=False, relu=True, e2=False),  # pre1
    dict(kc6=False, fy=False, relu=True, e2=False),  # pre2
    dict(kc6=False, fy=True, relu=True, e2=False),   # end0
    dict(kc6=False, fy=False, relu=True, e2=False),  # end1
    dict(kc6=False, fy=False, relu=False, e2=True),  # end2
]

RG_PAIRS = [[0, 1], [2, 3], [4, 5], [6, 7]]


def _ranges():
    r, t0 = [], 0
    while t0 < NTH:
        r.append((t0, min(RANGE_T, NTH - t0)))
        t0 += RANGE_T
    return r


def build_program(P):
    """P: per-local-tile slice counts (len NTH), identical on all cores."""
    SOFF = np.concatenate([[0], np.cumsum(P)]).astype(int)   # slice offsets
    SLOT_TOT = int(SOFF[-1]) * 128

    nc = bacc.Bacc(num_devices=N_CORES, num_swdge_queues=2)

    Din = {}
    def din(name, shape, dt):
        Din[name] = nc.declare_dram_parameter(name, list(shape), dt, isOutput=False)
    def dout(name, shape, dt):
        Din[name] = nc.declare_dram_parameter(name, list(shape), dt, isOutput=True)

    din("xT3", (3, NPADH), f32)
    din("cxT3", (3, NCPAD), f32)
    din("negf2", (128, NTH), f32)
    din("h0T", (6, NPADH), bf16)
    din("W0", (6, H), bf16)
    din("W1", (128, 4, H), bf16)      # p-major k-chunked
    din("W2", (128, 4, H), bf16)
    din("W3a", (128, 4, H), bf16)
    din("W3b", (3, H), bf16)
    din("W4", (128, 4, H), bf16)
    din("W5", (128, 4, 128), bf16)
    din("brows", (1, 6, H), bf16)
    din("ones1", (1, 128), bf16)
    din("identb", (128, 128), bf16)
    din("identf", (128, 128), f32)
    din("dv2", (128, NTH), f32)
    din("sTw", (128, SLOT_TOT // 128, 128), bf16)
    din("groww", (128, SLOT_TOT // 16), i16)
    din("ctab", (NCPAD, 128), bf16)

    g_half = nc.dram_tensor("g_half", [NPADH, H], bf16)
    g_full = nc.dram_tensor("g_full", [NPAD, H], bf16)
    g2_half = nc.dram_tensor("g2_half", [NPADH, 128], bf16)
    g2_full = nc.dram_tensor("g2_full", [NPAD, 128], bf16)
    h_d = nc.dram_tensor("h_d", [NPADH, H], bf16)
    fy_d = nc.dram_tensor("fy_d", [3, NPADH], bf16)
    dout("out_nm", (NPADH, 128), f32)

    es = ExitStack()
    def sb(name, shape, dt):
        return es.enter_context(nc.sbuf_tensor(name, list(shape), dt))
    def psum(name, shape, dt):
        return es.enter_context(nc.psum_tensor(name, list(shape), dt))

    xt_s = [sb(f"xt_s{i}", (3, 128), f32) for i in range(2)]
    cxT3_s = sb("cxT3_s", (3, NCPAD), f32)
    negf2_s = sb("negf2_s", (128, NTH), f32)
    h0_s = [sb(f"h0_s{i}", (6, 128), bf16) for i in range(2)]
    W0_s = sb("W0_s", (6, H), bf16)
    W1_s = sb("W1_s", (128, 4, H), bf16)
    W2_s = sb("W2_s", (128, 4, H), bf16)
    W3a_s = sb("W3a_s", (128, 4, H), bf16)
    W3b_s = sb("W3b_s", (3, H), bf16)
    W4_s = sb("W4_s", (128, 4, H), bf16)
    W5_s = sb("W5_s", (128, 4, 128), bf16)
    brows_s = sb("brows_s", (1, 6, H), bf16)
    ones1_s = sb("ones1_s", (1, 128), bf16)
    identb_s = sb("identb_s", (128, 128), bf16)
    identf_s = sb("identf_s", (128, 128), f32)
    dv2_s = sb("dv2_s", (128, NTH), f32)
    gro_s = [sb(f"gro_s{i}", (128, MAXSL * 8), i16) for i in range(GB)]

    hT_s = [sb(f"hT_s{i}", (128, 4, RANGE_T * 128), bf16) for i in range(2)]
    gsb_s = [sb(f"gsb_s{i}", (128, H), bf16) for i in range(4)]
    hsb_s = [sb(f"hsb_s{i}", (128, H), bf16) for i in range(4)]
    osb_s = [sb(f"osb_s{i}", (128, 128), f32) for i in range(2)]
    gath_s = [sb(f"gath_s{i}", (128, MAXSL, H), bf16) for i in range(GB)]
    gath2_s = [sb(f"gath2_s{i}", (128, MAXSL, 128), bf16) for i in range(GB)]
    sT_s = [sb(f"sT_s{i}", (128, MAXSL, 128), bf16) for i in range(GB)]
    gt_s = [sb(f"gt_s{i}", (128, H), bf16) for i in range(GB)]
    dgw_s = [sb(f"dgw_s{i}", (128, 128), bf16) for i in range(GB)]

    nd2_s = [sb(f"nd2_s{i}", (128, NCPAD), f32) for i in range(2)]
    bm_s = sb("bm_s", (128, 8, NTH), f32)
    bi_s = sb("bi_s", (128, 8, NTH), u16)
    d2c_s = sb("d2c_s", (128, 3, NTH), f32)
    w_s = sb("w_s", (128, 3, NTH), f32)
    wsum_s = sb("wsum_s", (128, NTH), f32)
    rs_s = sb("rs_s", (128, NTH), f32)
    wnb_s = sb("wnb_s", (128, 3, NTH), f32)
    wrap_s = sb("wrap_s", (128, 3, NTH, 8), u16)
    gk_s = [[sb(f"gk_s{k}_{i}", (128, ICH, 128), bf16) for i in range(2)]
            for k in range(3)]
    diag3_s = [sb(f"diag3_s{i}", (128, 3, 128), bf16) for i in range(2)]
    fyw_s = [sb(f"fyw_s{i}", (3, 128), bf16) for i in range(2)]
    fyr_s = [sb(f"fyr_s{i}", (3, 128), bf16) for i in range(2)]

    pz = [psum(f"pz{i}", (128, H), f32) for i in range(3)]
    pa = [psum(f"pa{i}", (128, H), f32) for i in range(3)]

    class Sem:
        def __init__(self, name):
            self.h = es.enter_context(nc.semaphore(name))
            self.n = 0
        def inc(self, k):
            self.n += k
            return (self.h, self.n)
        def now(self):
            return (self.h, self.n)

    class Ring:
        def __init__(self, name, n):
            self.sems = [Sem(f"{name}{i}") for i in range(n)]
            self.nslots = n
        def write(self, slot, k=16):
            s = self.sems[slot % self.nslots]
            return s.inc(k)
        def last(self, slot):
            s = self.sems[slot % self.nslots]
            return (s.h, s.n)
        def all(self):
            return [(s.h, s.n) for s in self.sems]

    def wait_all(engine, ring):
        for sv in ring.all():
            wait(engine, sv)

    s_in = Sem("s_in")
    s_gprep = Sem("s_gprep")
    s_iprep = Sem("s_iprep")
    s_cc = Sem("s_cc")
    s_kpe = Sem("s_kpe"); s_kact = Sem("s_kact"); s_kmax = Sem("s_kmax")
    s_wn = Sem("s_wn"); s_wrap = Sem("s_wrap")
    s_dg = Sem("s_dg"); s_dg2 = Sem("s_dg2")
    s_ipe = Sem("s_ipe"); s_fy = Sem("s_fy")
    s_zpe = Sem("s_zpe")
    s_zact = Sem("s_zact"); s_ape = Sem("s_ape"); s_aact = Sem("s_aact")

    Q = {e: [] for e in ("sync", "tensor", "vector", "scalar", "gpsimd")}
    checkpoints = []
    def checkpoint():
        checkpoints.append({e: len(Q[e]) for e in Q})
    def emit(engine, fn):
        Q[engine].append(fn)
    def wait(engine, semv):
        s, v = semv
        if v > 0:
            emit(engine, lambda e, s=s, v=v: e.wait_ge(s, v))

    r_gk = Ring("r_gk", 2)     # interp table gathers (per gk buf)
    r_xt = Ring("r_xt", 2)     # xT3 tile loads
    r_h0 = Ring("r_h0", 2)     # h0T tile loads
    r_gro = Ring("r_gro", GB)  # gather idx loads
    r_fyw = Ring("r_fyw", 2)   # finey dram writes
    r_fyr = Ring("r_fyr", 2)   # finey tile loads
    r_hT = Ring("r_hT", 2)     # transpose loads (per hT buf)
    r_g = Ring("r_g", GB)      # agg gathers (per gath buf)
    r_s = Ring("r_s", GB)      # S loads (per sT buf)
    r_gt = Ring("r_gt", GB)    # g self-tile loads
    r_gw = Ring("r_gw", 4)     # g_half dram writes (per gsb buf)
    r_hw = Ring("r_hw", 4)     # h dram writes (per hsb buf)
    r_ow = Ring("r_ow", 2)     # out writes (per osb buf)

    # ============ input loads ============
    loads = [
        (cxT3_s[:], "cxT3"), (negf2_s[:], "negf2"),
        (W0_s[:], "W0"), (W1_s[:], "W1"), (W2_s[:], "W2"),
        (W3a_s[:], "W3a"), (W3b_s[:], "W3b"), (W4_s[:], "W4"), (W5_s[:], "W5"),
        (brows_s[:], "brows"), (ones1_s[:], "ones1"), (identb_s[:], "identb"),
        (identf_s[:], "identf"), (dv2_s[:], "dv2"),
    ]
    for dst, srcn in loads:
        sm = s_in.inc(16)
        emit("sync", lambda e, d=dst, s=srcn, sm=sm: e.dma_start(
            out=d, in_=Din[s][:]).then_inc(sm[0], 16))
    IN_ALL = s_in.now()
    checkpoint()   # 0: loads

    # ============ KNN selection ============
    wait("tensor", IN_ALL)
    wait("scalar", IN_ALL)
    wait("vector", IN_ALL)
    NQ = NCPAD // 512
    for t in range(NTH):
        if t >= 2:
            wait("sync", (s_kpe.h, NQ * (t - 1)))
        sm = r_xt.write(t)
        emit("sync", lambda e, t=t, sm=sm: e.dma_start(
            out=xt_s[t % 2][:], in_=Din["xT3"][:, t * 128:(t + 1) * 128]
        ).then_inc(sm[0], 16))
        wait("tensor", r_xt.last(t))
        for q in range(NQ):
            gq = NQ * t + q
            if gq >= 3:
                wait("tensor", (s_kact.h, gq - 2))
            sm = s_kpe.inc(1)
            emit("tensor", lambda e, t=t, q=q, gq=gq, sm=sm: e.matmul(
                pz[gq % 3][:, 0:512], xt_s[t % 2][:],
                cxT3_s[:, q * 512:(q + 1) * 512],
                start=True, stop=True).then_inc(sm[0], 1))
        for q in range(NQ):
            gq = NQ * t + q
            wait("scalar", (s_kpe.h, gq + 1))
            if t >= 2 and q == 0:
                wait("scalar", (s_kmax.h, t - 1))
            sm = s_kact.inc(1)
            emit("scalar", lambda e, t=t, q=q, gq=gq, sm=sm: e.activation(
                nd2_s[t % 2][:, q * 512:(q + 1) * 512], pz[gq % 3][:, 0:512],
                ACTF.Identity, bias=negf2_s[:, t:t + 1], scale=1.0
            ).then_inc(sm[0], 1))
        wait("vector", (s_kact.h, NQ * (t + 1)))
        emit("vector", lambda e, t=t: e.max(bm_s[:, :, t], nd2_s[t % 2][:]))
        emit("vector", lambda e: e.drain())
        emit("vector", lambda e, t=t: e.max_index(
            bi_s[:, :, t], bm_s[:, :, t], nd2_s[t % 2][:]))
        sm = s_kmax.inc(1)
        emit("vector", lambda e, sm=sm: e.drain().then_inc(sm[0], 1))

    checkpoint()   # 1: knn select
    # weights on DVE
    emit("vector", lambda e: e.tensor_scalar(
        out=d2c_s[:], in0=bm_s[:, 0:3, :], scalar1=-1.0, scalar2=1e-16,
        op0=ALU.mult, op1=ALU.max))
    emit("vector", lambda e: e.drain())
    emit("vector", lambda e: e.reciprocal(w_s[:], d2c_s[:]))
    emit("vector", lambda e: e.drain())
    emit("vector", lambda e: e.tensor_reduce(
        out=wsum_s[:], in_=bass.AP(w_s, 0, [[3 * NTH, 128], [1, NTH], [NTH, 3]]),
        axis=mybir.AxisListType.X, op=ALU.add))
    emit("vector", lambda e: e.drain())
    emit("vector", lambda e: e.reciprocal(rs_s[:], wsum_s[:]))
    emit("vector", lambda e: e.drain())
    emit("vector", lambda e: e.tensor_tensor(
        out=wnb_s[:], in0=w_s[:],
        in1=bass.AP(rs_s, 0, [[NTH, 128], [0, 3], [1, NTH]]),
        op=ALU.mult))
    sm = s_wn.inc(1)
    emit("vector", lambda e, sm=sm: e.drain().then_inc(sm[0], 1))

    # wrapped idx build (gpsimd)
    wait("gpsimd", (s_kmax.h, NTH))
    for k in range(3):
        for g in range(8):
            sm = s_wrap.inc(16)
            emit("gpsimd", lambda e, k=k, g=g, sm=sm: e.dma_start(
                out=wrap_s[0:16, k, :, g],
                in_=bi_s[16 * g:16 * (g + 1), k, :],
            ).then_inc(sm[0], 16))
    wait("gpsimd", s_wrap.now())
    for rep in range(1, 8):
        sm = s_wrap.inc(16)
        emit("gpsimd", lambda e, rep=rep, sm=sm: e.dma_start(
            out=wrap_s[16 * rep:16 * (rep + 1)],
            in_=wrap_s[0:16],
        ).then_inc(sm[0], 16))
    WRAP_ALL = s_wrap.now()

    # interp
    wait("gpsimd", WRAP_ALL)
    wait("vector", s_wn.now())
    n_ich = (NTH + ICH - 1) // ICH
    for c in range(n_ich):
        t0 = c * ICH
        ntile = min(ICH, NTH - t0)
        for k in range(3):
            sm = r_gk.write(c)
            smp = s_iprep.inc(1)
            emit("gpsimd", lambda e, k=k, c=c, t0=t0, nt=ntile, sm=sm, smp=smp:
                 e.dma_gather(
                     out_ap=gk_s[k][c % 2][:, 0:nt, :],
                     in_ap=Din["ctab"][:],
                     idxs_ap=wrap_s[:, k, t0:t0 + nt, :].bitcast(i16),
                     num_idxs=nt * 128, num_idxs_reg=nt * 128,
                     elem_size=128,
                     prepare_only=True, sem=r_gk.sems[c % 2].h,
                     queue_num=c % 2,
                 ).then_inc(smp[0], 1))
        wait("gpsimd", s_iprep.now())
        if c >= 2:
            wait("gpsimd", (s_ipe.h, (c - 1) * ICH))
        emit("gpsimd", lambda e, c=c: e.trigger_dma(
            count=3, queue_num=c % 2))
        GK_NOW = r_gk.last(c)
        for tt in range(ntile):
            t = t0 + tt
            if t >= 2:
                wait("vector", (s_ipe.h, t - 1))
            emit("vector", lambda e, t=t: e.tensor_tensor(
                out=diag3_s[t % 2][:],
                in0=bass.AP(identf_s, 0, [[128, 128], [0, 3], [1, 128]]),
                in1=bass.AP(wnb_s, t, [[3 * NTH, 128], [NTH, 3], [0, 128]]),
                op=ALU.mult))
            sm = s_dg.inc(1)
            emit("vector", lambda e, sm=sm: e.drain().then_inc(sm[0], 1))
            wait("tensor", GK_NOW)
            wait("tensor", (s_dg.h, s_dg.n))
            if t >= 3:
                wait("tensor", (s_fy.h, t - 2))    # psum WAR
            for k in range(3):
                sm = s_ipe.inc(1) if k == 2 else None
                def mk_interp(t=t, tt=tt, k=k, c=c, sm=sm):
                    def f(e):
                        ins = e.matmul(
                            pa[t % 3][:, 0:128], gk_s[k][c % 2][:, tt, :],
                            diag3_s[t % 2][:, k, :],
                            start=(k == 0), stop=(k == 2))
                        if sm:
                            ins.then_inc(sm[0], 1)
                    return f
                emit("tensor", mk_interp())
            wait("scalar", (s_ipe.h, s_ipe.n))
            wait("scalar", r_fyw.last(t))
            sm = s_fy.inc(1)
            emit("scalar", lambda e, t=t, sm=sm: e.activation(
                fyw_s[t % 2][:], pa[t % 3][0:3, 0:128],
                ACTF.Copy, bias=0.0, scale=1.0).then_inc(sm[0], 1))
            wait("sync", (s_fy.h, s_fy.n))
            sm = r_fyw.write(t)
            emit("sync", lambda e, t=t, sm=sm: e.dma_start(
                out=fy_d[:, t * 128:(t + 1) * 128],
                in_=fyw_s[t % 2][:]).then_inc(sm[0], 16))
    FY_ALL = s_fy.now()
    KACT_ALL = s_kact.now()
    checkpoint()   # 2: interp

    # ============ GCN layers ============
    WCH = {1: W1_s, 2: W2_s, 3: W3a_s, 4: W4_s, 5: W5_s}

    for li, L in enumerate(LAYERS):
        width = 128 if L["e2"] else H
        ghalf = g2_half if L["e2"] else g_half
        gfull = g2_full if L["e2"] else g_full

        # ---------- dense (own half) ----------
        zpe_base = s_zpe.n
        zact_base = s_zact.n
        cc_prev = s_cc.n          # AG of previous layer

        def dense_epilogue(t, width=width, ghalf=ghalf, cc_prev=cc_prev):
            wait("scalar", (s_zpe.h, zpe_base + t + 1))
            wait("scalar", r_gw.last(t))
            sm = s_zact.inc(1)
            emit("scalar", lambda e, t=t, w=width, sm=sm: e.activation(
                gsb_s[t % 4][:, 0:w], pz[t % 3][:, 0:w], ACTF.Copy,
                bias=0.0, scale=1.0).then_inc(sm[0], 1))
            wait("sync", (s_zact.h, s_zact.n))
            if t == 0:
                wait("sync", (s_cc.h, cc_prev))   # WAR vs prev AG read
            sm = r_gw.write(t)
            emit("sync", lambda e, t=t, gd=ghalf, w=width, sm=sm: e.dma_start(
                out=gd[t * 128:(t + 1) * 128, :],
                in_=gsb_s[t % 4][:, 0:w]).then_inc(sm[0], 16))

        if li == 0:
            wait("tensor", KACT_ALL)      # pz WAR vs KNN ACT
            for t in range(NTH):
                if t >= 2:
                    wait("sync", (s_zpe.h, zpe_base + t - 1))
                sm = r_h0.write(t)
                emit("sync", lambda e, t=t, sm=sm: e.dma_start(
                    out=h0_s[t % 2][:], in_=Din["h0T"][:, t * 128:(t + 1) * 128]
                ).then_inc(sm[0], 16))
                wait("tensor", r_h0.last(t))
                wait("tensor", (s_zact.h,
                                zact_base if t < 3 else zact_base + t - 2))
                sm = s_zpe.inc(1)
                emit("tensor", lambda e, t=t, sm=sm: e.matmul(
                    pz[t % 3][:, 0:H], h0_s[t % 2][:],
                    W0_s[:], start=True, stop=True).then_inc(sm[0], 1))
                dense_epilogue(t)
        else:
            Wl = WCH[li]
            range_zpe = []
            for ri, (rt0, rnt) in enumerate(_ranges()):
                wait("sync", (s_zpe.h,
                              zpe_base if ri < 2 else range_zpe[ri - 2]))
                for cch in range(4):
                    sm = r_hT.write(ri)
                    emit("sync", lambda e, ri=ri, rt0=rt0, rnt=rnt, c=cch, sm=sm:
                         e.dma_start_transpose(
                             hT_s[ri % 2][:, c, 0:rnt * 128],
                             h_d[rt0 * 128:(rt0 + rnt) * 128,
                                 c * 128:(c + 1) * 128],
                         ).then_inc(sm[0], 16))
                wait("tensor", r_hT.last(ri))
                if li == 3 and ri == 0:
                    wait_all("sync", r_fyw)
                for tt in range(rnt):
                    t = rt0 + tt
                    range_last = (tt == rnt - 1)
                    if L["fy"]:
                        if t >= 2:
                            wait("sync", (s_zpe.h, zpe_base + t - 1))
                        sm = r_fyr.write(t)
                        emit("sync", lambda e, t=t, sm=sm: e.dma_start(
                            out=fyr_s[t % 2][:],
                            in_=fy_d[:, t * 128:(t + 1) * 128]
                        ).then_inc(sm[0], 16))
                    wait("tensor", (s_zact.h,
                                    zact_base if t < 3 else zact_base + t - 2))
                    for cch in range(4):
                        last = (cch == 3) and not L["fy"]
                        sm = s_zpe.inc(1) if last else None
                        def mk_dense(t=t, tt=tt, ri=ri, cch=cch, Wl=Wl,
                                     w=width, last=last, sm=sm):
                            def f(e):
                                ins = e.matmul(
                                    pz[t % 3][:, 0:w],
                                    hT_s[ri % 2][:, cch,
                                                 tt * 128:(tt + 1) * 128],
                                    Wl[:, cch, 0:w],
                                    start=(cch == 0), stop=last)
                                if sm:
                                    ins.then_inc(sm[0], 1)
                            return f
                        emit("tensor", mk_dense())
                    if L["fy"]:
                        wait("tensor", r_fyr.last(t))
                        sm = s_zpe.inc(1)
                        emit("tensor", lambda e, t=t, sm=sm: e.matmul(
                            pz[t % 3][:, 0:H],
                            fyr_s[t % 2][:],
                            W3b_s[:], start=False, stop=True).then_inc(sm[0], 1))
                    if range_last:
                        range_zpe.append(s_zpe.n)
                    dense_epilogue(t)
        checkpoint()   # dense of this layer done

        # ---------- AllGather g_half -> g_full ----------
        wait_all("gpsimd", r_gw)          # all dense writes landed
        for sv in r_g.all():              # WAR: prev-layer gathers read g_full
            wait("gpsimd", sv)
        sm = s_cc.inc(1)
        emit("gpsimd", lambda e, hh=ghalf, ff=gfull, sm=sm: e.collective_compute(
            "AllGather",
            ALU.bypass,
            replica_groups=RG_PAIRS,
            ins=[hh.ap().opt()],
            outs=[ff.ap().opt()],
        ).then_inc(sm[0], 1))
        CC_NOW = s_cc.now()

        # ---------- agg (own dest tiles) ----------
        gbufs = gath2_s if L["e2"] else gath_s
        ape_base = s_ape.n
        aact_base = s_aact.n
        if li == 0:
            wait("tensor", (s_fy.h, NTH))   # pa WAR vs interp
            wait("vector", (s_fy.h, NTH))   # diag3/dgw WAR vs interp reads? (pa only)
        for t in range(NTH):
            nsl = int(P[t])
            # --- idx load (sync) ---
            wait("sync", r_g.last(t))          # gro/sT buf WAR (prep+mm of t-GB done)
            sm = r_gro.write(t)
            emit("sync", lambda e, t=t, nsl=nsl, sm=sm: e.dma_start(
                out=gro_s[t % GB][:, 0:nsl * 8],
                in_=Din["groww"][:, SOFF[t] * 8:(SOFF[t] + nsl) * 8]
            ).then_inc(sm[0], 16))
            # --- S load (sync) ---
            wait("sync", (s_ape.h,
                          ape_base if t < GB else ape_base + t - GB + 1))
            sm = r_s.write(t)
            emit("sync", lambda e, t=t, nsl=nsl, sm=sm: e.dma_start(
                out=sT_s[t % GB][:, 0:nsl, :],
                in_=Din["sTw"][:, SOFF[t]:SOFF[t] + nsl, :]).then_inc(sm[0], 16))
            # --- g self tile load (sync) ---
            wait("sync", r_gw.last(t))         # dense write t landed
            sm = r_gt.write(t)
            emit("sync", lambda e, t=t, gd=ghalf, w=width, sm=sm: e.dma_start(
                out=gt_s[t % GB][:, 0:w],
                in_=gd[t * 128:(t + 1) * 128, :]).then_inc(sm[0], 16))
            # --- gather prep + trigger (gpsimd) ---
            wait("gpsimd", r_gro.last(t))
            sm = r_g.write(t)
            smp = s_gprep.inc(1)
            emit("gpsimd", lambda e, t=t, nsl=nsl, gf=gfull, gb=gbufs, w=width,
                 sm=sm, smp=smp: e.dma_gather(
                     out_ap=gb[t % GB][:, 0:nsl, 0:w],
                     in_ap=gf[:],
                     idxs_ap=gro_s[t % GB][:, 0:nsl * 8],
                     num_idxs=nsl * 128, num_idxs_reg=nsl * 128, elem_size=w,
                     single_packet=False,
                     prepare_only=True, sem=r_g.sems[t % GB].h,
                     queue_num=t % 2,
                 ).then_inc(smp[0], 1))
            wait("gpsimd", s_gprep.now())
            wait("gpsimd", (s_ape.h,
                            ape_base if t < GB else ape_base + t - GB + 1))
            wait("gpsimd", CC_NOW)             # g_full ready
            emit("gpsimd", lambda e, t=t: e.trigger_dma(
                count=1, queue_num=t % 2))
            # --- diag weight build (vector) ---
            wait("vector", (s_ape.h,
                            ape_base if t < GB else ape_base + t - GB + 1))
            emit("vector", lambda e, t=t: e.tensor_scalar(
                out=dgw_s[t % GB][:], in0=identb_s[:],
                scalar1=dv2_s[:, t:t + 1], scalar2=None, op0=ALU.mult))
            sm = s_dg2.inc(1)
            emit("vector", lambda e, sm=sm: e.drain().then_inc(sm[0], 1))
            # --- matmuls (tensor) ---
            wait("tensor", r_g.last(t))
            wait("tensor", r_s.last(t))
            wait("tensor", r_gt.last(t))
            wait("tensor", (s_dg2.h, s_dg2.n))
            wait("tensor", (s_aact.h,
                            aact_base if t < 3 else aact_base + t - 2))
            for sl in range(nsl):
                emit("tensor", lambda e, t=t, sl=sl, gb=gbufs, w=width:
                     e.matmul(
                         pa[t % 3][:, 0:w],
                         sT_s[t % GB][:, sl, :],
                         gb[t % GB][:, sl, 0:w],
                         start=(sl == 0), stop=False))
            emit("tensor", lambda e, t=t, w=width: e.matmul(
                pa[t % 3][:, 0:w], dgw_s[t % GB][:],
                gt_s[t % GB][:, 0:w], start=(nsl == 0), stop=False))
            sm = s_ape.inc(1)
            emit("tensor", lambda e, t=t, li=li, w=width, sm=sm: e.matmul(
                pa[t % 3][:, 0:w], ones1_s[:],
                brows_s[:, li, 0:w], start=False, stop=True
            ).then_inc(sm[0], 1))
            # --- epilogue (scalar + sync) ---
            wait("scalar", (s_ape.h, s_ape.n))
            if L["e2"]:
                wait("scalar", r_ow.last(t))
            else:
                wait("scalar", r_hw.last(t))
            sm = s_aact.inc(1)
            if L["e2"]:
                emit("scalar", lambda e, t=t, sm=sm: e.activation(
                    osb_s[t % 2][:], pa[t % 3][:, 0:128], ACTF.Copy,
                    bias=0.0, scale=1.0).then_inc(sm[0], 1))
            else:
                emit("scalar", lambda e, t=t, sm=sm: e.activation(
                    hsb_s[t % 4][:], pa[t % 3][:, 0:H], ACTF.Relu,
                    bias=0.0, scale=1.0).then_inc(sm[0], 1))
            wait("sync", (s_aact.h, s_aact.n))
            if L["e2"]:
                sm = r_ow.write(t)
                emit("sync", lambda e, t=t, sm=sm: e.dma_start(
                    out=Din["out_nm"][t * 128:(t + 1) * 128, :],
                    in_=osb_s[t % 2][:]).then_inc(sm[0], 16))
            else:
                sm = r_hw.write(t)
                emit("sync", lambda e, t=t, sm=sm: e.dma_start(
                    out=h_d[t * 128:(t + 1) * 128, :],
                    in_=hsb_s[t % 4][:]).then_inc(sm[0], 16))
        if not L["e2"]:
            wait_all("sync", r_hw)   # barrier before next layer's hT loads
        checkpoint()   # 3+li

    wait_all("sync", r_ow)
    wait_all("sync", r_hw)
    checkpoint()
    if PHASE < len(checkpoints):
        cut = checkpoints[PHASE]
        for e in Q:
            Q[e] = Q[e][:cut[e]]

    with nc.allow_non_contiguous_dma(reason="wrapped idx build"), \
            nc.Block() as block:
        @block.sync
        def _(e):
            for fn in Q["sync"]:
                fn(e)

        @block.tensor
        def _(e):
            for fn in Q["tensor"]:
                fn(e)

        @block.vector
        def _(e):
            for fn in Q["vector"]:
                fn(e)

        @block.scalar
        def _(e):
            for fn in Q["scalar"]:
                fn(e)

        @block.gpsimd
        def _(e):
            for fn in Q["gpsimd"]:
                fn(e)

    nc.finalize()
    return nc


# ================= host side =================

def host_prep(inputs):
    x = np.asarray(inputs["x"], np.float32)
    sdf = np.asarray(inputs["sdf"], np.float32)
    edge_index = np.asarray(inputs["edge_index"], np.int64)
    coarse_x = np.asarray(inputs["coarse_x"], np.float32)
    coarse_y = np.asarray(inputs["coarse_y"], np.float32)
    Ws = {k: np.asarray(inputs[k], np.float32) for k in (
        "pre_W0", "pre_W1", "pre_W2", "end_W0", "end_W1", "end_W2")}
    bs = {k: np.asarray(inputs[k], np.float32) for k in (
        "pre_b0", "pre_b1", "pre_b2", "end_b0", "end_b1", "end_b2")}

    cxT3 = np.zeros((3, NCPAD), np.float32)
    cxT3[0, :NC] = 2 * coarse_x[:, 0]
    cxT3[1, :NC] = 2 * coarse_x[:, 1]
    cxT3[2, :NC] = -(coarse_x[:, 0] ** 2 + coarse_x[:, 1] ** 2)
    cxT3[0, NC:] = 2e4; cxT3[1, NC:] = 2e4; cxT3[2, NC:] = -2e8

    brows = np.zeros((6, H), np.float32)
    for i, k in enumerate(("pre_b0", "pre_b1", "pre_b2", "end_b0", "end_b1")):
        brows[i] = bs[k]
    brows[5, :OUT] = bs["end_b2"]

    W5 = np.zeros((H, 128), np.float32)
    W5[:, :OUT] = Ws["end_W2"]

    def pmaj(w):   # [512, X] -> [128, 4, X]
        return np.ascontiguousarray(
            w.reshape(4, 128, w.shape[1]).transpose(1, 0, 2))

    common = dict(
        cxT3=cxT3,
        W0=Ws["pre_W0"].astype(bfnp),
        W1=pmaj(Ws["pre_W1"]).astype(bfnp),
        W2=pmaj(Ws["pre_W2"]).astype(bfnp),
        W3a=pmaj(Ws["end_W0"][OUT:]).astype(bfnp),
        W3b=Ws["end_W0"][:OUT].astype(bfnp),
        W4=pmaj(Ws["end_W1"]).astype(bfnp),
        W5=pmaj(W5).astype(bfnp),
        brows=brows.astype(bfnp)[None],
        ones1=np.ones((1, 128), bfnp),
        identb=np.eye(128, dtype=np.float32).astype(bfnp),
        identf=np.eye(128, dtype=np.float32),
    )

    # ---- pass 1: per-sample node->slot assignment + per-core tile stats ----
    samples = []
    for s in range(B):
        xs = x[s * NF:(s + 1) * NF]
        e = edge_index[:, s * E_PER:(s + 1) * E_PER] - s * NF
        cy = coarse_y[s * NC:(s + 1) * NC]

        deg = np.bincount(e[1], minlength=NF).astype(np.float32) + 1.0
        dinv = (1.0 / np.sqrt(deg)).astype(np.float32)

        # balanced global tile assignment (snake over degree-sorted nodes)
        order = np.argsort(-deg, kind="stable")
        tile_seq = np.arange(NT)
        snake = np.concatenate([tile_seq, tile_seq[::-1]])
        bins = np.resize(snake, NF)
        gtile = np.empty(NF, np.int64)   # node -> global tile
        lane = np.empty(NF, np.int64)
        for t in range(NT):
            sel = np.where(bins == t)[0]
            gtile[order[sel]] = t
            lane[order[sel]] = np.arange(len(sel))

        # in-edge count per global tile (excl self loops)
        cin = np.bincount(gtile[e[1]], minlength=NT)

        # per half: order local tiles by in-edge count desc
        half = (gtile >= NTH).astype(np.int64)
        ltile = np.empty(NF, np.int64)
        tile_perm = {}   # (half, local pos) -> global tile
        sc = np.zeros((2, NTH), np.int64)
        for p in range(2):
            gts = np.arange(p * NTH, (p + 1) * NTH)
            perm = gts[np.argsort(-cin[gts], kind="stable")]
            tile_perm[p] = perm
            inv = np.empty(NTH, np.int64)
            inv[perm - p * NTH] = np.arange(NTH)
            mask = half == p
            ltile[mask] = inv[gtile[mask] - p * NTH]
            sc[p] = (np.ceil(cin[perm] / 128)).astype(np.int64)

        # node -> row in g_full (AllGather layout)
        nidg = half * NPADH + ltile * 128 + lane
        samples.append(dict(xs=xs, e=e, cy=cy, dinv=dinv, half=half,
                            ltile=ltile, lane=lane, nidg=nidg, sc=sc))

    # slice profile: elementwise max over all 8 cores (descending by constr.)
    P = np.zeros(NTH, np.int64)
    for sm in samples:
        P = np.maximum(P, sm["sc"].max(axis=0))
    P = np.maximum(P, 1)
    assert P.max() <= MAXSL, f"slice overflow {P.max()}"
    SOFF = np.concatenate([[0], np.cumsum(P)]).astype(int)
    SLOT_TOT = int(SOFF[-1]) * 128

    # ---- pass 2: per-core arrays ----
    in_maps, metas = [], []
    for s in range(B):
        sm = samples[s]
        xs, e, cy = sm["xs"], sm["e"], sm["cy"]
        dinv, half, ltile, lane, nidg = (
            sm["dinv"], sm["half"], sm["ltile"], sm["lane"], sm["nidg"])

        dinv_pad = np.ones(2 * NPADH, np.float32)
        dinv_pad[nidg] = dinv

        ctab = np.zeros((NCPAD, 128), np.float32)
        ctab[:NC, 0:OUT] = cy
        ctab = ctab.astype(bfnp)

        for p in range(2):
            own = half == p
            lrow = ltile * 128 + lane          # local row id (own nodes)

            # edges into this half (excl self loops)
            emask = half[e[1]] == p
            erow = nidg[e[0][emask]]           # g_full row of source
            ecol_t = ltile[e[1][emask]]        # local dest tile
            ecol_l = (ltile * 128 + lane)[e[1][emask]] % 128   # dest lane
            ewt = dinv[e[0][emask]] * dinv[e[1][emask]]

            o = np.argsort(ecol_t, kind="stable")
            erow, ecol_t, ecol_l, ewt = erow[o], ecol_t[o], ecol_l[o], ewt[o]
            tstart = np.searchsorted(ecol_t, np.arange(NTH))
            cnts = (np.searchsorted(ecol_t, np.arange(NTH), side="right")
                    - tstart)
            assert (cnts <= P * 128).all(), "profile overflow"

            rank = np.arange(len(ecol_t)) - np.repeat(tstart, cnts)
            slot = (SOFF[ecol_t] * 128 + rank).astype(np.int64)

            grow = np.zeros(SLOT_TOT, np.int16)
            grow[slot] = erow.astype(np.int16)
            sT = np.zeros((SLOT_TOT, 128), np.float32)
            sT[slot, ecol_l] = ewt
            sTw = np.ascontiguousarray(
                sT.reshape(SLOT_TOT // 128, 128, 128).transpose(1, 0, 2)
            ).astype(bfnp)
            tmp = np.ascontiguousarray(grow.reshape(SLOT_TOT // 16, 16).T)
            groww = np.ascontiguousarray(np.tile(tmp, (8, 1)).astype(np.int16))

            # self-loop diag weights (dinv^2 per own slot; 1.0 on pads)
            dv2 = np.ones(NPADH, np.float32)
            dv2[lrow[own]] = dinv[own] ** 2
            dv2 = np.ascontiguousarray(dv2.reshape(NTH, 128).T)

            # node features / positions at local slots
            f01 = np.full((NPADH, 2), 1e3, np.float32)
            f01[lrow[own]] = xs[own][:, 0:2]
            xT3 = np.ones((3, NPADH), np.float32)
            xT3[0] = f01[:, 0]; xT3[1] = f01[:, 1]
            negf2 = np.ascontiguousarray(
                (-(f01[:, 0] ** 2 + f01[:, 1] ** 2)).reshape(NTH, 128).T)

            h0 = np.zeros((NPADH, 6), np.float32)
            h0[lrow[own], 0:D_IN] = xs[own]
            h0[lrow[own], D_IN] = sdf[own, 0]
            h0T = np.ascontiguousarray(h0.T).astype(bfnp)

            m = dict(common)
            m.update(xT3=xT3, negf2=negf2, h0T=h0T, sTw=sTw, groww=groww,
                     ctab=ctab, dv2=dv2)
            in_maps.append(m)
        metas.append(nidg)

    return in_maps, metas, tuple(P.tolist())


_prog_cache = {}


def kernel(**inputs):
    in_maps, metas, P = host_prep(inputs)
    if _prog_cache.get("P") != P:
        _prog_cache["nc"] = build_program(np.array(P))
        _prog_cache["P"] = P
    nc = _prog_cache["nc"]

    res = run_bass_kernel_spmd(nc, in_maps, list(range(N_CORES)))
    global _last_exec_ns, _last_trace
    _last_exec_ns = res.exec_time_ns
    _last_trace = res.instructions_and_trace

    out = np.empty((B * NF, OUT), np.float32)
    for s in range(B):
        full = np.concatenate([
            np.asarray(res.results[2 * s]["out_nm"]),
            np.asarray(res.results[2 * s + 1]["out_nm"]),
        ], axis=0)
        out[s * NF:(s + 1) * NF] = full[metas[s], 0:OUT]
    return out


# revision 23
# speedup vs baseline: 2.1022x; 1.0246x over previous
"""CFD-GCN Trainium2 kernel: 6-layer GCN on a batched random mesh graph +
KNN interpolation, distributed over 8 NeuronCores.

Each sample (4 total) is split across a PAIR of cores: core 2s owns node
tiles 0..78, core 2s+1 owns 79..157 (79 tiles of 128 nodes each). Dense
(h@W), KNN selection, interpolation and aggregation all run on the owned
half; a pair AllGather shares the dense output g each layer so gathers can
read any source node. Self-loops are folded into a per-tile diagonal
matmul instead of gather slots. Edge-gather descriptors are generated with
prepare_only + trigger_dma on 2 SWDGE queues.

Self-contained: hardcodes all shapes; the slice profile (slots per dest
tile) is derived from the inputs on first call and baked into the program.
kernel(**inputs) -> np.ndarray [80000, 3].
"""
import sys

sys.path.insert(0, "/opt/trn_rl_repo")

import numpy as np
import ml_dtypes

from concourse import bass, bacc
from concourse.bass_utils import run_bass_kernel_spmd
import concourse.mybir as mybir
from contextlib import ExitStack

f32, bf16 = mybir.dt.float32, mybir.dt.bfloat16
i16, u16 = mybir.dt.int16, mybir.dt.uint16
ALU = mybir.AluOpType
ACTF = mybir.ActivationFunctionType
bfnp = ml_dtypes.bfloat16

# ---------------- problem constants ----------------
B, NF, NC, H, D_IN, OUT = 4, 20000, 2000, 512, 5, 3
E_PER = 6 * NF
NT = 158                      # global node tiles per sample
NTH = 79                      # node tiles per core (half sample)
NPAD = NT * 128               # 20224
NPADH = NTH * 128             # 10112
NCPAD = 2048                  # padded coarse count
MAXSL = 8                     # max 128-slot edge slices per dest tile
RANGE_T = 16                  # node tiles per hT transpose-load range
ICH = 8                       # interp gather chunk (tiles)
N_CORES = 8
GB = 4                        # agg ring depth (gather bufs)
PHASE = 99                    # debug: truncate program after checkpoint N

LAYERS = [
    dict(kc6=True, fy=False, relu=True, e2=False),   # pre0
    dict(kc6=False, fy=False, relu=True, e2=False),  # pre1
    dict(kc6=False, fy=False, relu=True, e2=False),  # pre2
    dict(kc6=False, fy=True, relu=True, e2=False),   # end0
    dict(kc6=False, fy=False, relu=True, e2=False),  # end1
    dict(kc6=False, fy=False, relu=False, e2=True),  # end2
]

RG_PAIRS = [[0, 1], [2, 3], [4, 5], [6, 7]]


def _ranges():
    r, t0 = [], 0
    while t0 < NTH:
        r.append((t0, min(RANGE_T, NTH - t0)))
        t0 += RANGE_T
    return r


def build_program(Po, Pp):
    """Po/Pp: per-local-tile own/peer slice counts, identical on all cores."""
    P = Po + Pp
    SOFF = np.concatenate([[0], np.cumsum(P)]).astype(int)   # slice offsets
    SLOT_TOT = int(SOFF[-1]) * 128

    nc = bacc.Bacc(num_devices=N_CORES, num_swdge_queues=4)

    Din = {}
    def din(name, shape, dt):
        Din[name] = nc.declare_dram_parameter(name, list(shape), dt, isOutput=False)
    def dout(name, shape, dt):
        Din[name] = nc.declare_dram_parameter(name, list(shape), dt, isOutput=True)

    din("xT3", (3, NPADH), f32)
    din("cxT3", (3, NCPAD), f32)
    din("negf2", (128, NTH), f32)
    din("h0T", (6, NPADH), bf16)
    din("W0", (6, H), bf16)
    din("W1", (128, 4, H), bf16)      # p-major k-chunked
    din("W2", (128, 4, H), bf16)
    din("W3a", (128, 4, H), bf16)
    din("W3b", (3, H), bf16)
    din("W4", (128, 4, H), bf16)
    din("W5", (128, 4, 128), bf16)
    din("brows", (1, 6, H), bf16)
    din("ones1", (1, 128), bf16)
    din("identb", (128, 128), bf16)
    din("identf", (128, 128), f32)
    din("dv2", (128, NTH), f32)
    din("sTw", (128, SLOT_TOT // 128, 128), bf16)
    din("groww", (128, SLOT_TOT // 16), i16)
    din("ctab", (NCPAD, 128), bf16)

    g_half = nc.dram_tensor("g_half", [NPADH, H], bf16)
    g_full = nc.dram_tensor("g_full", [NPAD, H], bf16)
    g2_half = nc.dram_tensor("g2_half", [NPADH, 128], bf16)
    g2_full = nc.dram_tensor("g2_full", [NPAD, 128], bf16)
    h_d = nc.dram_tensor("h_d", [NPADH, H], bf16)
    fy_d = nc.dram_tensor("fy_d", [3, NPADH], bf16)
    dout("out_nm", (NPADH, 128), f32)

    es = ExitStack()
    def sb(name, shape, dt):
        return es.enter_context(nc.sbuf_tensor(name, list(shape), dt))
    def psum(name, shape, dt):
        return es.enter_context(nc.psum_tensor(name, list(shape), dt))

    xt_s = [sb(f"xt_s{i}", (3, 128), f32) for i in range(2)]
    cxT3_s = sb("cxT3_s", (3, NCPAD), f32)
    negf2_s = sb("negf2_s", (128, NTH), f32)
    h0_s = [sb(f"h0_s{i}", (6, 128), bf16) for i in range(2)]
    W0_s = sb("W0_s", (6, H), bf16)
    W1_s = sb("W1_s", (128, 4, H), bf16)
    W2_s = sb("W2_s", (128, 4, H), bf16)
    W3a_s = sb("W3a_s", (128, 4, H), bf16)
    W3b_s = sb("W3b_s", (3, H), bf16)
    W4_s = sb("W4_s", (128, 4, H), bf16)
    W5_s = sb("W5_s", (128, 4, 128), bf16)
    brows_s = sb("brows_s", (1, 6, H), bf16)
    ones1_s = sb("ones1_s", (1, 128), bf16)
    identb_s = sb("identb_s", (128, 128), bf16)
    identf_s = sb("identf_s", (128, 128), f32)
    dv2_s = sb("dv2_s", (128, NTH), f32)
    gro_s = [sb(f"gro_s{i}", (128, MAXSL * 8), i16) for i in range(GB)]

    hT_s = [sb(f"hT_s{i}", (128, 4, RANGE_T * 128), bf16) for i in range(2)]
    gsb_s = [sb(f"gsb_s{i}", (128, H), bf16) for i in range(4)]
    hsb_s = [sb(f"hsb_s{i}", (128, H), bf16) for i in range(4)]
    osb_s = [sb(f"osb_s{i}", (128, 128), f32) for i in range(2)]
    gath_s = [sb(f"gath_s{i}", (128, MAXSL, H), bf16) for i in range(GB)]
    gath2_s = [sb(f"gath2_s{i}", (128, MAXSL, 128), bf16) for i in range(GB)]
    sT_s = [sb(f"sT_s{i}", (128, MAXSL, 128), bf16) for i in range(GB)]
    gt_s = [sb(f"gt_s{i}", (128, H), bf16) for i in range(GB)]
    dgw_s = [sb(f"dgw_s{i}", (128, 128), bf16) for i in range(GB)]

    nd2_s = [sb(f"nd2_s{i}", (128, NCPAD), f32) for i in range(2)]
    bm_s = sb("bm_s", (128, 8, NTH), f32)
    bi_s = sb("bi_s", (128, 8, NTH), u16)
    d2c_s = sb("d2c_s", (128, 3, NTH), f32)
    w_s = sb("w_s", (128, 3, NTH), f32)
    wsum_s = sb("wsum_s", (128, NTH), f32)
    rs_s = sb("rs_s", (128, NTH), f32)
    wnb_s = sb("wnb_s", (128, 3, NTH), f32)
    wrap_s = sb("wrap_s", (128, 3, NTH, 8), u16)
    gk_s = [[sb(f"gk_s{k}_{i}", (128, ICH, 128), bf16) for i in range(2)]
            for k in range(3)]
    diag3_s = [sb(f"diag3_s{i}", (128, 3, 128), bf16) for i in range(2)]
    fyw_s = [sb(f"fyw_s{i}", (3, 128), bf16) for i in range(2)]
    fyr_s = [sb(f"fyr_s{i}", (3, 128), bf16) for i in range(2)]

    pz = [psum(f"pz{i}", (128, H), f32) for i in range(3)]
    pa = [psum(f"pa{i}", (128, H), f32) for i in range(4)]

    class Sem:
        def __init__(self, name):
            self.h = es.enter_context(nc.semaphore(name))
            self.n = 0
        def inc(self, k):
            self.n += k
            return (self.h, self.n)
        def now(self):
            return (self.h, self.n)

    class Ring:
        def __init__(self, name, n):
            self.sems = [Sem(f"{name}{i}") for i in range(n)]
            self.nslots = n
        def write(self, slot, k=16):
            s = self.sems[slot % self.nslots]
            return s.inc(k)
        def last(self, slot):
            s = self.sems[slot % self.nslots]
            return (s.h, s.n)
        def all(self):
            return [(s.h, s.n) for s in self.sems]

    def wait_all(engine, ring):
        for sv in ring.all():
            wait(engine, sv)

    s_in = Sem("s_in")
    s_gprep = Sem("s_gprep")
    s_iprep = Sem("s_iprep")
    s_cc = Sem("s_cc")
    s_kpe = Sem("s_kpe"); s_kact = Sem("s_kact"); s_kmax = Sem("s_kmax")
    s_wn = Sem("s_wn"); s_wrap = Sem("s_wrap")
    s_dg = Sem("s_dg"); s_dg2 = Sem("s_dg2")
    s_ipe = Sem("s_ipe"); s_fy = Sem("s_fy")
    s_zpe = Sem("s_zpe")
    s_zact = Sem("s_zact"); s_ape = Sem("s_ape"); s_aact = Sem("s_aact")

    Q = {e: [] for e in ("sync", "tensor", "vector", "scalar", "gpsimd")}
    checkpoints = []
    def checkpoint():
        checkpoints.append({e: len(Q[e]) for e in Q})
    def emit(engine, fn):
        Q[engine].append(fn)
    def wait(engine, semv):
        s, v = semv
        if v > 0:
            emit(engine, lambda e, s=s, v=v: e.wait_ge(s, v))

    r_gk = Ring("r_gk", 2)     # interp table gathers (per gk buf)
    r_xt = Ring("r_xt", 2)     # xT3 tile loads
    r_h0 = Ring("r_h0", 2)     # h0T tile loads
    r_gro = Ring("r_gro", GB)  # gather idx loads
    r_fyw = Ring("r_fyw", 2)   # finey dram writes
    r_fyr = Ring("r_fyr", 2)   # finey tile loads
    r_hT = Ring("r_hT", 2)     # transpose loads (per hT buf)
    r_g = Ring("r_g", GB)      # agg own gathers (per gath buf)
    r_gp = Ring("r_gp", GB)    # agg peer gathers (per gath buf)
    r_s = Ring("r_s", GB)      # S loads (per sT buf)
    r_gt = Ring("r_gt", GB)    # g self-tile loads
    r_gw = Ring("r_gw", 4)     # g_half dram writes (per gsb buf)
    r_hw = Ring("r_hw", 4)     # h dram writes (per hsb buf)
    r_ow = Ring("r_ow", 2)     # out writes (per osb buf)

    # ============ input loads ============
    loads = [
        (cxT3_s[:], "cxT3"), (negf2_s[:], "negf2"),
        (W0_s[:], "W0"), (W1_s[:], "W1"), (W2_s[:], "W2"),
        (W3a_s[:], "W3a"), (W3b_s[:], "W3b"), (W4_s[:], "W4"), (W5_s[:], "W5"),
        (brows_s[:], "brows"), (ones1_s[:], "ones1"), (identb_s[:], "identb"),
        (identf_s[:], "identf"), (dv2_s[:], "dv2"),
    ]
    for dst, srcn in loads:
        sm = s_in.inc(16)
        emit("sync", lambda e, d=dst, s=srcn, sm=sm: e.dma_start(
            out=d, in_=Din[s][:]).then_inc(sm[0], 16))
    IN_ALL = s_in.now()
    checkpoint()   # 0: loads

    # ============ KNN selection ============
    wait("tensor", IN_ALL)
    wait("scalar", IN_ALL)
    wait("vector", IN_ALL)
    NQ = NCPAD // 512
    for t in range(NTH):
        if t >= 2:
            wait("sync", (s_kpe.h, NQ * (t - 1)))
        sm = r_xt.write(t)
        emit("sync", lambda e, t=t, sm=sm: e.dma_start(
            out=xt_s[t % 2][:], in_=Din["xT3"][:, t * 128:(t + 1) * 128]
        ).then_inc(sm[0], 16))
        wait("tensor", r_xt.last(t))
        for q in range(NQ):
            gq = NQ * t + q
            if gq >= 3:
                wait("tensor", (s_kact.h, gq - 2))
            sm = s_kpe.inc(1)
            emit("tensor", lambda e, t=t, q=q, gq=gq, sm=sm: e.matmul(
                pz[gq % 3][:, 0:512], xt_s[t % 2][:],
                cxT3_s[:, q * 512:(q + 1) * 512],
                start=True, stop=True).then_inc(sm[0], 1))
        for q in range(NQ):
            gq = NQ * t + q
            wait("scalar", (s_kpe.h, gq + 1))
            if t >= 2 and q == 0:
                wait("scalar", (s_kmax.h, t - 1))
            sm = s_kact.inc(1)
            emit("scalar", lambda e, t=t, q=q, gq=gq, sm=sm: e.activation(
                nd2_s[t % 2][:, q * 512:(q + 1) * 512], pz[gq % 3][:, 0:512],
                ACTF.Identity, bias=negf2_s[:, t:t + 1], scale=1.0
            ).then_inc(sm[0], 1))
        wait("vector", (s_kact.h, NQ * (t + 1)))
        emit("vector", lambda e, t=t: e.max(bm_s[:, :, t], nd2_s[t % 2][:]))
        emit("vector", lambda e: e.drain())
        emit("vector", lambda e, t=t: e.max_index(
            bi_s[:, :, t], bm_s[:, :, t], nd2_s[t % 2][:]))
        sm = s_kmax.inc(1)
        emit("vector", lambda e, sm=sm: e.drain().then_inc(sm[0], 1))

    checkpoint()   # 1: knn select
    # weights on DVE
    emit("vector", lambda e: e.tensor_scalar(
        out=d2c_s[:], in0=bm_s[:, 0:3, :], scalar1=-1.0, scalar2=1e-16,
        op0=ALU.mult, op1=ALU.max))
    emit("vector", lambda e: e.drain())
    emit("vector", lambda e: e.reciprocal(w_s[:], d2c_s[:]))
    emit("vector", lambda e: e.drain())
    emit("vector", lambda e: e.tensor_reduce(
        out=wsum_s[:], in_=bass.AP(w_s, 0, [[3 * NTH, 128], [1, NTH], [NTH, 3]]),
        axis=mybir.AxisListType.X, op=ALU.add))
    emit("vector", lambda e: e.drain())
    emit("vector", lambda e: e.reciprocal(rs_s[:], wsum_s[:]))
    emit("vector", lambda e: e.drain())
    emit("vector", lambda e: e.tensor_tensor(
        out=wnb_s[:], in0=w_s[:],
        in1=bass.AP(rs_s, 0, [[NTH, 128], [0, 3], [1, NTH]]),
        op=ALU.mult))
    sm = s_wn.inc(1)
    emit("vector", lambda e, sm=sm: e.drain().then_inc(sm[0], 1))

    # wrapped idx build (gpsimd)
    wait("gpsimd", (s_kmax.h, NTH))
    for k in range(3):
        for g in range(8):
            sm = s_wrap.inc(16)
            emit("gpsimd", lambda e, k=k, g=g, sm=sm: e.dma_start(
                out=wrap_s[0:16, k, :, g],
                in_=bi_s[16 * g:16 * (g + 1), k, :],
            ).then_inc(sm[0], 16))
    wait("gpsimd", s_wrap.now())
    for rep in range(1, 8):
        sm = s_wrap.inc(16)
        emit("gpsimd", lambda e, rep=rep, sm=sm: e.dma_start(
            out=wrap_s[16 * rep:16 * (rep + 1)],
            in_=wrap_s[0:16],
        ).then_inc(sm[0], 16))
    WRAP_ALL = s_wrap.now()

    # interp
    wait("gpsimd", WRAP_ALL)
    wait("vector", s_wn.now())
    n_ich = (NTH + ICH - 1) // ICH
    for c in range(n_ich):
        t0 = c * ICH
        ntile = min(ICH, NTH - t0)
        for k in range(3):
            sm = r_gk.write(c)
            smp = s_iprep.inc(1)
            emit("gpsimd", lambda e, k=k, c=c, t0=t0, nt=ntile, sm=sm, smp=smp:
                 e.dma_gather(
                     out_ap=gk_s[k][c % 2][:, 0:nt, :],
                     in_ap=Din["ctab"][:],
                     idxs_ap=wrap_s[:, k, t0:t0 + nt, :].bitcast(i16),
                     num_idxs=nt * 128, num_idxs_reg=nt * 128,
                     elem_size=128,
                     prepare_only=True, sem=r_gk.sems[c % 2].h,
                     queue_num=c % 2,
                 ).then_inc(smp[0], 1))
        wait("gpsimd", s_iprep.now())
        if c >= 2:
            wait("gpsimd", (s_ipe.h, (c - 1) * ICH))
        emit("gpsimd", lambda e, c=c: e.trigger_dma(
            count=3, queue_num=c % 2))
        GK_NOW = r_gk.last(c)
        for tt in range(ntile):
            t = t0 + tt
            if t >= 2:
                wait("vector", (s_ipe.h, t - 1))
            emit("vector", lambda e, t=t: e.tensor_tensor(
                out=diag3_s[t % 2][:],
                in0=bass.AP(identf_s, 0, [[128, 128], [0, 3], [1, 128]]),
                in1=bass.AP(wnb_s, t, [[3 * NTH, 128], [NTH, 3], [0, 128]]),
                op=ALU.mult))
            sm = s_dg.inc(1)
            emit("vector", lambda e, sm=sm: e.drain().then_inc(sm[0], 1))
            wait("tensor", GK_NOW)
            wait("tensor", (s_dg.h, s_dg.n))
            if t >= 4:
                wait("tensor", (s_fy.h, t - 3))    # psum WAR
            for k in range(3):
                sm = s_ipe.inc(1) if k == 2 else None
                def mk_interp(t=t, tt=tt, k=k, c=c, sm=sm):
                    def f(e):
                        ins = e.matmul(
                            pa[t % 4][:, 0:128], gk_s[k][c % 2][:, tt, :],
                            diag3_s[t % 2][:, k, :],
                            start=(k == 0), stop=(k == 2))
                        if sm:
                            ins.then_inc(sm[0], 1)
                    return f
                emit("tensor", mk_interp())
            wait("scalar", (s_ipe.h, s_ipe.n))
            wait("scalar", r_fyw.last(t))
            sm = s_fy.inc(1)
            emit("scalar", lambda e, t=t, sm=sm: e.activation(
                fyw_s[t % 2][:], pa[t % 4][0:3, 0:128],
                ACTF.Copy, bias=0.0, scale=1.0).then_inc(sm[0], 1))
            wait("sync", (s_fy.h, s_fy.n))
            sm = r_fyw.write(t)
            emit("sync", lambda e, t=t, sm=sm: e.dma_start(
                out=fy_d[:, t * 128:(t + 1) * 128],
                in_=fyw_s[t % 2][:]).then_inc(sm[0], 16))
    FY_ALL = s_fy.now()
    KACT_ALL = s_kact.now()
    checkpoint()   # 2: interp

    # ============ GCN layers ============
    WCH = {1: W1_s, 2: W2_s, 3: W3a_s, 4: W4_s, 5: W5_s}

    for li, L in enumerate(LAYERS):
        width = 128 if L["e2"] else H
        ghalf = g2_half if L["e2"] else g_half
        gfull = g2_full if L["e2"] else g_full

        # ---------- dense (own half) ----------
        zpe_base = s_zpe.n
        zact_base = s_zact.n
        cc_prev = s_cc.n          # AG of previous layer
        rg_prev = r_g.all()       # prev-layer own gathers reading g_half

        def dense_epilogue(t, width=width, ghalf=ghalf, cc_prev=cc_prev,
                           rg_prev=rg_prev):
            wait("scalar", (s_zpe.h, zpe_base + t + 1))
            wait("scalar", r_gw.last(t))
            sm = s_zact.inc(1)
            emit("scalar", lambda e, t=t, w=width, sm=sm: e.activation(
                gsb_s[t % 4][:, 0:w], pz[t % 3][:, 0:w], ACTF.Copy,
                bias=0.0, scale=1.0).then_inc(sm[0], 1))
            wait("sync", (s_zact.h, s_zact.n))
            if t == 0:
                wait("sync", (s_cc.h, cc_prev))   # WAR vs prev AG read
                for sv in rg_prev:                # WAR vs prev own gathers
                    wait("sync", sv)
            sm = r_gw.write(t)
            emit("sync", lambda e, t=t, gd=ghalf, w=width, sm=sm: e.dma_start(
                out=gd[t * 128:(t + 1) * 128, :],
                in_=gsb_s[t % 4][:, 0:w]).then_inc(sm[0], 16))

        if li == 0:
            wait("tensor", KACT_ALL)      # pz WAR vs KNN ACT
            for t in range(NTH):
                if t >= 2:
                    wait("sync", (s_zpe.h, zpe_base + t - 1))
                sm = r_h0.write(t)
                emit("sync", lambda e, t=t, sm=sm: e.dma_start(
                    out=h0_s[t % 2][:], in_=Din["h0T"][:, t * 128:(t + 1) * 128]
                ).then_inc(sm[0], 16))
                wait("tensor", r_h0.last(t))
                wait("tensor", (s_zact.h,
                                zact_base if t < 3 else zact_base + t - 2))
                sm = s_zpe.inc(1)
                emit("tensor", lambda e, t=t, sm=sm: e.matmul(
                    pz[t % 3][:, 0:H], h0_s[t % 2][:],
                    W0_s[:], start=True, stop=True).then_inc(sm[0], 1))
                dense_epilogue(t)
        else:
            Wl = WCH[li]
            range_zpe = []
            for ri, (rt0, rnt) in enumerate(_ranges()):
                wait("sync", (s_zpe.h,
                              zpe_base if ri < 2 else range_zpe[ri - 2]))
                for cch in range(4):
                    sm = r_hT.write(ri)
                    emit("sync", lambda e, ri=ri, rt0=rt0, rnt=rnt, c=cch, sm=sm:
                         e.dma_start_transpose(
                             hT_s[ri % 2][:, c, 0:rnt * 128],
                             h_d[rt0 * 128:(rt0 + rnt) * 128,
                                 c * 128:(c + 1) * 128],
                         ).then_inc(sm[0], 16))
                wait("tensor", r_hT.last(ri))
                if li == 3 and ri == 0:
                    wait_all("sync", r_fyw)
                for tt in range(rnt):
                    t = rt0 + tt
                    range_last = (tt == rnt - 1)
                    if L["fy"]:
                        if t >= 2:
                            wait("sync", (s_zpe.h, zpe_base + t - 1))
                        sm = r_fyr.write(t)
                        emit("sync", lambda e, t=t, sm=sm: e.dma_start(
                            out=fyr_s[t % 2][:],
                            in_=fy_d[:, t * 128:(t + 1) * 128]
                        ).then_inc(sm[0], 16))
                    wait("tensor", (s_zact.h,
                                    zact_base if t < 3 else zact_base + t - 2))
                    for cch in range(4):
                        last = (cch == 3) and not L["fy"]
                        sm = s_zpe.inc(1) if last else None
                        def mk_dense(t=t, tt=tt, ri=ri, cch=cch, Wl=Wl,
                                     w=width, last=last, sm=sm):
                            def f(e):
                                ins = e.matmul(
                                    pz[t % 3][:, 0:w],
                                    hT_s[ri % 2][:, cch,
                                                 tt * 128:(tt + 1) * 128],
                                    Wl[:, cch, 0:w],
                                    start=(cch == 0), stop=last)
                                if sm:
                                    ins.then_inc(sm[0], 1)
                            return f
                        emit("tensor", mk_dense())
                    if L["fy"]:
                        wait("tensor", r_fyr.last(t))
                        sm = s_zpe.inc(1)
                        emit("tensor", lambda e, t=t, sm=sm: e.matmul(
                            pz[t % 3][:, 0:H],
                            fyr_s[t % 2][:],
                            W3b_s[:], start=False, stop=True).then_inc(sm[0], 1))
                    if range_last:
                        range_zpe.append(s_zpe.n)
                    dense_epilogue(t)
        checkpoint()   # dense of this layer done

        # ---------- AllGather g_half -> g_full ----------
        wait_all("gpsimd", r_gw)          # all dense writes landed
        for sv in r_gp.all():             # WAR: prev-layer peer gathers
            wait("gpsimd", sv)
        RGW_ALL = r_gw.all()
        sm = s_cc.inc(1)
        emit("gpsimd", lambda e, hh=ghalf, ff=gfull, sm=sm: e.collective_compute(
            "AllGather",
            ALU.bypass,
            replica_groups=RG_PAIRS,
            ins=[hh.ap().opt()],
            outs=[ff.ap().opt()],
        ).then_inc(sm[0], 1))
        CC_NOW = s_cc.now()

        # ---------- agg (own dest tiles) ----------
        gbufs = gath2_s if L["e2"] else gath_s
        ape_base = s_ape.n
        aact_base = s_aact.n
        gprep_base = s_gprep.n
        if li == 0:
            wait("tensor", (s_fy.h, NTH))   # pa WAR vs interp
        for t in range(NTH):
            po, pp = int(Po[t]), int(Pp[t])
            nsl = po + pp
            # --- idx load (sync): gro buf freed once prep of t-GB ran ---
            wait("sync", (s_gprep.h,
                          gprep_base if t < GB
                          else gprep_base + 2 * (t - GB) + 2))
            sm = r_gro.write(t)
            emit("sync", lambda e, t=t, nsl=nsl, sm=sm: e.dma_start(
                out=gro_s[t % GB][:, 0:nsl * 8],
                in_=Din["groww"][:, SOFF[t] * 8:(SOFF[t] + nsl) * 8]
            ).then_inc(sm[0], 16))
            # --- S load (sync) ---
            wait("sync", (s_ape.h,
                          ape_base if t < GB else ape_base + t - GB + 1))
            sm = r_s.write(t)
            emit("sync", lambda e, t=t, nsl=nsl, sm=sm: e.dma_start(
                out=sT_s[t % GB][:, 0:nsl, :],
                in_=Din["sTw"][:, SOFF[t]:SOFF[t] + nsl, :]).then_inc(sm[0], 16))
            # --- g self tile load (sync) ---
            wait("sync", r_gw.last(t))         # dense write t landed
            sm = r_gt.write(t)
            emit("sync", lambda e, t=t, gd=ghalf, w=width, sm=sm: e.dma_start(
                out=gt_s[t % GB][:, 0:w],
                in_=gd[t * 128:(t + 1) * 128, :]).then_inc(sm[0], 16))
            # --- gather preps (gpsimd): own half + peer half ---
            wait("gpsimd", r_gro.last(t))
            smo = r_g.write(t)
            smp = s_gprep.inc(1)
            emit("gpsimd", lambda e, t=t, po=po, gh=ghalf, gb=gbufs, w=width,
                 smp=smp: e.dma_gather(
                     out_ap=gb[t % GB][:, 0:po, 0:w],
                     in_ap=gh[:],
                     idxs_ap=gro_s[t % GB][:, 0:po * 8],
                     num_idxs=po * 128, num_idxs_reg=po * 128, elem_size=w,
                     single_packet=False,
                     prepare_only=True, sem=r_g.sems[t % GB].h,
                     queue_num=t % 2,
                 ).then_inc(smp[0], 1))
            smq = r_gp.write(t)
            smp = s_gprep.inc(1)
            emit("gpsimd", lambda e, t=t, po=po, pp=pp, gf=gfull, gb=gbufs,
                 w=width, smp=smp: e.dma_gather(
                     out_ap=gb[t % GB][:, po:po + pp, 0:w],
                     in_ap=gf[:],
                     idxs_ap=gro_s[t % GB][:, po * 8:(po + pp) * 8],
                     num_idxs=pp * 128, num_idxs_reg=pp * 128, elem_size=w,
                     single_packet=False,
                     prepare_only=True, sem=r_gp.sems[t % GB].h,
                     queue_num=2 + t % 2,
                 ).then_inc(smp[0], 1))
            wait("gpsimd", s_gprep.now())
            # own trigger: needs gath buf free + ALL own dense writes
            wait("gpsimd", (s_ape.h,
                            ape_base if t < GB else ape_base + t - GB + 1))
            if t == 0:
                for sv in RGW_ALL:
                    wait("gpsimd", sv)
            emit("gpsimd", lambda e, t=t: e.trigger_dma(
                count=1, queue_num=t % 2))
            # peer trigger: additionally needs the AllGather
            if t == 0:
                wait("gpsimd", CC_NOW)
            emit("gpsimd", lambda e, t=t: e.trigger_dma(
                count=1, queue_num=2 + t % 2))
            # --- diag weight build (vector) ---
            wait("vector", (s_ape.h,
                            ape_base if t < GB else ape_base + t - GB + 1))
            emit("vector", lambda e, t=t: e.tensor_scalar(
                out=dgw_s[t % GB][:], in0=identb_s[:],
                scalar1=dv2_s[:, t:t + 1], scalar2=None, op0=ALU.mult))
            sm = s_dg2.inc(1)
            emit("vector", lambda e, sm=sm: e.drain().then_inc(sm[0], 1))
            # --- matmuls (tensor): own slices + diag, then peer, then bias ---
            wait("tensor", r_g.last(t))
            wait("tensor", r_s.last(t))
            wait("tensor", r_gt.last(t))
            wait("tensor", (s_dg2.h, s_dg2.n))
            wait("tensor", (s_aact.h,
                            aact_base if t < 4 else aact_base + t - 3))
            for sl in range(po):
                emit("tensor", lambda e, t=t, sl=sl, gb=gbufs, w=width:
                     e.matmul(
                         pa[t % 4][:, 0:w],
                         sT_s[t % GB][:, sl, :],
                         gb[t % GB][:, sl, 0:w],
                         start=(sl == 0), stop=False))
            emit("tensor", lambda e, t=t, w=width: e.matmul(
                pa[t % 4][:, 0:w], dgw_s[t % GB][:],
                gt_s[t % GB][:, 0:w], start=False, stop=False))
            wait("tensor", r_gp.last(t))
            for sl in range(po, po + pp):
                emit("tensor", lambda e, t=t, sl=sl, gb=gbufs, w=width:
                     e.matmul(
                         pa[t % 4][:, 0:w],
                         sT_s[t % GB][:, sl, :],
                         gb[t % GB][:, sl, 0:w],
                         start=False, stop=False))
            sm = s_ape.inc(1)
            emit("tensor", lambda e, t=t, li=li, w=width, sm=sm: e.matmul(
                pa[t % 4][:, 0:w], ones1_s[:],
                brows_s[:, li, 0:w], start=False, stop=True
            ).then_inc(sm[0], 1))
            # --- epilogue (scalar + sync) ---
            wait("scalar", (s_ape.h, s_ape.n))
            if L["e2"]:
                wait("scalar", r_ow.last(t))
            else:
                wait("scalar", r_hw.last(t))
            sm = s_aact.inc(1)
            if L["e2"]:
                emit("scalar", lambda e, t=t, sm=sm: e.activation(
                    osb_s[t % 2][:], pa[t % 4][:, 0:128], ACTF.Copy,
                    bias=0.0, scale=1.0).then_inc(sm[0], 1))
            else:
                emit("scalar", lambda e, t=t, sm=sm: e.activation(
                    hsb_s[t % 4][:], pa[t % 4][:, 0:H], ACTF.Relu,
                    bias=0.0, scale=1.0).then_inc(sm[0], 1))
            wait("sync", (s_aact.h, s_aact.n))
            if L["e2"]:
                sm = r_ow.write(t)
                emit("sync", lambda e, t=t, sm=sm: e.dma_start(
                    out=Din["out_nm"][t * 128:(t + 1) * 128, :],
                    in_=osb_s[t % 2][:]).then_inc(sm[0], 16))
            else:
                sm = r_hw.write(t)
                emit("sync", lambda e, t=t, sm=sm: e.dma_start(
                    out=h_d[t * 128:(t + 1) * 128, :],
                    in_=hsb_s[t % 4][:]).then_inc(sm[0], 16))
        if not L["e2"]:
            wait_all("sync", r_hw)   # barrier before next layer's hT loads
        checkpoint()   # 3+li

    wait_all("sync", r_ow)
    wait_all("sync", r_hw)
    checkpoint()
    if PHASE < len(checkpoints):
        cut = checkpoints[PHASE]
        for e in Q:
            Q[e] = Q[e][:cut[e]]

    with nc.allow_non_contiguous_dma(reason="wrapped idx build"), \
            nc.Block() as block:
        @block.sync
        def _(e):
            for fn in Q["sync"]:
                fn(e)

        @block.tensor
        def _(e):
            for fn in Q["tensor"]:
                fn(e)

        @block.vector
        def _(e):
            for fn in Q["vector"]:
                fn(e)

        @block.scalar
        def _(e):
            for fn in Q["scalar"]:
                fn(e)

        @block.gpsimd
        def _(e):
            for fn in Q["gpsimd"]:
                fn(e)

    nc.finalize()
    return nc


# ================= host side =================

def host_prep(inputs):
    x = np.asarray(inputs["x"], np.float32)
    sdf = np.asarray(inputs["sdf"], np.float32)
    edge_index = np.asarray(inputs["edge_index"], np.int64)
    coarse_x = np.asarray(inputs["coarse_x"], np.float32)
    coarse_y = np.asarray(inputs["coarse_y"], np.float32)
    Ws = {k: np.asarray(inputs[k], np.float32) for k in (
        "pre_W0", "pre_W1", "pre_W2", "end_W0", "end_W1", "end_W2")}
    bs = {k: np.asarray(inputs[k], np.float32) for k in (
        "pre_b0", "pre_b1", "pre_b2", "end_b0", "end_b1", "end_b2")}

    cxT3 = np.zeros((3, NCPAD), np.float32)
    cxT3[0, :NC] = 2 * coarse_x[:, 0]
    cxT3[1, :NC] = 2 * coarse_x[:, 1]
    cxT3[2, :NC] = -(coarse_x[:, 0] ** 2 + coarse_x[:, 1] ** 2)
    cxT3[0, NC:] = 2e4; cxT3[1, NC:] = 2e4; cxT3[2, NC:] = -2e8

    brows = np.zeros((6, H), np.float32)
    for i, k in enumerate(("pre_b0", "pre_b1", "pre_b2", "end_b0", "end_b1")):
        brows[i] = bs[k]
    brows[5, :OUT] = bs["end_b2"]

    W5 = np.zeros((H, 128), np.float32)
    W5[:, :OUT] = Ws["end_W2"]

    def pmaj(w):   # [512, X] -> [128, 4, X]
        return np.ascontiguousarray(
            w.reshape(4, 128, w.shape[1]).transpose(1, 0, 2))

    common = dict(
        cxT3=cxT3,
        W0=Ws["pre_W0"].astype(bfnp),
        W1=pmaj(Ws["pre_W1"]).astype(bfnp),
        W2=pmaj(Ws["pre_W2"]).astype(bfnp),
        W3a=pmaj(Ws["end_W0"][OUT:]).astype(bfnp),
        W3b=Ws["end_W0"][:OUT].astype(bfnp),
        W4=pmaj(Ws["end_W1"]).astype(bfnp),
        W5=pmaj(W5).astype(bfnp),
        brows=brows.astype(bfnp)[None],
        ones1=np.ones((1, 128), bfnp),
        identb=np.eye(128, dtype=np.float32).astype(bfnp),
        identf=np.eye(128, dtype=np.float32),
    )

    # ---- pass 1: per-sample node->slot assignment + per-core tile stats ----
    samples = []
    for s in range(B):
        xs = x[s * NF:(s + 1) * NF]
        e = edge_index[:, s * E_PER:(s + 1) * E_PER] - s * NF
        cy = coarse_y[s * NC:(s + 1) * NC]

        deg = np.bincount(e[1], minlength=NF).astype(np.float32) + 1.0
        dinv = (1.0 / np.sqrt(deg)).astype(np.float32)

        # balanced global tile assignment (snake over degree-sorted nodes)
        order = np.argsort(-deg, kind="stable")
        tile_seq = np.arange(NT)
        snake = np.concatenate([tile_seq, tile_seq[::-1]])
        bins = np.resize(snake, NF)
        gtile = np.empty(NF, np.int64)   # node -> global tile
        lane = np.empty(NF, np.int64)
        for t in range(NT):
            sel = np.where(bins == t)[0]
            gtile[order[sel]] = t
            lane[order[sel]] = np.arange(len(sel))

        # in-edge count per global tile (excl self loops)
        cin = np.bincount(gtile[e[1]], minlength=NT)

        # per half: order local tiles by in-edge count desc
        half = (gtile >= NTH).astype(np.int64)
        ltile = np.empty(NF, np.int64)
        sco = np.zeros((2, NTH), np.int64)   # own-source slices per tile
        scp = np.zeros((2, NTH), np.int64)   # peer-source slices per tile
        src_half = half[e[0]]
        dst_half = half[e[1]]
        for p in range(2):
            gts = np.arange(p * NTH, (p + 1) * NTH)
            perm = gts[np.argsort(-cin[gts], kind="stable")]
            inv = np.empty(NTH, np.int64)
            inv[perm - p * NTH] = np.arange(NTH)
            mask = half == p
            ltile[mask] = inv[gtile[mask] - p * NTH]
        for p in range(2):
            emask = dst_half == p
            dt_ = ltile[e[1][emask]]
            own = src_half[emask] == p
            co = np.bincount(dt_[own], minlength=NTH)
            cp = np.bincount(dt_[~own], minlength=NTH)
            sco[p] = (np.ceil(co / 128)).astype(np.int64)
            scp[p] = (np.ceil(cp / 128)).astype(np.int64)

        # node -> row in g_full (AllGather layout)
        nidg = half * NPADH + ltile * 128 + lane
        samples.append(dict(xs=xs, e=e, cy=cy, dinv=dinv, half=half,
                            ltile=ltile, lane=lane, nidg=nidg,
                            sco=sco, scp=scp))

    # slice profiles: elementwise max over all 8 cores
    Po = np.zeros(NTH, np.int64)
    Pp = np.zeros(NTH, np.int64)
    for sm in samples:
        Po = np.maximum(Po, sm["sco"].max(axis=0))
        Pp = np.maximum(Pp, sm["scp"].max(axis=0))
    Po = np.maximum(Po, 1)
    Pp = np.maximum(Pp, 1)
    P = Po + Pp
    assert P.max() <= MAXSL, f"slice overflow {P.max()}"
    SOFF = np.concatenate([[0], np.cumsum(P)]).astype(int)
    SLOT_TOT = int(SOFF[-1]) * 128

    # ---- pass 2: per-core arrays ----
    in_maps, metas = [], []
    for s in range(B):
        sm = samples[s]
        xs, e, cy = sm["xs"], sm["e"], sm["cy"]
        dinv, half, ltile, lane, nidg = (
            sm["dinv"], sm["half"], sm["ltile"], sm["lane"], sm["nidg"])

        dinv_pad = np.ones(2 * NPADH, np.float32)
        dinv_pad[nidg] = dinv

        ctab = np.zeros((NCPAD, 128), np.float32)
        ctab[:NC, 0:OUT] = cy
        ctab = ctab.astype(bfnp)

        for p in range(2):
            own = half == p
            lrow = ltile * 128 + lane          # local row id (own nodes)

            grow = np.zeros(SLOT_TOT, np.int16)
            sT = np.zeros((SLOT_TOT, 128), np.float32)
            # two slot groups per tile: own sources (local g_half rows) at
            # SOFF[t]*128, peer sources (g_full rows) at (SOFF[t]+Po[t])*128
            for grp in range(2):
                emask = (half[e[1]] == p) & ((half[e[0]] == p) == (grp == 0))
                if grp == 0:
                    erow = (ltile * 128 + lane)[e[0][emask]]   # local row
                else:
                    erow = nidg[e[0][emask]]                   # g_full row
                ecol_t = ltile[e[1][emask]]        # local dest tile
                ecol_l = lrow[e[1][emask]] % 128   # dest lane
                ewt = dinv[e[0][emask]] * dinv[e[1][emask]]

                o = np.argsort(ecol_t, kind="stable")
                erow, ecol_t, ecol_l, ewt = (
                    erow[o], ecol_t[o], ecol_l[o], ewt[o])
                tstart = np.searchsorted(ecol_t, np.arange(NTH))
                cnts = (np.searchsorted(ecol_t, np.arange(NTH), side="right")
                        - tstart)
                cap = Po if grp == 0 else Pp
                assert (cnts <= cap * 128).all(), "profile overflow"

                rank = np.arange(len(ecol_t)) - np.repeat(tstart, cnts)
                base = SOFF[ecol_t] + (0 if grp == 0 else Po[ecol_t])
                slot = (base * 128 + rank).astype(np.int64)
                grow[slot] = erow.astype(np.int16)
                sT[slot, ecol_l] = ewt
            sTw = np.ascontiguousarray(
                sT.reshape(SLOT_TOT // 128, 128, 128).transpose(1, 0, 2)
            ).astype(bfnp)
            tmp = np.ascontiguousarray(grow.reshape(SLOT_TOT // 16, 16).T)
            groww = np.ascontiguousarray(np.tile(tmp, (8, 1)).astype(np.int16))

            # self-loop diag weights (dinv^2 per own slot; 1.0 on pads)
            dv2 = np.ones(NPADH, np.float32)
            dv2[lrow[own]] = dinv[own] ** 2
            dv2 = np.ascontiguousarray(dv2.reshape(NTH, 128).T)

            # node features / positions at local slots
            f01 = np.full((NPADH, 2), 1e3, np.float32)
            f01[lrow[own]] = xs[own][:, 0:2]
            xT3 = np.ones((3, NPADH), np.float32)
            xT3[0] = f01[:, 0]; xT3[1] = f01[:, 1]
            negf2 = np.ascontiguousarray(
                (-(f01[:, 0] ** 2 + f01[:, 1] ** 2)).reshape(NTH, 128).T)

            h0 = np.zeros((NPADH, 6), np.float32)
            h0[lrow[own], 0:D_IN] = xs[own]
            h0[lrow[own], D_IN] = sdf[own, 0]
            h0T = np.ascontiguousarray(h0.T).astype(bfnp)

            m = dict(common)
            m.update(xT3=xT3, negf2=negf2, h0T=h0T, sTw=sTw, groww=groww,
                     ctab=ctab, dv2=dv2)
            in_maps.append(m)
        metas.append(nidg)

    return in_maps, metas, (tuple(Po.tolist()), tuple(Pp.tolist()))


_prog_cache = {}


def kernel(**inputs):
    in_maps, metas, P = host_prep(inputs)
    if _prog_cache.get("P") != P:
        _prog_cache["nc"] = build_program(np.array(P[0]), np.array(P[1]))
        _prog_cache["P"] = P
    nc = _prog_cache["nc"]

    res = run_bass_kernel_spmd(nc, in_maps, list(range(N_CORES)))
    global _last_exec_ns, _last_trace
    _last_exec_ns = res.exec_time_ns
    _last_trace = res.instructions_and_trace

    out = np.empty((B * NF, OUT), np.float32)
    for s in range(B):
        full = np.concatenate([
            np.asarray(res.results[2 * s]["out_nm"]),
            np.asarray(res.results[2 * s + 1]["out_nm"]),
        ], axis=0)
        out[s * NF:(s + 1) * NF] = full[metas[s], 0:OUT]
    return out


# revision 32
# speedup vs baseline: 2.2078x; 1.0502x over previous
"""CFD-GCN Trainium2 kernel: 6-layer GCN on a batched random mesh graph +
KNN interpolation, distributed over 8 NeuronCores.

Each sample (4 total) is split across a PAIR of cores: core 2s owns node
tiles 0..78, core 2s+1 owns 79..157 (79 tiles of 128 nodes). Dense (h@W),
KNN selection, interpolation and aggregation all run on the owned half.
Per layer, a 2-chunk pair AllGather publishes the dense output g; each
tile's edge gather is split into an own-half gather (reads local g_half,
no collective wait) and a peer-half gather (reads g_full, waits the
AllGather). Self-loops ride in the own-gather slots. Descriptors are
generated with prepare_only + trigger_dma on 4 SWDGE queues. g_half and
g_full ping-pong across layers so dense l overlaps aggregation l-1
(per-tile h-write marks instead of a layer barrier); the dense
psum->SBUF copy runs on the vector engine to stay clear of the scalar
queue.

Self-contained: hardcodes all shapes; the slice profiles (own/peer slots
per dest tile) are derived from the inputs on first call and baked into
the program. kernel(**inputs) -> np.ndarray [80000, 3].
"""
import sys

sys.path.insert(0, "/opt/trn_rl_repo")

import numpy as np
import ml_dtypes

from concourse import bass, bacc
from concourse.bass_utils import run_bass_kernel_spmd
import concourse.mybir as mybir
from contextlib import ExitStack

f32, bf16 = mybir.dt.float32, mybir.dt.bfloat16
i16, u16 = mybir.dt.int16, mybir.dt.uint16
ALU = mybir.AluOpType
ACTF = mybir.ActivationFunctionType
bfnp = ml_dtypes.bfloat16

# ---------------- problem constants ----------------
B, NF, NC, H, D_IN, OUT = 4, 20000, 2000, 512, 5, 3
E_PER = 6 * NF
NT = 158                      # global node tiles per sample
NTH = 79                      # node tiles per core (half sample)
NPAD = NT * 128               # 20224
NPADH = NTH * 128             # 10112
NCPAD = 2048                  # padded coarse count
RANGE_T = 16                  # node tiles per hT transpose-load range
ICH = 8                       # interp gather chunk (tiles)
N_CORES = 8
GB = 4                        # agg ring depth (gather bufs)
CHK = 40                      # AllGather chunk boundary (tiles)
PHASE = 99                    # debug: truncate program after checkpoint N

LAYERS = [
    dict(kc6=True, fy=False, relu=True, e2=False),   # pre0
    dict(kc6=False, fy=False, relu=True, e2=False),  # pre1
    dict(kc6=False, fy=False, relu=True, e2=False),  # pre2
    dict(kc6=False, fy=True, relu=True, e2=False),   # end0
    dict(kc6=False, fy=False, relu=True, e2=False),  # end1
    dict(kc6=False, fy=False, relu=False, e2=True),  # end2
]

RG_PAIRS = [[0, 1], [2, 3], [4, 5], [6, 7]]


def _ranges():
    r, t0 = [], 0
    while t0 < NTH:
        r.append((t0, min(RANGE_T, NTH - t0)))
        t0 += RANGE_T
    return r


def build_program(Po, Pp):
    """Po/Pp: per-local-tile own/peer slice counts, identical on all cores."""
    P = Po + Pp
    SOFF = np.concatenate([[0], np.cumsum(P)]).astype(int)      # slice offs
    SOFFC = (SOFF * 136).astype(int)                            # sgt col offs
    MAXP = int(P.max())

    nc = bacc.Bacc(num_devices=N_CORES, num_swdge_queues=4)

    Din = {}
    def din(name, shape, dt):
        Din[name] = nc.declare_dram_parameter(name, list(shape), dt, isOutput=False)
    def dout(name, shape, dt):
        Din[name] = nc.declare_dram_parameter(name, list(shape), dt, isOutput=True)

    din("xT3", (3, NPADH), f32)
    din("cxT3", (3, NCPAD), f32)
    din("negf2", (128, NTH), f32)
    din("h0T", (6, NPADH), bf16)
    din("W0", (6, H), bf16)
    din("W1", (128, 4, H), bf16)      # p-major k-chunked
    din("W2", (128, 4, H), bf16)
    din("W3a", (128, 4, H), bf16)
    din("W3b", (3, H), bf16)
    din("W4", (128, 4, H), bf16)
    din("W5", (128, 4, 128), bf16)
    din("brows", (1, 6, H), bf16)
    din("ones1", (1, 128), bf16)
    din("identf", (128, 128), f32)
    din("sgt", (128, int(SOFFC[-1])), bf16)   # per tile: S p-major | idxs
    din("ctab", (NCPAD, 128), bf16)

    g_half = [nc.dram_tensor(f"g_half{i}", [NPADH, H], bf16) for i in range(2)]
    g_full = [nc.dram_tensor(f"g_full{i}", [NPAD, H], bf16) for i in range(2)]
    g2_half = nc.dram_tensor("g2_half", [NPADH, 128], bf16)
    g2_full = nc.dram_tensor("g2_full", [NPAD, 128], bf16)
    h_d = nc.dram_tensor("h_d", [NPADH, H], bf16)
    fy_d = nc.dram_tensor("fy_d", [3, NPADH], bf16)
    dout("out_nm", (NPADH, 128), f32)

    es = ExitStack()
    def sb(name, shape, dt):
        return es.enter_context(nc.sbuf_tensor(name, list(shape), dt))
    def psum(name, shape, dt):
        return es.enter_context(nc.psum_tensor(name, list(shape), dt))

    xt_s = [sb(f"xt_s{i}", (3, 128), f32) for i in range(2)]
    cxT3_s = sb("cxT3_s", (3, NCPAD), f32)
    negf2_s = sb("negf2_s", (128, NTH), f32)
    h0_s = [sb(f"h0_s{i}", (6, 128), bf16) for i in range(2)]
    W0_s = sb("W0_s", (6, H), bf16)
    W1_s = sb("W1_s", (128, 4, H), bf16)
    W2_s = sb("W2_s", (128, 4, H), bf16)
    W3a_s = sb("W3a_s", (128, 4, H), bf16)
    W3b_s = sb("W3b_s", (3, H), bf16)
    W4_s = sb("W4_s", (128, 4, H), bf16)
    W5_s = sb("W5_s", (128, 4, 128), bf16)
    brows_s = sb("brows_s", (1, 6, H), bf16)
    ones1_s = sb("ones1_s", (1, 128), bf16)
    identf_s = sb("identf_s", (128, 128), f32)

    hT_s = [sb(f"hT_s{i}", (128, 4, RANGE_T * 128), bf16) for i in range(2)]
    gsb_s = [sb(f"gsb_s{i}", (128, H), bf16) for i in range(4)]
    hsb_s = [sb(f"hsb_s{i}", (128, H), bf16) for i in range(4)]
    osb_s = [sb(f"osb_s{i}", (128, 128), f32) for i in range(2)]
    gath_s = [sb(f"gath_s{i}", (128, MAXP, H), bf16) for i in range(GB)]
    gath2_s = [sb(f"gath2_s{i}", (128, MAXP, 128), bf16) for i in range(GB)]
    sgt_s = [sb(f"sgt_s{i}", (128, MAXP * 136), bf16) for i in range(GB)]

    nd2_s = [sb(f"nd2_s{i}", (128, NCPAD), f32) for i in range(2)]
    bm_s = sb("bm_s", (128, 8, NTH), f32)
    bi_s = sb("bi_s", (128, 8, NTH), u16)
    d2c_s = sb("d2c_s", (128, 3, NTH), f32)
    w_s = sb("w_s", (128, 3, NTH), f32)
    wsum_s = sb("wsum_s", (128, NTH), f32)
    rs_s = sb("rs_s", (128, NTH), f32)
    wnb_s = sb("wnb_s", (128, 3, NTH), f32)
    wrap_s = sb("wrap_s", (128, 3, NTH, 8), u16)
    gk_s = [[sb(f"gk_s{k}_{i}", (128, ICH, 128), bf16) for i in range(2)]
            for k in range(3)]
    diag3_s = [sb(f"diag3_s{i}", (128, 3, 128), bf16) for i in range(2)]
    fyw_s = [sb(f"fyw_s{i}", (3, 128), bf16) for i in range(2)]
    fyr_s = [sb(f"fyr_s{i}", (3, 128), bf16) for i in range(2)]

    pz = [psum(f"pz{i}", (128, H), f32) for i in range(3)]
    pa = [psum(f"pa{i}", (128, H), f32) for i in range(4)]

    class Sem:
        def __init__(self, name):
            self.h = es.enter_context(nc.semaphore(name))
            self.n = 0
        def inc(self, k):
            self.n += k
            return (self.h, self.n)
        def now(self):
            return (self.h, self.n)

    class Ring:
        def __init__(self, name, n):
            self.sems = [Sem(f"{name}{i}") for i in range(n)]
            self.nslots = n
        def write(self, slot, k=16):
            s = self.sems[slot % self.nslots]
            return s.inc(k)
        def last(self, slot):
            s = self.sems[slot % self.nslots]
            return (s.h, s.n)
        def all(self):
            return [(s.h, s.n) for s in self.sems]

    def wait_all(engine, ring):
        for sv in ring.all():
            wait(engine, sv)

    s_in = Sem("s_in")
    s_gprep = Sem("s_gprep")
    s_iprep = Sem("s_iprep")
    s_cc = Sem("s_cc")
    s_kpe = Sem("s_kpe"); s_kact = Sem("s_kact"); s_kmax = Sem("s_kmax")
    s_wn = Sem("s_wn"); s_wrap = Sem("s_wrap")
    s_dg = Sem("s_dg")
    s_ipe = Sem("s_ipe"); s_fy = Sem("s_fy")
    s_zpe = Sem("s_zpe")
    s_zact = Sem("s_zact"); s_ape = Sem("s_ape"); s_aact = Sem("s_aact")

    Q = {e: [] for e in ("sync", "tensor", "vector", "scalar", "gpsimd")}
    checkpoints = []
    def checkpoint():
        checkpoints.append({e: len(Q[e]) for e in Q})
    def emit(engine, fn):
        Q[engine].append(fn)
    def wait(engine, semv):
        s, v = semv
        if v > 0:
            emit(engine, lambda e, s=s, v=v: e.wait_ge(s, v))

    r_gk = Ring("r_gk", 2)     # interp table gathers (per gk buf)
    r_xt = Ring("r_xt", 2)     # xT3 tile loads
    r_h0 = Ring("r_h0", 2)     # h0T tile loads
    r_fyw = Ring("r_fyw", 2)   # finey dram writes
    r_fyr = Ring("r_fyr", 2)   # finey tile loads
    r_hT = Ring("r_hT", 2)     # transpose loads (per hT buf)
    r_g = Ring("r_g", GB)      # agg own gathers (per gath buf)
    r_gp = Ring("r_gp", GB)    # agg peer gathers (per gath buf)
    r_sg = Ring("r_sg", GB)    # combined S+idx loads
    r_gw = Ring("r_gw", 4)     # g_half dram writes (per gsb buf)
    r_hw = Ring("r_hw", 4)     # h dram writes (per hsb buf)
    r_ow = Ring("r_ow", 2)     # out writes (per osb buf)

    # ============ input loads ============
    loads = [
        (cxT3_s[:], "cxT3"), (negf2_s[:], "negf2"),
        (W0_s[:], "W0"), (W1_s[:], "W1"), (W2_s[:], "W2"),
        (W3a_s[:], "W3a"), (W3b_s[:], "W3b"), (W4_s[:], "W4"), (W5_s[:], "W5"),
        (brows_s[:], "brows"), (ones1_s[:], "ones1"), (identf_s[:], "identf"),
    ]
    for dst, srcn in loads:
        sm = s_in.inc(16)
        emit("sync", lambda e, d=dst, s=srcn, sm=sm: e.dma_start(
            out=d, in_=Din[s][:]).then_inc(sm[0], 16))
    IN_ALL = s_in.now()
    checkpoint()   # 0: loads

    # ============ KNN selection ============
    wait("tensor", IN_ALL)
    wait("scalar", IN_ALL)
    wait("vector", IN_ALL)
    NQ = NCPAD // 512
    for t in range(NTH):
        if t >= 2:
            wait("sync", (s_kpe.h, NQ * (t - 1)))
        sm = r_xt.write(t)
        emit("sync", lambda e, t=t, sm=sm: e.dma_start(
            out=xt_s[t % 2][:], in_=Din["xT3"][:, t * 128:(t + 1) * 128]
        ).then_inc(sm[0], 16))
        wait("tensor", r_xt.last(t))
        for q in range(NQ):
            gq = NQ * t + q
            if gq >= 3:
                wait("tensor", (s_kact.h, gq - 2))
            sm = s_kpe.inc(1)
            emit("tensor", lambda e, t=t, q=q, gq=gq, sm=sm: e.matmul(
                pz[gq % 3][:, 0:512], xt_s[t % 2][:],
                cxT3_s[:, q * 512:(q + 1) * 512],
                start=True, stop=True).then_inc(sm[0], 1))
        for q in range(NQ):
            gq = NQ * t + q
            wait("scalar", (s_kpe.h, gq + 1))
            if t >= 2 and q == 0:
                wait("scalar", (s_kmax.h, t - 1))
            sm = s_kact.inc(1)
            emit("scalar", lambda e, t=t, q=q, gq=gq, sm=sm: e.activation(
                nd2_s[t % 2][:, q * 512:(q + 1) * 512], pz[gq % 3][:, 0:512],
                ACTF.Identity, bias=negf2_s[:, t:t + 1], scale=1.0
            ).then_inc(sm[0], 1))
        wait("vector", (s_kact.h, NQ * (t + 1)))
        emit("vector", lambda e, t=t: e.max(bm_s[:, :, t], nd2_s[t % 2][:]))
        emit("vector", lambda e: e.drain())
        emit("vector", lambda e, t=t: e.max_index(
            bi_s[:, :, t], bm_s[:, :, t], nd2_s[t % 2][:]))
        sm = s_kmax.inc(1)
        emit("vector", lambda e, sm=sm: e.drain().then_inc(sm[0], 1))

    checkpoint()   # 1: knn select
    # weights on DVE
    emit("vector", lambda e: e.tensor_scalar(
        out=d2c_s[:], in0=bm_s[:, 0:3, :], scalar1=-1.0, scalar2=1e-16,
        op0=ALU.mult, op1=ALU.max))
    emit("vector", lambda e: e.drain())
    emit("vector", lambda e: e.reciprocal(w_s[:], d2c_s[:]))
    emit("vector", lambda e: e.drain())
    emit("vector", lambda e: e.tensor_reduce(
        out=wsum_s[:], in_=bass.AP(w_s, 0, [[3 * NTH, 128], [1, NTH], [NTH, 3]]),
        axis=mybir.AxisListType.X, op=ALU.add))
    emit("vector", lambda e: e.drain())
    emit("vector", lambda e: e.reciprocal(rs_s[:], wsum_s[:]))
    emit("vector", lambda e: e.drain())
    emit("vector", lambda e: e.tensor_tensor(
        out=wnb_s[:], in0=w_s[:],
        in1=bass.AP(rs_s, 0, [[NTH, 128], [0, 3], [1, NTH]]),
        op=ALU.mult))
    sm = s_wn.inc(1)
    emit("vector", lambda e, sm=sm: e.drain().then_inc(sm[0], 1))

    # wrapped idx build (gpsimd)
    wait("gpsimd", (s_kmax.h, NTH))
    for k in range(3):
        for g in range(8):
            sm = s_wrap.inc(16)
            emit("gpsimd", lambda e, k=k, g=g, sm=sm: e.dma_start(
                out=wrap_s[0:16, k, :, g],
                in_=bi_s[16 * g:16 * (g + 1), k, :],
            ).then_inc(sm[0], 16))
    wait("gpsimd", s_wrap.now())
    for rep in range(1, 8):
        sm = s_wrap.inc(16)
        emit("gpsimd", lambda e, rep=rep, sm=sm: e.dma_start(
            out=wrap_s[16 * rep:16 * (rep + 1)],
            in_=wrap_s[0:16],
        ).then_inc(sm[0], 16))
    WRAP_ALL = s_wrap.now()

    # interp
    wait("gpsimd", WRAP_ALL)
    wait("vector", s_wn.now())
    n_ich = (NTH + ICH - 1) // ICH
    for c in range(n_ich):
        t0 = c * ICH
        ntile = min(ICH, NTH - t0)
        for k in range(3):
            sm = r_gk.write(c)
            smp = s_iprep.inc(1)
            emit("gpsimd", lambda e, k=k, c=c, t0=t0, nt=ntile, sm=sm, smp=smp:
                 e.dma_gather(
                     out_ap=gk_s[k][c % 2][:, 0:nt, :],
                     in_ap=Din["ctab"][:],
                     idxs_ap=wrap_s[:, k, t0:t0 + nt, :].bitcast(i16),
                     num_idxs=nt * 128, num_idxs_reg=nt * 128,
                     elem_size=128,
                     prepare_only=True, sem=r_gk.sems[c % 2].h,
                     queue_num=c % 2,
                 ).then_inc(smp[0], 1))
        wait("gpsimd", s_iprep.now())
        if c >= 2:
            wait("gpsimd", (s_ipe.h, (c - 1) * ICH))
        emit("gpsimd", lambda e, c=c: e.trigger_dma(
            count=3, queue_num=c % 2))
        GK_NOW = r_gk.last(c)
        for tt in range(ntile):
            t = t0 + tt
            if t >= 2:
                wait("vector", (s_ipe.h, t - 1))
            emit("vector", lambda e, t=t: e.tensor_tensor(
                out=diag3_s[t % 2][:],
                in0=bass.AP(identf_s, 0, [[128, 128], [0, 3], [1, 128]]),
                in1=bass.AP(wnb_s, t, [[3 * NTH, 128], [NTH, 3], [0, 128]]),
                op=ALU.mult))
            sm = s_dg.inc(1)
            emit("vector", lambda e, sm=sm: e.drain().then_inc(sm[0], 1))
            wait("tensor", GK_NOW)
            wait("tensor", (s_dg.h, s_dg.n))
            if t >= 4:
                wait("tensor", (s_fy.h, t - 3))    # psum WAR
            for k in range(3):
                sm = s_ipe.inc(1) if k == 2 else None
                def mk_interp(t=t, tt=tt, k=k, c=c, sm=sm):
                    def f(e):
                        ins = e.matmul(
                            pa[t % 4][:, 0:128], gk_s[k][c % 2][:, tt, :],
                            diag3_s[t % 2][:, k, :],
                            start=(k == 0), stop=(k == 2))
                        if sm:
                            ins.then_inc(sm[0], 1)
                    return f
                emit("tensor", mk_interp())
            wait("scalar", (s_ipe.h, s_ipe.n))
            wait("scalar", r_fyw.last(t))
            sm = s_fy.inc(1)
            emit("scalar", lambda e, t=t, sm=sm: e.activation(
                fyw_s[t % 2][:], pa[t % 4][0:3, 0:128],
                ACTF.Copy, bias=0.0, scale=1.0).then_inc(sm[0], 1))
            wait("sync", (s_fy.h, s_fy.n))
            sm = r_fyw.write(t)
            emit("sync", lambda e, t=t, sm=sm: e.dma_start(
                out=fy_d[:, t * 128:(t + 1) * 128],
                in_=fyw_s[t % 2][:]).then_inc(sm[0], 16))
    FY_ALL = s_fy.now()
    KACT_ALL = s_kact.now()
    checkpoint()   # 2: interp

    # ============ GCN layers ============
    WCH = {1: W1_s, 2: W2_s, 3: W3a_s, 4: W4_s, 5: W5_s}
    layer_state = []   # per layer: rg/rgp/cc snapshots + h-write marks

    for li, L in enumerate(LAYERS):
        width = 128 if L["e2"] else H
        ghalf = g2_half if L["e2"] else g_half[li % 2]
        gfull = g2_full if L["e2"] else g_full[li % 2]
        war = layer_state[li - 2] if (li >= 2 and not L["e2"]) else None
        prev = layer_state[li - 1] if li >= 1 else None

        # ---------- dense (own half) ----------
        zpe_base = s_zpe.n
        zact_base = s_zact.n
        gw_marks = []

        def dense_epilogue(t, width=width, ghalf=ghalf, war=war):
            wait("vector", (s_zpe.h, zpe_base + t + 1))
            wait("vector", r_gw.last(t))
            sm = s_zact.inc(1)
            emit("vector", lambda e, t=t, w=width: e.tensor_scalar(
                out=gsb_s[t % 4][:, 0:w], in0=pz[t % 3][:, 0:w],
                scalar1=1.0, scalar2=None, op0=ALU.mult))
            emit("vector", lambda e, sm=sm: e.drain().then_inc(sm[0], 1))
            wait("sync", (s_zact.h, s_zact.n))
            if t == 0 and war is not None:
                wait("sync", war["cc"])           # WAR vs AG reads (li-2)
                for sv in war["rg"]:              # WAR vs own gathers (li-2)
                    wait("sync", sv)
            sm = r_gw.write(t)
            emit("sync", lambda e, t=t, gd=ghalf, w=width, sm=sm: e.dma_start(
                out=gd[t * 128:(t + 1) * 128, :],
                in_=gsb_s[t % 4][:, 0:w]).then_inc(sm[0], 16))
            gw_marks.append(r_gw.all())

        if li == 0:
            wait("tensor", KACT_ALL)      # pz WAR vs KNN ACT
            for t in range(NTH):
                if t >= 2:
                    wait("sync", (s_zpe.h, zpe_base + t - 1))
                sm = r_h0.write(t)
                emit("sync", lambda e, t=t, sm=sm: e.dma_start(
                    out=h0_s[t % 2][:], in_=Din["h0T"][:, t * 128:(t + 1) * 128]
                ).then_inc(sm[0], 16))
                wait("tensor", r_h0.last(t))
                wait("tensor", (s_zact.h,
                                zact_base if t < 3 else zact_base + t - 2))
                sm = s_zpe.inc(1)
                emit("tensor", lambda e, t=t, sm=sm: e.matmul(
                    pz[t % 3][:, 0:H], h0_s[t % 2][:],
                    W0_s[:], start=True, stop=True).then_inc(sm[0], 1))
                dense_epilogue(t)
        else:
            Wl = WCH[li]
            range_zpe = []
            for ri, (rt0, rnt) in enumerate(_ranges()):
                wait("sync", (s_zpe.h,
                              zpe_base if ri < 2 else range_zpe[ri - 2]))
                for sv in prev["hmarks"][rt0 + rnt - 1]:   # h tiles ready
                    wait("sync", sv)
                for cch in range(4):
                    sm = r_hT.write(ri)
                    emit("sync", lambda e, ri=ri, rt0=rt0, rnt=rnt, c=cch, sm=sm:
                         e.dma_start_transpose(
                             hT_s[ri % 2][:, c, 0:rnt * 128],
                             h_d[rt0 * 128:(rt0 + rnt) * 128,
                                 c * 128:(c + 1) * 128],
                         ).then_inc(sm[0], 16))
                wait("tensor", r_hT.last(ri))
                if li == 3 and ri == 0:
                    wait_all("sync", r_fyw)
                for tt in range(rnt):
                    t = rt0 + tt
                    range_last = (tt == rnt - 1)
                    if L["fy"]:
                        if t >= 2:
                            wait("sync", (s_zpe.h, zpe_base + t - 1))
                        sm = r_fyr.write(t)
                        emit("sync", lambda e, t=t, sm=sm: e.dma_start(
                            out=fyr_s[t % 2][:],
                            in_=fy_d[:, t * 128:(t + 1) * 128]
                        ).then_inc(sm[0], 16))
                    wait("tensor", (s_zact.h,
                                    zact_base if t < 3 else zact_base + t - 2))
                    for cch in range(4):
                        last = (cch == 3) and not L["fy"]
                        sm = s_zpe.inc(1) if last else None
                        def mk_dense(t=t, tt=tt, ri=ri, cch=cch, Wl=Wl,
                                     w=width, last=last, sm=sm):
                            def f(e):
                                ins = e.matmul(
                                    pz[t % 3][:, 0:w],
                                    hT_s[ri % 2][:, cch,
                                                 tt * 128:(tt + 1) * 128],
                                    Wl[:, cch, 0:w],
                                    start=(cch == 0), stop=last)
                                if sm:
                                    ins.then_inc(sm[0], 1)
                            return f
                        emit("tensor", mk_dense())
                    if L["fy"]:
                        wait("tensor", r_fyr.last(t))
                        sm = s_zpe.inc(1)
                        emit("tensor", lambda e, t=t, sm=sm: e.matmul(
                            pz[t % 3][:, 0:H],
                            fyr_s[t % 2][:],
                            W3b_s[:], start=False, stop=True).then_inc(sm[0], 1))
                    if range_last:
                        range_zpe.append(s_zpe.n)
                    dense_epilogue(t)
        checkpoint()   # dense of this layer done

        # ---------- AllGather g_half -> g_full (2 chunks) ----------
        for ci, (c0, c1) in enumerate([(0, CHK), (CHK, NTH)]):
            for sv in gw_marks[c1 - 1]:
                wait("gpsimd", sv)
            if ci == 0 and war is not None:
                for sv in war["rgp"]:     # WAR vs peer gathers (li-2)
                    wait("gpsimd", sv)
            rows = (c1 - c0) * 128
            sm = s_cc.inc(1)
            emit("gpsimd", lambda e, hh=ghalf, ff=gfull, c0=c0, rows=rows,
                 w=width, sm=sm: e.collective_compute(
                     "AllGather",
                     ALU.bypass,
                     replica_groups=RG_PAIRS,
                     ins=[bass.AP(hh, c0 * 128 * w,
                                  [[w, rows], [1, w]]).opt()],
                     outs=[bass.AP(ff, 2 * c0 * 128 * w,
                                   [[w, 2 * rows], [1, w]]).opt()],
                 ).then_inc(sm[0], 1))
        CC_NOW = s_cc.now()

        # ---------- agg (own dest tiles) ----------
        gbufs = gath2_s if L["e2"] else gath_s
        ape_base = s_ape.n
        aact_base = s_aact.n
        gprep_base = s_gprep.n
        hmarks = []
        if li == 0:
            wait("tensor", (s_fy.h, NTH))   # pa WAR vs interp
        for t in range(NTH):
            po, pp, nsl = int(Po[t]), int(Pp[t]), int(P[t])
            # --- combined S+idx load (sync) ---
            wait("sync", (s_gprep.h,
                          gprep_base if t < GB
                          else gprep_base + 2 * (t - GB) + 2))
            wait("sync", (s_ape.h,
                          ape_base if t < GB else ape_base + t - GB + 1))
            sm = r_sg.write(t)
            emit("sync", lambda e, t=t, nsl=nsl, sm=sm: e.dma_start(
                out=sgt_s[t % GB][:, 0:nsl * 136],
                in_=Din["sgt"][:, SOFFC[t]:SOFFC[t] + nsl * 136]
            ).then_inc(sm[0], 16))
            # --- gather preps (gpsimd): own half + peer half ---
            wait("gpsimd", r_sg.last(t))
            smo = r_g.write(t)
            smp = s_gprep.inc(1)
            emit("gpsimd", lambda e, t=t, po=po, nsl=nsl, gh=ghalf, gb=gbufs,
                 w=width, smp=smp: e.dma_gather(
                     out_ap=gb[t % GB][:, 0:po, 0:w],
                     in_ap=gh[:],
                     idxs_ap=sgt_s[t % GB][:, nsl * 128:
                                           nsl * 128 + po * 8].bitcast(i16),
                     num_idxs=po * 128, num_idxs_reg=po * 128, elem_size=w,
                     single_packet=False,
                     prepare_only=True, sem=r_g.sems[t % GB].h,
                     queue_num=t % 2,
                 ).then_inc(smp[0], 1))
            smq = r_gp.write(t)
            smp = s_gprep.inc(1)
            emit("gpsimd", lambda e, t=t, po=po, pp=pp, nsl=nsl, gf=gfull,
                 gb=gbufs, w=width, smp=smp: e.dma_gather(
                     out_ap=gb[t % GB][:, po:po + pp, 0:w],
                     in_ap=gf[:],
                     idxs_ap=sgt_s[t % GB][:, nsl * 128 + po * 8:
                                           nsl * 136].bitcast(i16),
                     num_idxs=pp * 128, num_idxs_reg=pp * 128, elem_size=w,
                     single_packet=False,
                     prepare_only=True, sem=r_gp.sems[t % GB].h,
                     queue_num=2 + t % 2,
                 ).then_inc(smp[0], 1))
            wait("gpsimd", s_gprep.now())
            # own trigger: needs gath buf free + ALL own dense writes
            wait("gpsimd", (s_ape.h,
                            ape_base if t < GB else ape_base + t - GB + 1))
            if t == 0:
                for sv in gw_marks[NTH - 1]:
                    wait("gpsimd", sv)
            emit("gpsimd", lambda e, t=t: e.trigger_dma(
                count=1, queue_num=t % 2))
            # peer trigger: additionally needs the AllGather
            if t == 0:
                wait("gpsimd", CC_NOW)
            emit("gpsimd", lambda e, t=t: e.trigger_dma(
                count=1, queue_num=2 + t % 2))
            # --- matmuls (tensor): own slices, then peer, then bias ---
            wait("tensor", r_g.last(t))
            wait("tensor", (s_aact.h,
                            aact_base if t < 4 else aact_base + t - 3))
            for sl in range(po):
                emit("tensor", lambda e, t=t, sl=sl, gb=gbufs, w=width:
                     e.matmul(
                         pa[t % 4][:, 0:w],
                         sgt_s[t % GB][:, sl * 128:(sl + 1) * 128],
                         gb[t % GB][:, sl, 0:w],
                         start=(sl == 0), stop=False))
            wait("tensor", r_gp.last(t))
            for sl in range(po, nsl):
                emit("tensor", lambda e, t=t, sl=sl, gb=gbufs, w=width:
                     e.matmul(
                         pa[t % 4][:, 0:w],
                         sgt_s[t % GB][:, sl * 128:(sl + 1) * 128],
                         gb[t % GB][:, sl, 0:w],
                         start=False, stop=False))
            sm = s_ape.inc(1)
            emit("tensor", lambda e, t=t, li=li, w=width, sm=sm: e.matmul(
                pa[t % 4][:, 0:w], ones1_s[:],
                brows_s[:, li, 0:w], start=False, stop=True
            ).then_inc(sm[0], 1))
            # --- epilogue (scalar + sync) ---
            wait("scalar", (s_ape.h, s_ape.n))
            if L["e2"]:
                wait("scalar", r_ow.last(t))
            else:
                wait("scalar", r_hw.last(t))
            sm = s_aact.inc(1)
            if L["e2"]:
                emit("scalar", lambda e, t=t, sm=sm: e.activation(
                    osb_s[t % 2][:], pa[t % 4][:, 0:128], ACTF.Copy,
                    bias=0.0, scale=1.0).then_inc(sm[0], 1))
            else:
                emit("scalar", lambda e, t=t, sm=sm: e.activation(
                    hsb_s[t % 4][:], pa[t % 4][:, 0:H], ACTF.Relu,
                    bias=0.0, scale=1.0).then_inc(sm[0], 1))
            wait("sync", (s_aact.h, s_aact.n))
            if L["e2"]:
                sm = r_ow.write(t)
                emit("sync", lambda e, t=t, sm=sm: e.dma_start(
                    out=Din["out_nm"][t * 128:(t + 1) * 128, :],
                    in_=osb_s[t % 2][:]).then_inc(sm[0], 16))
            else:
                sm = r_hw.write(t)
                emit("sync", lambda e, t=t, sm=sm: e.dma_start(
                    out=h_d[t * 128:(t + 1) * 128, :],
                    in_=hsb_s[t % 4][:]).then_inc(sm[0], 16))
            hmarks.append(r_hw.all())
        layer_state.append(dict(rg=r_g.all(), rgp=r_gp.all(),
                                cc=s_cc.now(), hmarks=hmarks))
        checkpoint()   # 3+li

    wait_all("sync", r_ow)
    wait_all("sync", r_hw)
    checkpoint()
    if PHASE < len(checkpoints):
        cut = checkpoints[PHASE]
        for e in Q:
            Q[e] = Q[e][:cut[e]]

    with nc.allow_non_contiguous_dma(reason="wrapped idx build"), \
            nc.Block() as block:
        @block.sync
        def _(e):
            for fn in Q["sync"]:
                fn(e)

        @block.tensor
        def _(e):
            for fn in Q["tensor"]:
                fn(e)

        @block.vector
        def _(e):
            for fn in Q["vector"]:
                fn(e)

        @block.scalar
        def _(e):
            for fn in Q["scalar"]:
                fn(e)

        @block.gpsimd
        def _(e):
            for fn in Q["gpsimd"]:
                fn(e)

    nc.finalize()
    return nc


# ================= host side =================

def host_prep(inputs):
    x = np.asarray(inputs["x"], np.float32)
    sdf = np.asarray(inputs["sdf"], np.float32)
    edge_index = np.asarray(inputs["edge_index"], np.int64)
    coarse_x = np.asarray(inputs["coarse_x"], np.float32)
    coarse_y = np.asarray(inputs["coarse_y"], np.float32)
    Ws = {k: np.asarray(inputs[k], np.float32) for k in (
        "pre_W0", "pre_W1", "pre_W2", "end_W0", "end_W1", "end_W2")}
    bs = {k: np.asarray(inputs[k], np.float32) for k in (
        "pre_b0", "pre_b1", "pre_b2", "end_b0", "end_b1", "end_b2")}

    cxT3 = np.zeros((3, NCPAD), np.float32)
    cxT3[0, :NC] = 2 * coarse_x[:, 0]
    cxT3[1, :NC] = 2 * coarse_x[:, 1]
    cxT3[2, :NC] = -(coarse_x[:, 0] ** 2 + coarse_x[:, 1] ** 2)
    cxT3[0, NC:] = 2e4; cxT3[1, NC:] = 2e4; cxT3[2, NC:] = -2e8

    brows = np.zeros((6, H), np.float32)
    for i, k in enumerate(("pre_b0", "pre_b1", "pre_b2", "end_b0", "end_b1")):
        brows[i] = bs[k]
    brows[5, :OUT] = bs["end_b2"]

    W5 = np.zeros((H, 128), np.float32)
    W5[:, :OUT] = Ws["end_W2"]

    def pmaj(w):   # [512, X] -> [128, 4, X]
        return np.ascontiguousarray(
            w.reshape(4, 128, w.shape[1]).transpose(1, 0, 2))

    common = dict(
        cxT3=cxT3,
        W0=Ws["pre_W0"].astype(bfnp),
        W1=pmaj(Ws["pre_W1"]).astype(bfnp),
        W2=pmaj(Ws["pre_W2"]).astype(bfnp),
        W3a=pmaj(Ws["end_W0"][OUT:]).astype(bfnp),
        W3b=Ws["end_W0"][:OUT].astype(bfnp),
        W4=pmaj(Ws["end_W1"]).astype(bfnp),
        W5=pmaj(W5).astype(bfnp),
        brows=brows.astype(bfnp)[None],
        ones1=np.ones((1, 128), bfnp),
        identf=np.eye(128, dtype=np.float32),
    )

    # ---- pass 1: per-sample node->slot assignment + per-core tile stats ----
    samples = []
    for s in range(B):
        xs = x[s * NF:(s + 1) * NF]
        e = edge_index[:, s * E_PER:(s + 1) * E_PER] - s * NF
        cy = coarse_y[s * NC:(s + 1) * NC]

        deg = np.bincount(e[1], minlength=NF).astype(np.float32) + 1.0
        dinv = (1.0 / np.sqrt(deg)).astype(np.float32)

        # balanced global tile assignment (snake over degree-sorted nodes)
        order = np.argsort(-deg, kind="stable")
        tile_seq = np.arange(NT)
        snake = np.concatenate([tile_seq, tile_seq[::-1]])
        bins = np.resize(snake, NF)
        gtile = np.empty(NF, np.int64)   # node -> global tile
        lane = np.empty(NF, np.int64)
        for t in range(NT):
            sel = np.where(bins == t)[0]
            gtile[order[sel]] = t
            lane[order[sel]] = np.arange(len(sel))

        # self loops ride in the own-source group
        e_aug = np.concatenate([e, np.stack([np.arange(NF)] * 2)], axis=1)

        # in-edge count per global tile (incl self loops)
        cin = np.bincount(gtile[e_aug[1]], minlength=NT)

        # per half: order local tiles by in-edge count desc
        half = (gtile >= NTH).astype(np.int64)
        ltile = np.empty(NF, np.int64)
        sco = np.zeros((2, NTH), np.int64)   # own-source slices per tile
        scp = np.zeros((2, NTH), np.int64)   # peer-source slices per tile
        for p in range(2):
            gts = np.arange(p * NTH, (p + 1) * NTH)
            perm = gts[np.argsort(-cin[gts], kind="stable")]
            inv = np.empty(NTH, np.int64)
            inv[perm - p * NTH] = np.arange(NTH)
            mask = half == p
            ltile[mask] = inv[gtile[mask] - p * NTH]
        src_half = half[e_aug[0]]
        dst_half = half[e_aug[1]]
        for p in range(2):
            emask = dst_half == p
            dt_ = ltile[e_aug[1][emask]]
            own = src_half[emask] == p
            sco[p] = np.bincount(dt_[own], minlength=NTH)   # own counts
            scp[p] = np.bincount(dt_, minlength=NTH)        # total counts

        # node -> row in g_full (chunk-major AllGather layout: the 2-chunk
        # AG writes [even c0..CHK | odd c0..CHK | even CHK.. | odd CHK..])
        nidg = np.where(
            ltile < CHK,
            half * (CHK * 128) + ltile * 128 + lane,
            2 * CHK * 128 + half * ((NTH - CHK) * 128)
            + (ltile - CHK) * 128 + lane)
        # node -> row in concat(even out_nm, odd out_nm) (output assembly)
        nidl = half * NPADH + ltile * 128 + lane
        samples.append(dict(xs=xs, e=e_aug, cy=cy, dinv=dinv, half=half,
                            ltile=ltile, lane=lane, nidg=nidg, nidl=nidl,
                            sco=sco, scp=scp))

    # profiles: own group sized to the min own count over cores (zero own
    # padding; overflow spills into the peer group which reads g_full)
    own_min = np.full(NTH, 1 << 30, np.int64)
    tot_max = np.zeros(NTH, np.int64)
    for sm in samples:
        own_min = np.minimum(own_min, sm["sco"].min(axis=0))
        tot_max = np.maximum(tot_max, sm["scp"].max(axis=0))
    Po = np.maximum(own_min // 128, 1)
    Pp = np.maximum(np.ceil((tot_max - Po * 128) / 128).astype(np.int64), 1)
    P = Po + Pp
    assert P.max() <= 12, f"slice overflow {P.max()}"
    SOFF = np.concatenate([[0], np.cumsum(P)]).astype(int)
    SLOT_TOT = int(SOFF[-1]) * 128
    SOFFC = (SOFF * 136).astype(int)

    # ---- pass 2: per-core arrays ----
    in_maps, metas = [], []
    for s in range(B):
        smp_ = samples[s]
        xs, e, cy = smp_["xs"], smp_["e"], smp_["cy"]
        dinv, half, ltile, lane, nidg = (
            smp_["dinv"], smp_["half"], smp_["ltile"], smp_["lane"],
            smp_["nidg"])

        ctab = np.zeros((NCPAD, 128), np.float32)
        ctab[:NC, 0:OUT] = cy
        ctab = ctab.astype(bfnp)

        for p in range(2):
            own = half == p
            lrow = ltile * 128 + lane          # local row id (own nodes)

            grow = np.zeros(SLOT_TOT, np.int16)
            sT = np.zeros((SLOT_TOT, 128), np.float32)
            # two slot groups per tile: first Po[t]*128 own-source edges
            # (local g_half rows, incl self loops) at SOFF[t]*128; all
            # remaining edges (own overflow + peer sources, g_full rows)
            # at (SOFF[t]+Po[t])*128
            emask = half[e[1]] == p
            e_src, e_dst = e[0][emask], e[1][emask]
            is_own = half[e_src] == p
            ecol_t = ltile[e_dst]
            # order: per tile, own-source edges first
            o = np.lexsort((~is_own, ecol_t))
            e_src, e_dst, is_own = e_src[o], e_dst[o], is_own[o]
            ecol_t = ecol_t[o]
            ecol_l = lrow[e_dst] % 128
            ewt = dinv[e_src] * dinv[e_dst]
            tstart = np.searchsorted(ecol_t, np.arange(NTH))
            cnts = (np.searchsorted(ecol_t, np.arange(NTH), side="right")
                    - tstart)
            assert (cnts <= P * 128).all(), "profile overflow"
            rank = np.arange(len(ecol_t)) - np.repeat(tstart, cnts)
            in_own = rank < np.repeat(Po * 128, cnts)
            # own group must contain only own-source edges
            assert not (in_own & ~is_own).any(), "own group underfilled"
            slot = (SOFF[ecol_t] * 128 + rank).astype(np.int64)
            erow = np.where(in_own, lrow[e_src], nidg[e_src])
            grow[slot] = erow.astype(np.int16)
            sT[slot, ecol_l] = ewt

            # combined per-tile [S p-major | idx] tensor
            tmp = np.ascontiguousarray(grow.reshape(SLOT_TOT // 16, 16).T)
            growc = np.ascontiguousarray(np.tile(tmp, (8, 1)))  # [128, S/16]
            sgt = np.zeros((128, int(SOFFC[-1])), bfnp)
            for t in range(NTH):
                nsl = int(P[t]); base = int(SOFFC[t])
                blk = sT[SOFF[t] * 128:(SOFF[t] + nsl) * 128]
                pm = blk.reshape(nsl, 128, 128).transpose(1, 0, 2)
                sgt[:, base:base + nsl * 128] = (
                    pm.reshape(128, nsl * 128).astype(bfnp))
                gb = np.ascontiguousarray(
                    growc[:, SOFF[t] * 8:(SOFF[t] + nsl) * 8])
                sgt[:, base + nsl * 128:base + nsl * 136] = gb.view(bfnp)

            # node features / positions at local slots
            f01 = np.full((NPADH, 2), 1e3, np.float32)
            f01[lrow[own]] = xs[own][:, 0:2]
            xT3 = np.ones((3, NPADH), np.float32)
            xT3[0] = f01[:, 0]; xT3[1] = f01[:, 1]
            negf2 = np.ascontiguousarray(
                (-(f01[:, 0] ** 2 + f01[:, 1] ** 2)).reshape(NTH, 128).T)

            h0 = np.zeros((NPADH, 6), np.float32)
            h0[lrow[own], 0:D_IN] = xs[own]
            h0[lrow[own], D_IN] = sdf[own, 0]
            h0T = np.ascontiguousarray(h0.T).astype(bfnp)

            m = dict(common)
            m.update(xT3=xT3, negf2=negf2, h0T=h0T, sgt=sgt, ctab=ctab)
            in_maps.append(m)
        metas.append(smp_["nidl"])

    return in_maps, metas, (tuple(Po.tolist()), tuple(Pp.tolist()))


_prog_cache = {}


def kernel(**inputs):
    in_maps, metas, P = host_prep(inputs)
    if _prog_cache.get("P") != P:
        _prog_cache["nc"] = build_program(np.array(P[0]), np.array(P[1]))
        _prog_cache["P"] = P
    nc = _prog_cache["nc"]

    res = run_bass_kernel_spmd(nc, in_maps, list(range(N_CORES)))
    global _last_exec_ns, _last_trace
    _last_exec_ns = res.exec_time_ns
    _last_trace = res.instructions_and_trace

    out = np.empty((B * NF, OUT), np.float32)
    for s in range(B):
        full = np.concatenate([
            np.asarray(res.results[2 * s]["out_nm"]),
            np.asarray(res.results[2 * s + 1]["out_nm"]),
        ], axis=0)
        out[s * NF:(s + 1) * NF] = full[metas[s], 0:OUT]
    return out


# revision 35
# speedup vs baseline: 4.3645x; 1.9769x over previous
"""CFD-GCN Trainium2 kernel: 6-layer GCN on a batched random mesh graph +
KNN interpolation, distributed over 8 NeuronCores.

Each sample (4 total) is split across a PAIR of cores: core 2s owns node
tiles 0..78, core 2s+1 owns 79..157 (79 tiles of 128 nodes). Dense (h@W),
KNN selection, interpolation and aggregation all run on the owned half.
Per layer, a 2-chunk pair AllGather publishes the dense output g; each
tile's edge gather is split into an own-half gather (reads local g_half,
no collective wait) and a peer-half gather (reads g_full, waits the
AllGather). Self-loops ride in the own-gather slots. Descriptors are
generated with prepare_only + trigger_dma on 4 SWDGE queues. g_half and
g_full ping-pong across layers so dense l overlaps aggregation l-1
(per-tile h-write marks instead of a layer barrier); the dense
psum->SBUF copy runs on the vector engine to stay clear of the scalar
queue.

Self-contained: hardcodes all shapes; the slice profiles (own/peer slots
per dest tile) are derived from the inputs on first call and baked into
the program. kernel(**inputs) -> np.ndarray [80000, 3].
"""
import sys

sys.path.insert(0, "/opt/trn_rl_repo")

import numpy as np
import ml_dtypes

from concourse import bass, bacc
from concourse.bass_utils import run_bass_kernel_spmd
import concourse.mybir as mybir
from contextlib import ExitStack

f32, bf16 = mybir.dt.float32, mybir.dt.bfloat16
i16, u16 = mybir.dt.int16, mybir.dt.uint16
ALU = mybir.AluOpType
ACTF = mybir.ActivationFunctionType
bfnp = ml_dtypes.bfloat16

# ---------------- problem constants ----------------
B, NF, NC, H, D_IN, OUT = 4, 20000, 2000, 512, 5, 3
E_PER = 6 * NF
NT = 158                      # global node tiles per sample
NTH = 79                      # node tiles per core (half sample)
NPAD = NT * 128               # 20224
NPADH = NTH * 128             # 10112
NCPAD = 2048                  # padded coarse count
RANGE_T = 16                  # node tiles per hT transpose-load range
ICH = 8                       # interp gather chunk (tiles)
N_CORES = 8
GB = 4                        # agg ring depth (gather bufs)
CHK = 40                      # AllGather chunk boundary (tiles)
PHASE = 99                    # debug: truncate program after checkpoint N

LAYERS = [
    dict(kc6=True, fy=False, relu=True, e2=False),   # pre0
    dict(kc6=False, fy=False, relu=True, e2=False),  # pre1
    dict(kc6=False, fy=False, relu=True, e2=False),  # pre2
    dict(kc6=False, fy=True, relu=True, e2=False),   # end0
    dict(kc6=False, fy=False, relu=True, e2=False),  # end1
    dict(kc6=False, fy=False, relu=False, e2=True),  # end2
]

RG_PAIRS = [[0, 1], [2, 3], [4, 5], [6, 7]]


def _ranges():
    r, t0 = [], 0
    while t0 < NTH:
        r.append((t0, min(RANGE_T, NTH - t0)))
        t0 += RANGE_T
    return r


def build_program(Po, Pp):
    """Po/Pp: per-local-tile own/peer slice counts, identical on all cores."""
    P = Po + Pp
    SOFF = np.concatenate([[0], np.cumsum(P)]).astype(int)      # slice offs
    SOFFC = (SOFF * 136).astype(int)                            # sgt col offs
    MAXP = int(P.max())

    nc = bacc.Bacc(num_devices=N_CORES, num_swdge_queues=4)

    Din = {}
    def din(name, shape, dt):
        Din[name] = nc.declare_dram_parameter(name, list(shape), dt, isOutput=False)
    def dout(name, shape, dt):
        Din[name] = nc.declare_dram_parameter(name, list(shape), dt, isOutput=True)

    din("xT3", (3, NPADH), f32)
    din("cxT3", (3, NCPAD), f32)
    din("negf2", (128, NTH), f32)
    din("h0T", (6, NPADH), bf16)
    din("W0", (6, H), bf16)
    din("W1", (128, 4, H), bf16)      # p-major k-chunked
    din("W2", (128, 4, H), bf16)
    din("W3a", (128, 4, H), bf16)
    din("W3b", (3, H), bf16)
    din("W4", (128, 4, H), bf16)
    din("W5", (128, 4, 128), bf16)
    din("brows", (1, 6, H), bf16)
    din("ones1", (1, 128), bf16)
    din("identf", (128, 128), f32)
    din("sgt", (128, int(SOFFC[-1])), bf16)   # per tile: S p-major | idxs
    din("ctab", (NCPAD, 128), bf16)

    g_half = [nc.dram_tensor(f"g_half{i}", [NPADH, H], bf16) for i in range(2)]
    g_full = [nc.dram_tensor(f"g_full{i}", [NPAD, H], bf16) for i in range(2)]
    g2_half = nc.dram_tensor("g2_half", [NPADH, 128], bf16)
    g2_full = nc.dram_tensor("g2_full", [NPAD, 128], bf16)
    h_d = nc.dram_tensor("h_d", [NPADH, H], bf16)
    fy_d = nc.dram_tensor("fy_d", [3, NPADH], bf16)
    dout("out_nm", (NPADH, 128), f32)

    es = ExitStack()
    def sb(name, shape, dt):
        return es.enter_context(nc.sbuf_tensor(name, list(shape), dt))
    def psum(name, shape, dt):
        return es.enter_context(nc.psum_tensor(name, list(shape), dt))

    xt_s = [sb(f"xt_s{i}", (3, 128), f32) for i in range(2)]
    cxT3_s = sb("cxT3_s", (3, NCPAD), f32)
    negf2_s = sb("negf2_s", (128, NTH), f32)
    h0_s = [sb(f"h0_s{i}", (6, 128), bf16) for i in range(2)]
    W0_s = sb("W0_s", (6, H), bf16)
    W1_s = sb("W1_s", (128, 4, H), bf16)
    W2_s = sb("W2_s", (128, 4, H), bf16)
    W3a_s = sb("W3a_s", (128, 4, H), bf16)
    W3b_s = sb("W3b_s", (3, H), bf16)
    W4_s = sb("W4_s", (128, 4, H), bf16)
    W5_s = sb("W5_s", (128, 4, 128), bf16)
    brows_s = sb("brows_s", (1, 6, H), bf16)
    ones1_s = sb("ones1_s", (1, 128), bf16)
    identf_s = sb("identf_s", (128, 128), f32)

    hT_s = [sb(f"hT_s{i}", (128, 4, RANGE_T * 128), bf16) for i in range(2)]
    gsb_s = [sb(f"gsb_s{i}", (128, H), bf16) for i in range(4)]
    hsb_s = [sb(f"hsb_s{i}", (128, H), bf16) for i in range(4)]
    osb_s = [sb(f"osb_s{i}", (128, 128), f32) for i in range(2)]
    gath_s = [sb(f"gath_s{i}", (128, MAXP, H), bf16) for i in range(GB)]
    gath2_s = [sb(f"gath2_s{i}", (128, MAXP, 128), bf16) for i in range(GB)]
    sgt_s = [sb(f"sgt_s{i}", (128, MAXP * 136), bf16) for i in range(GB)]

    nd2_s = [sb(f"nd2_s{i}", (128, NCPAD), f32) for i in range(2)]
    bm_s = sb("bm_s", (128, 8, NTH), f32)
    bi_s = sb("bi_s", (128, 8, NTH), u16)
    d2c_s = sb("d2c_s", (128, 3, NTH), f32)
    w_s = sb("w_s", (128, 3, NTH), f32)
    wsum_s = sb("wsum_s", (128, NTH), f32)
    rs_s = sb("rs_s", (128, NTH), f32)
    wnb_s = sb("wnb_s", (128, 3, NTH), f32)
    wrap_s = sb("wrap_s", (128, 3, NTH, 8), u16)
    gk_s = [[sb(f"gk_s{k}_{i}", (128, ICH, 128), bf16) for i in range(2)]
            for k in range(3)]
    diag3_s = [sb(f"diag3_s{i}", (128, 3, 128), bf16) for i in range(2)]
    fyw_s = [sb(f"fyw_s{i}", (3, 128), bf16) for i in range(2)]
    fyr_s = [sb(f"fyr_s{i}", (3, 128), bf16) for i in range(2)]

    pz = [psum(f"pz{i}", (128, H), f32) for i in range(3)]
    pa = [psum(f"pa{i}", (128, H), f32) for i in range(4)]

    class Sem:
        def __init__(self, name):
            self.h = es.enter_context(nc.semaphore(name))
            self.n = 0
        def inc(self, k):
            self.n += k
            return (self.h, self.n)
        def now(self):
            return (self.h, self.n)

    class Ring:
        def __init__(self, name, n):
            self.sems = [Sem(f"{name}{i}") for i in range(n)]
            self.nslots = n
        def write(self, slot, k=16):
            s = self.sems[slot % self.nslots]
            return s.inc(k)
        def last(self, slot):
            s = self.sems[slot % self.nslots]
            return (s.h, s.n)
        def all(self):
            return [(s.h, s.n) for s in self.sems]

    def wait_all(engine, ring):
        for sv in ring.all():
            wait(engine, sv)

    s_in = Sem("s_in")
    s_gprep = Sem("s_gprep")
    s_iprep = Sem("s_iprep")
    s_cc = Sem("s_cc")
    s_kpe = Sem("s_kpe"); s_kact = Sem("s_kact"); s_kmax = Sem("s_kmax")
    s_wn = Sem("s_wn"); s_wrap = Sem("s_wrap")
    s_dg = Sem("s_dg")
    s_ipe = Sem("s_ipe"); s_fy = Sem("s_fy")
    s_zpe = Sem("s_zpe")
    s_zact = Sem("s_zact"); s_ape = Sem("s_ape"); s_aact = Sem("s_aact")

    Q = {e: [] for e in ("sync", "tensor", "vector", "scalar", "gpsimd")}
    checkpoints = []
    def checkpoint():
        checkpoints.append({e: len(Q[e]) for e in Q})
    def emit(engine, fn):
        Q[engine].append(fn)
    def wait(engine, semv):
        s, v = semv
        if v > 0:
            emit(engine, lambda e, s=s, v=v: e.wait_ge(s, v))

    r_gk = Ring("r_gk", 2)     # interp table gathers (per gk buf)
    r_xt = Ring("r_xt", 2)     # xT3 tile loads
    r_h0 = Ring("r_h0", 2)     # h0T tile loads
    r_fyw = Ring("r_fyw", 2)   # finey dram writes
    r_fyr = Ring("r_fyr", 2)   # finey tile loads
    r_hT = Ring("r_hT", 2)     # transpose loads (per hT buf)
    r_g = Ring("r_g", GB)      # agg own gathers (per gath buf)
    r_gp = Ring("r_gp", GB)    # agg peer gathers (per gath buf)
    r_sg = Ring("r_sg", GB)    # combined S+idx loads
    r_gw = Ring("r_gw", 4)     # g_half dram writes (per gsb buf)
    r_hw = Ring("r_hw", 4)     # h dram writes (per hsb buf)
    r_ow = Ring("r_ow", 2)     # out writes (per osb buf)

    # ============ input loads ============
    loads = [
        (cxT3_s[:], "cxT3"), (negf2_s[:], "negf2"),
        (W0_s[:], "W0"), (W1_s[:], "W1"), (W2_s[:], "W2"),
        (W3a_s[:], "W3a"), (W3b_s[:], "W3b"), (W4_s[:], "W4"), (W5_s[:], "W5"),
        (brows_s[:], "brows"), (ones1_s[:], "ones1"), (identf_s[:], "identf"),
    ]
    for dst, srcn in loads:
        sm = s_in.inc(16)
        emit("sync", lambda e, d=dst, s=srcn, sm=sm: e.dma_start(
            out=d, in_=Din[s][:]).then_inc(sm[0], 16))
    IN_ALL = s_in.now()
    checkpoint()   # 0: loads

    # ============ KNN selection ============
    wait("tensor", IN_ALL)
    wait("scalar", IN_ALL)
    wait("vector", IN_ALL)
    NQ = NCPAD // 512
    for t in range(NTH):
        if t >= 2:
            wait("sync", (s_kpe.h, NQ * (t - 1)))
        sm = r_xt.write(t)
        emit("sync", lambda e, t=t, sm=sm: e.dma_start(
            out=xt_s[t % 2][:], in_=Din["xT3"][:, t * 128:(t + 1) * 128]
        ).then_inc(sm[0], 16))
        wait("tensor", r_xt.last(t))
        for q in range(NQ):
            gq = NQ * t + q
            if gq >= 3:
                wait("tensor", (s_kact.h, gq - 2))
            sm = s_kpe.inc(1)
            emit("tensor", lambda e, t=t, q=q, gq=gq, sm=sm: e.matmul(
                pz[gq % 3][:, 0:512], xt_s[t % 2][:],
                cxT3_s[:, q * 512:(q + 1) * 512],
                start=True, stop=True).then_inc(sm[0], 1))
        for q in range(NQ):
            gq = NQ * t + q
            wait("scalar", (s_kpe.h, gq + 1))
            if t >= 2 and q == 0:
                wait("scalar", (s_kmax.h, t - 1))
            sm = s_kact.inc(1)
            emit("scalar", lambda e, t=t, q=q, gq=gq, sm=sm: e.activation(
                nd2_s[t % 2][:, q * 512:(q + 1) * 512], pz[gq % 3][:, 0:512],
                ACTF.Identity, bias=negf2_s[:, t:t + 1], scale=1.0
            ).then_inc(sm[0], 1))
        wait("vector", (s_kact.h, NQ * (t + 1)))
        emit("vector", lambda e, t=t: e.max(bm_s[:, :, t], nd2_s[t % 2][:]))
        emit("vector", lambda e: e.drain())
        emit("vector", lambda e, t=t: e.max_index(
            bi_s[:, :, t], bm_s[:, :, t], nd2_s[t % 2][:]))
        sm = s_kmax.inc(1)
        emit("vector", lambda e, sm=sm: e.drain().then_inc(sm[0], 1))

    checkpoint()   # 1: knn select
    # weights on DVE
    emit("vector", lambda e: e.tensor_scalar(
        out=d2c_s[:], in0=bm_s[:, 0:3, :], scalar1=-1.0, scalar2=1e-16,
        op0=ALU.mult, op1=ALU.max))
    emit("vector", lambda e: e.drain())
    emit("vector", lambda e: e.reciprocal(w_s[:], d2c_s[:]))
    emit("vector", lambda e: e.drain())
    emit("vector", lambda e: e.tensor_reduce(
        out=wsum_s[:], in_=bass.AP(w_s, 0, [[3 * NTH, 128], [1, NTH], [NTH, 3]]),
        axis=mybir.AxisListType.X, op=ALU.add))
    emit("vector", lambda e: e.drain())
    emit("vector", lambda e: e.reciprocal(rs_s[:], wsum_s[:]))
    emit("vector", lambda e: e.drain())
    emit("vector", lambda e: e.tensor_tensor(
        out=wnb_s[:], in0=w_s[:],
        in1=bass.AP(rs_s, 0, [[NTH, 128], [0, 3], [1, NTH]]),
        op=ALU.mult))
    sm = s_wn.inc(1)
    emit("vector", lambda e, sm=sm: e.drain().then_inc(sm[0], 1))

    # wrapped idx build (gpsimd)
    wait("gpsimd", (s_kmax.h, NTH))
    for k in range(3):
        for g in range(8):
            sm = s_wrap.inc(16)
            emit("gpsimd", lambda e, k=k, g=g, sm=sm: e.dma_start(
                out=wrap_s[0:16, k, :, g],
                in_=bi_s[16 * g:16 * (g + 1), k, :],
            ).then_inc(sm[0], 16))
    wait("gpsimd", s_wrap.now())
    for rep in range(1, 8):
        sm = s_wrap.inc(16)
        emit("gpsimd", lambda e, rep=rep, sm=sm: e.dma_start(
            out=wrap_s[16 * rep:16 * (rep + 1)],
            in_=wrap_s[0:16],
        ).then_inc(sm[0], 16))
    WRAP_ALL = s_wrap.now()

    # interp
    wait("gpsimd", WRAP_ALL)
    wait("vector", s_wn.now())
    n_ich = (NTH + ICH - 1) // ICH
    for c in range(n_ich):
        t0 = c * ICH
        ntile = min(ICH, NTH - t0)
        for k in range(3):
            sm = r_gk.write(c)
            smp = s_iprep.inc(1)
            emit("gpsimd", lambda e, k=k, c=c, t0=t0, nt=ntile, sm=sm, smp=smp:
                 e.dma_gather(
                     out_ap=gk_s[k][c % 2][:, 0:nt, :],
                     in_ap=Din["ctab"][:],
                     idxs_ap=wrap_s[:, k, t0:t0 + nt, :].bitcast(i16),
                     num_idxs=nt * 128, num_idxs_reg=nt * 128,
                     elem_size=128,
                     prepare_only=True, sem=r_gk.sems[c % 2].h,
                     queue_num=c % 2,
                 ).then_inc(smp[0], 1))
        wait("gpsimd", s_iprep.now())
        if c >= 2:
            wait("gpsimd", (s_ipe.h, (c - 1) * ICH))
        emit("gpsimd", lambda e, c=c: e.trigger_dma(
            count=3, queue_num=c % 2))
        GK_NOW = r_gk.last(c)
        for tt in range(ntile):
            t = t0 + tt
            if t >= 2:
                wait("vector", (s_ipe.h, t - 1))
            emit("vector", lambda e, t=t: e.tensor_tensor(
                out=diag3_s[t % 2][:],
                in0=bass.AP(identf_s, 0, [[128, 128], [0, 3], [1, 128]]),
                in1=bass.AP(wnb_s, t, [[3 * NTH, 128], [NTH, 3], [0, 128]]),
                op=ALU.mult))
            sm = s_dg.inc(1)
            emit("vector", lambda e, sm=sm: e.drain().then_inc(sm[0], 1))
            wait("tensor", GK_NOW)
            wait("tensor", (s_dg.h, s_dg.n))
            if t >= 4:
                wait("tensor", (s_fy.h, t - 3))    # psum WAR
            for k in range(3):
                sm = s_ipe.inc(1) if k == 2 else None
                def mk_interp(t=t, tt=tt, k=k, c=c, sm=sm):
                    def f(e):
                        ins = e.matmul(
                            pa[t % 4][:, 0:128], gk_s[k][c % 2][:, tt, :],
                            diag3_s[t % 2][:, k, :],
                            start=(k == 0), stop=(k == 2))
                        if sm:
                            ins.then_inc(sm[0], 1)
                    return f
                emit("tensor", mk_interp())
            wait("scalar", (s_ipe.h, s_ipe.n))
            wait("scalar", r_fyw.last(t))
            sm = s_fy.inc(1)
            emit("scalar", lambda e, t=t, sm=sm: e.activation(
                fyw_s[t % 2][:], pa[t % 4][0:3, 0:128],
                ACTF.Copy, bias=0.0, scale=1.0).then_inc(sm[0], 1))
            sm = r_fyw.write(t)
            emit("scalar", lambda e, t=t, sm=sm: e.dma_start(
                out=fy_d[:, t * 128:(t + 1) * 128],
                in_=fyw_s[t % 2][:]).then_inc(sm[0], 16))
    FY_ALL = s_fy.now()
    KACT_ALL = s_kact.now()
    checkpoint()   # 2: interp

    # ============ GCN layers ============
    WCH = {1: W1_s, 2: W2_s, 3: W3a_s, 4: W4_s, 5: W5_s}
    layer_state = []   # per layer: rg/rgp/cc snapshots + h-write marks

    for li, L in enumerate(LAYERS):
        width = 128 if L["e2"] else H
        ghalf = g2_half if L["e2"] else g_half[li % 2]
        gfull = g2_full if L["e2"] else g_full[li % 2]
        war = layer_state[li - 2] if (li >= 2 and not L["e2"]) else None
        prev = layer_state[li - 1] if li >= 1 else None

        # ---------- dense (own half) ----------
        zpe_base = s_zpe.n
        zact_base = s_zact.n
        gw_marks = []

        def dense_epilogue(t, width=width, ghalf=ghalf, war=war):
            wait("vector", (s_zpe.h, zpe_base + t + 1))
            wait("vector", r_gw.last(t))
            sm = s_zact.inc(1)
            emit("vector", lambda e, t=t, w=width: e.tensor_scalar(
                out=gsb_s[t % 4][:, 0:w], in0=pz[t % 3][:, 0:w],
                scalar1=1.0, scalar2=None, op0=ALU.mult))
            emit("vector", lambda e, sm=sm: e.drain().then_inc(sm[0], 1))
            wait("scalar", (s_zact.h, s_zact.n))
            if t == 0 and war is not None:
                wait("scalar", war["cc"])         # WAR vs AG reads (li-2)
                for sv in war["rg"]:              # WAR vs own gathers (li-2)
                    wait("scalar", sv)
            sm = r_gw.write(t)
            emit("scalar", lambda e, t=t, gd=ghalf, w=width, sm=sm: e.dma_start(
                out=gd[t * 128:(t + 1) * 128, :],
                in_=gsb_s[t % 4][:, 0:w]).then_inc(sm[0], 16))
            gw_marks.append(r_gw.all())

        if li == 0:
            wait("tensor", KACT_ALL)      # pz WAR vs KNN ACT
            for t in range(NTH):
                if t >= 2:
                    wait("sync", (s_zpe.h, zpe_base + t - 1))
                sm = r_h0.write(t)
                emit("sync", lambda e, t=t, sm=sm: e.dma_start(
                    out=h0_s[t % 2][:], in_=Din["h0T"][:, t * 128:(t + 1) * 128]
                ).then_inc(sm[0], 16))
                wait("tensor", r_h0.last(t))
                wait("tensor", (s_zact.h,
                                zact_base if t < 3 else zact_base + t - 2))
                sm = s_zpe.inc(1)
                emit("tensor", lambda e, t=t, sm=sm: e.matmul(
                    pz[t % 3][:, 0:H], h0_s[t % 2][:],
                    W0_s[:], start=True, stop=True).then_inc(sm[0], 1))
                dense_epilogue(t)
        else:
            Wl = WCH[li]
            range_zpe = []
            for ri, (rt0, rnt) in enumerate(_ranges()):
                wait("sync", (s_zpe.h,
                              zpe_base if ri < 2 else range_zpe[ri - 2]))
                for sv in prev["hmarks"][rt0 + rnt - 1]:   # h tiles ready
                    wait("sync", sv)
                for cch in range(4):
                    sm = r_hT.write(ri)
                    emit("sync", lambda e, ri=ri, rt0=rt0, rnt=rnt, c=cch, sm=sm:
                         e.dma_start_transpose(
                             hT_s[ri % 2][:, c, 0:rnt * 128],
                             h_d[rt0 * 128:(rt0 + rnt) * 128,
                                 c * 128:(c + 1) * 128],
                         ).then_inc(sm[0], 16))
                wait("tensor", r_hT.last(ri))
                if li == 3 and ri == 0:
                    wait_all("sync", r_fyw)
                for tt in range(rnt):
                    t = rt0 + tt
                    range_last = (tt == rnt - 1)
                    if L["fy"]:
                        if t >= 2:
                            wait("sync", (s_zpe.h, zpe_base + t - 1))
                        sm = r_fyr.write(t)
                        emit("sync", lambda e, t=t, sm=sm: e.dma_start(
                            out=fyr_s[t % 2][:],
                            in_=fy_d[:, t * 128:(t + 1) * 128]
                        ).then_inc(sm[0], 16))
                    wait("tensor", (s_zact.h,
                                    zact_base if t < 3 else zact_base + t - 2))
                    for cch in range(4):
                        last = (cch == 3) and not L["fy"]
                        sm = s_zpe.inc(1) if last else None
                        def mk_dense(t=t, tt=tt, ri=ri, cch=cch, Wl=Wl,
                                     w=width, last=last, sm=sm):
                            def f(e):
                                ins = e.matmul(
                                    pz[t % 3][:, 0:w],
                                    hT_s[ri % 2][:, cch,
                                                 tt * 128:(tt + 1) * 128],
                                    Wl[:, cch, 0:w],
                                    start=(cch == 0), stop=last)
                                if sm:
                                    ins.then_inc(sm[0], 1)
                            return f
                        emit("tensor", mk_dense())
                    if L["fy"]:
                        wait("tensor", r_fyr.last(t))
                        sm = s_zpe.inc(1)
                        emit("tensor", lambda e, t=t, sm=sm: e.matmul(
                            pz[t % 3][:, 0:H],
                            fyr_s[t % 2][:],
                            W3b_s[:], start=False, stop=True).then_inc(sm[0], 1))
                    if range_last:
                        range_zpe.append(s_zpe.n)
                    dense_epilogue(t)
        checkpoint()   # dense of this layer done

        # ---------- AllGather g_half -> g_full (2 chunks) ----------
        for ci, (c0, c1) in enumerate([(0, CHK), (CHK, NTH)]):
            for sv in gw_marks[c1 - 1]:
                wait("gpsimd", sv)
            if ci == 0 and war is not None:
                for sv in war["rgp"]:     # WAR vs peer gathers (li-2)
                    wait("gpsimd", sv)
            rows = (c1 - c0) * 128
            sm = s_cc.inc(1)
            emit("gpsimd", lambda e, hh=ghalf, ff=gfull, c0=c0, rows=rows,
                 w=width, sm=sm: e.collective_compute(
                     "AllGather",
                     ALU.bypass,
                     replica_groups=RG_PAIRS,
                     ins=[bass.AP(hh, c0 * 128 * w,
                                  [[w, rows], [1, w]]).opt()],
                     outs=[bass.AP(ff, 2 * c0 * 128 * w,
                                   [[w, 2 * rows], [1, w]]).opt()],
                 ).then_inc(sm[0], 1))
        CC_NOW = s_cc.now()

        # ---------- agg (own dest tiles) ----------
        gbufs = gath2_s if L["e2"] else gath_s
        ape_base = s_ape.n
        aact_base = s_aact.n
        gprep_base = s_gprep.n
        hmarks = []
        if li == 0:
            wait("tensor", (s_fy.h, NTH))   # pa WAR vs interp
        for t in range(NTH):
            po, pp, nsl = int(Po[t]), int(Pp[t]), int(P[t])
            # --- combined S+idx load (sync) ---
            wait("sync", (s_gprep.h,
                          gprep_base if t < GB
                          else gprep_base + 2 * (t - GB) + 2))
            wait("sync", (s_ape.h,
                          ape_base if t < GB else ape_base + t - GB + 1))
            sm = r_sg.write(t)
            emit("sync", lambda e, t=t, nsl=nsl, sm=sm: e.dma_start(
                out=sgt_s[t % GB][:, 0:nsl * 136],
                in_=Din["sgt"][:, SOFFC[t]:SOFFC[t] + nsl * 136]
            ).then_inc(sm[0], 16))
            # --- gather preps (gpsimd): own half + peer half ---
            wait("gpsimd", r_sg.last(t))
            smo = r_g.write(t)
            smp = s_gprep.inc(1)
            emit("gpsimd", lambda e, t=t, po=po, nsl=nsl, gh=ghalf, gb=gbufs,
                 w=width, smp=smp: e.dma_gather(
                     out_ap=gb[t % GB][:, 0:po, 0:w],
                     in_ap=gh[:],
                     idxs_ap=sgt_s[t % GB][:, nsl * 128:
                                           nsl * 128 + po * 8].bitcast(i16),
                     num_idxs=po * 128, num_idxs_reg=po * 128, elem_size=w,
                     single_packet=False,
                     prepare_only=True, sem=r_g.sems[t % GB].h,
                     queue_num=t % 2,
                 ).then_inc(smp[0], 1))
            smq = r_gp.write(t)
            smp = s_gprep.inc(1)
            emit("gpsimd", lambda e, t=t, po=po, pp=pp, nsl=nsl, gf=gfull,
                 gb=gbufs, w=width, smp=smp: e.dma_gather(
                     out_ap=gb[t % GB][:, po:po + pp, 0:w],
                     in_ap=gf[:],
                     idxs_ap=sgt_s[t % GB][:, nsl * 128 + po * 8:
                                           nsl * 136].bitcast(i16),
                     num_idxs=pp * 128, num_idxs_reg=pp * 128, elem_size=w,
                     single_packet=False,
                     prepare_only=True, sem=r_gp.sems[t % GB].h,
                     queue_num=2 + t % 2,
                 ).then_inc(smp[0], 1))
            wait("gpsimd", s_gprep.now())
            # own trigger: needs gath buf free + ALL own dense writes
            wait("gpsimd", (s_ape.h,
                            ape_base if t < GB else ape_base + t - GB + 1))
            if t == 0:
                for sv in gw_marks[NTH - 1]:
                    wait("gpsimd", sv)
            emit("gpsimd", lambda e, t=t: e.trigger_dma(
                count=1, queue_num=t % 2))
            # peer trigger: additionally needs the AllGather
            if t == 0:
                wait("gpsimd", CC_NOW)
            emit("gpsimd", lambda e, t=t: e.trigger_dma(
                count=1, queue_num=2 + t % 2))
            # --- matmuls (tensor): own slices, then peer, then bias ---
            wait("tensor", r_g.last(t))
            wait("tensor", (s_aact.h,
                            aact_base if t < 4 else aact_base + t - 3))
            for sl in range(po):
                emit("tensor", lambda e, t=t, sl=sl, gb=gbufs, w=width:
                     e.matmul(
                         pa[t % 4][:, 0:w],
                         sgt_s[t % GB][:, sl * 128:(sl + 1) * 128],
                         gb[t % GB][:, sl, 0:w],
                         start=(sl == 0), stop=False))
            wait("tensor", r_gp.last(t))
            for sl in range(po, nsl):
                emit("tensor", lambda e, t=t, sl=sl, gb=gbufs, w=width:
                     e.matmul(
                         pa[t % 4][:, 0:w],
                         sgt_s[t % GB][:, sl * 128:(sl + 1) * 128],
                         gb[t % GB][:, sl, 0:w],
                         start=False, stop=False))
            sm = s_ape.inc(1)
            emit("tensor", lambda e, t=t, li=li, w=width, sm=sm: e.matmul(
                pa[t % 4][:, 0:w], ones1_s[:],
                brows_s[:, li, 0:w], start=False, stop=True
            ).then_inc(sm[0], 1))
            # --- epilogue (scalar + sync) ---
            wait("scalar", (s_ape.h, s_ape.n))
            if L["e2"]:
                wait("scalar", r_ow.last(t))
            else:
                wait("scalar", r_hw.last(t))
            sm = s_aact.inc(1)
            if L["e2"]:
                emit("scalar", lambda e, t=t, sm=sm: e.activation(
                    osb_s[t % 2][:], pa[t % 4][:, 0:128], ACTF.Copy,
                    bias=0.0, scale=1.0).then_inc(sm[0], 1))
            else:
                emit("scalar", lambda e, t=t, sm=sm: e.activation(
                    hsb_s[t % 4][:], pa[t % 4][:, 0:H], ACTF.Relu,
                    bias=0.0, scale=1.0).then_inc(sm[0], 1))
            if L["e2"]:
                sm = r_ow.write(t)
                emit("scalar", lambda e, t=t, sm=sm: e.dma_start(
                    out=Din["out_nm"][t * 128:(t + 1) * 128, :],
                    in_=osb_s[t % 2][:]).then_inc(sm[0], 16))
            else:
                sm = r_hw.write(t)
                emit("scalar", lambda e, t=t, sm=sm: e.dma_start(
                    out=h_d[t * 128:(t + 1) * 128, :],
                    in_=hsb_s[t % 4][:]).then_inc(sm[0], 16))
            hmarks.append(r_hw.all())
        layer_state.append(dict(rg=r_g.all(), rgp=r_gp.all(),
                                cc=s_cc.now(), hmarks=hmarks))
        checkpoint()   # 3+li

    wait_all("sync", r_ow)
    wait_all("sync", r_hw)
    checkpoint()
    if PHASE < len(checkpoints):
        cut = checkpoints[PHASE]
        for e in Q:
            Q[e] = Q[e][:cut[e]]

    with nc.allow_non_contiguous_dma(reason="wrapped idx build"), \
            nc.Block() as block:
        @block.sync
        def _(e):
            for fn in Q["sync"]:
                fn(e)

        @block.tensor
        def _(e):
            for fn in Q["tensor"]:
                fn(e)

        @block.vector
        def _(e):
            for fn in Q["vector"]:
                fn(e)

        @block.scalar
        def _(e):
            for fn in Q["scalar"]:
                fn(e)

        @block.gpsimd
        def _(e):
            for fn in Q["gpsimd"]:
                fn(e)

    nc.finalize()
    return nc


# ================= host side =================

def host_prep(inputs):
    x = np.asarray(inputs["x"], np.float32)
    sdf = np.asarray(inputs["sdf"], np.float32)
    edge_index = np.asarray(inputs["edge_index"], np.int64)
    coarse_x = np.asarray(inputs["coarse_x"], np.float32)
    coarse_y = np.asarray(inputs["coarse_y"], np.float32)
    Ws = {k: np.asarray(inputs[k], np.float32) for k in (
        "pre_W0", "pre_W1", "pre_W2", "end_W0", "end_W1", "end_W2")}
    bs = {k: np.asarray(inputs[k], np.float32) for k in (
        "pre_b0", "pre_b1", "pre_b2", "end_b0", "end_b1", "end_b2")}

    cxT3 = np.zeros((3, NCPAD), np.float32)
    cxT3[0, :NC] = 2 * coarse_x[:, 0]
    cxT3[1, :NC] = 2 * coarse_x[:, 1]
    cxT3[2, :NC] = -(coarse_x[:, 0] ** 2 + coarse_x[:, 1] ** 2)
    cxT3[0, NC:] = 2e4; cxT3[1, NC:] = 2e4; cxT3[2, NC:] = -2e8

    brows = np.zeros((6, H), np.float32)
    for i, k in enumerate(("pre_b0", "pre_b1", "pre_b2", "end_b0", "end_b1")):
        brows[i] = bs[k]
    brows[5, :OUT] = bs["end_b2"]

    W5 = np.zeros((H, 128), np.float32)
    W5[:, :OUT] = Ws["end_W2"]

    def pmaj(w):   # [512, X] -> [128, 4, X]
        return np.ascontiguousarray(
            w.reshape(4, 128, w.shape[1]).transpose(1, 0, 2))

    common = dict(
        cxT3=cxT3,
        W0=Ws["pre_W0"].astype(bfnp),
        W1=pmaj(Ws["pre_W1"]).astype(bfnp),
        W2=pmaj(Ws["pre_W2"]).astype(bfnp),
        W3a=pmaj(Ws["end_W0"][OUT:]).astype(bfnp),
        W3b=Ws["end_W0"][:OUT].astype(bfnp),
        W4=pmaj(Ws["end_W1"]).astype(bfnp),
        W5=pmaj(W5).astype(bfnp),
        brows=brows.astype(bfnp)[None],
        ones1=np.ones((1, 128), bfnp),
        identf=np.eye(128, dtype=np.float32),
    )

    # ---- pass 1: per-sample node->slot assignment + per-core tile stats ----
    samples = []
    for s in range(B):
        xs = x[s * NF:(s + 1) * NF]
        e = edge_index[:, s * E_PER:(s + 1) * E_PER] - s * NF
        cy = coarse_y[s * NC:(s + 1) * NC]

        deg = np.bincount(e[1], minlength=NF).astype(np.float32) + 1.0
        dinv = (1.0 / np.sqrt(deg)).astype(np.float32)

        # balanced global tile assignment (snake over degree-sorted nodes)
        order = np.argsort(-deg, kind="stable")
        tile_seq = np.arange(NT)
        snake = np.concatenate([tile_seq, tile_seq[::-1]])
        bins = np.resize(snake, NF)
        gtile = np.empty(NF, np.int64)   # node -> global tile
        lane = np.empty(NF, np.int64)
        for t in range(NT):
            sel = np.where(bins == t)[0]
            gtile[order[sel]] = t
            lane[order[sel]] = np.arange(len(sel))

        # self loops ride in the own-source group
        e_aug = np.concatenate([e, np.stack([np.arange(NF)] * 2)], axis=1)

        # in-edge count per global tile (incl self loops)
        cin = np.bincount(gtile[e_aug[1]], minlength=NT)

        # per half: order local tiles by in-edge count desc
        half = (gtile >= NTH).astype(np.int64)
        ltile = np.empty(NF, np.int64)
        sco = np.zeros((2, NTH), np.int64)   # own-source slices per tile
        scp = np.zeros((2, NTH), np.int64)   # peer-source slices per tile
        for p in range(2):
            gts = np.arange(p * NTH, (p + 1) * NTH)
            perm = gts[np.argsort(-cin[gts], kind="stable")]
            inv = np.empty(NTH, np.int64)
            inv[perm - p * NTH] = np.arange(NTH)
            mask = half == p
            ltile[mask] = inv[gtile[mask] - p * NTH]
        src_half = half[e_aug[0]]
        dst_half = half[e_aug[1]]
        for p in range(2):
            emask = dst_half == p
            dt_ = ltile[e_aug[1][emask]]
            own = src_half[emask] == p
            sco[p] = np.bincount(dt_[own], minlength=NTH)   # own counts
            scp[p] = np.bincount(dt_, minlength=NTH)        # total counts

        # node -> row in g_full (chunk-major AllGather layout: the 2-chunk
        # AG writes [even c0..CHK | odd c0..CHK | even CHK.. | odd CHK..])
        nidg = np.where(
            ltile < CHK,
            half * (CHK * 128) + ltile * 128 + lane,
            2 * CHK * 128 + half * ((NTH - CHK) * 128)
            + (ltile - CHK) * 128 + lane)
        # node -> row in concat(even out_nm, odd out_nm) (output assembly)
        nidl = half * NPADH + ltile * 128 + lane
        samples.append(dict(xs=xs, e=e_aug, cy=cy, dinv=dinv, half=half,
                            ltile=ltile, lane=lane, nidg=nidg, nidl=nidl,
                            sco=sco, scp=scp))

    # profiles: own group sized to the min own count over cores (zero own
    # padding; overflow spills into the peer group which reads g_full)
    own_min = np.full(NTH, 1 << 30, np.int64)
    tot_max = np.zeros(NTH, np.int64)
    for sm in samples:
        own_min = np.minimum(own_min, sm["sco"].min(axis=0))
        tot_max = np.maximum(tot_max, sm["scp"].max(axis=0))
    Po = np.maximum(own_min // 128, 1)
    Pp = np.maximum(np.ceil((tot_max - Po * 128) / 128).astype(np.int64), 1)
    P = Po + Pp
    assert P.max() <= 12, f"slice overflow {P.max()}"
    SOFF = np.concatenate([[0], np.cumsum(P)]).astype(int)
    SLOT_TOT = int(SOFF[-1]) * 128
    SOFFC = (SOFF * 136).astype(int)

    # ---- pass 2: per-core arrays ----
    in_maps, metas = [], []
    for s in range(B):
        smp_ = samples[s]
        xs, e, cy = smp_["xs"], smp_["e"], smp_["cy"]
        dinv, half, ltile, lane, nidg = (
            smp_["dinv"], smp_["half"], smp_["ltile"], smp_["lane"],
            smp_["nidg"])

        ctab = np.zeros((NCPAD, 128), np.float32)
        ctab[:NC, 0:OUT] = cy
        ctab = ctab.astype(bfnp)

        for p in range(2):
            own = half == p
            lrow = ltile * 128 + lane          # local row id (own nodes)

            grow = np.zeros(SLOT_TOT, np.int16)
            sT = np.zeros((SLOT_TOT, 128), np.float32)
            # two slot groups per tile: first Po[t]*128 own-source edges
            # (local g_half rows, incl self loops) at SOFF[t]*128; all
            # remaining edges (own overflow + peer sources, g_full rows)
            # at (SOFF[t]+Po[t])*128
            emask = half[e[1]] == p
            e_src, e_dst = e[0][emask], e[1][emask]
            is_own = half[e_src] == p
            ecol_t = ltile[e_dst]
            # order: per tile, own-source edges first
            o = np.lexsort((~is_own, ecol_t))
            e_src, e_dst, is_own = e_src[o], e_dst[o], is_own[o]
            ecol_t = ecol_t[o]
            ecol_l = lrow[e_dst] % 128
            ewt = dinv[e_src] * dinv[e_dst]
            tstart = np.searchsorted(ecol_t, np.arange(NTH))
            cnts = (np.searchsorted(ecol_t, np.arange(NTH), side="right")
                    - tstart)
            assert (cnts <= P * 128).all(), "profile overflow"
            rank = np.arange(len(ecol_t)) - np.repeat(tstart, cnts)
            in_own = rank < np.repeat(Po * 128, cnts)
            # own group must contain only own-source edges
            assert not (in_own & ~is_own).any(), "own group underfilled"
            slot = (SOFF[ecol_t] * 128 + rank).astype(np.int64)
            erow = np.where(in_own, lrow[e_src], nidg[e_src])
            grow[slot] = erow.astype(np.int16)
            sT[slot, ecol_l] = ewt

            # combined per-tile [S p-major | idx] tensor
            tmp = np.ascontiguousarray(grow.reshape(SLOT_TOT // 16, 16).T)
            growc = np.ascontiguousarray(np.tile(tmp, (8, 1)))  # [128, S/16]
            sgt = np.zeros((128, int(SOFFC[-1])), bfnp)
            for t in range(NTH):
                nsl = int(P[t]); base = int(SOFFC[t])
                blk = sT[SOFF[t] * 128:(SOFF[t] + nsl) * 128]
                pm = blk.reshape(nsl, 128, 128).transpose(1, 0, 2)
                sgt[:, base:base + nsl * 128] = (
                    pm.reshape(128, nsl * 128).astype(bfnp))
                gb = np.ascontiguousarray(
                    growc[:, SOFF[t] * 8:(SOFF[t] + nsl) * 8])
                sgt[:, base + nsl * 128:base + nsl * 136] = gb.view(bfnp)

            # node features / positions at local slots
            f01 = np.full((NPADH, 2), 1e3, np.float32)
            f01[lrow[own]] = xs[own][:, 0:2]
            xT3 = np.ones((3, NPADH), np.float32)
            xT3[0] = f01[:, 0]; xT3[1] = f01[:, 1]
            negf2 = np.ascontiguousarray(
                (-(f01[:, 0] ** 2 + f01[:, 1] ** 2)).reshape(NTH, 128).T)

            h0 = np.zeros((NPADH, 6), np.float32)
            h0[lrow[own], 0:D_IN] = xs[own]
            h0[lrow[own], D_IN] = sdf[own, 0]
            h0T = np.ascontiguousarray(h0.T).astype(bfnp)

            m = dict(common)
            m.update(xT3=xT3, negf2=negf2, h0T=h0T, sgt=sgt, ctab=ctab)
            in_maps.append(m)
        metas.append(smp_["nidl"])

    return in_maps, metas, (tuple(Po.tolist()), tuple(Pp.tolist()))


_prog_cache = {}


def kernel(**inputs):
    in_maps, metas, P = host_prep(inputs)
    if _prog_cache.get("P") != P:
        _prog_cache["nc"] = build_program(np.array(P[0]), np.array(P[1]))
        _prog_cache["P"] = P
    nc = _prog_cache["nc"]

    res = run_bass_kernel_spmd(nc, in_maps, list(range(N_CORES)))
    global _last_exec_ns, _last_trace
    _last_exec_ns = res.exec_time_ns
    _last_trace = res.instructions_and_trace

    out = np.empty((B * NF, OUT), np.float32)
    for s in range(B):
        full = np.concatenate([
            np.asarray(res.results[2 * s]["out_nm"]),
            np.asarray(res.results[2 * s + 1]["out_nm"]),
        ], axis=0)
        out[s * NF:(s + 1) * NF] = full[metas[s], 0:OUT]
    return out
